# revision 20
# baseline (speedup 1.0000x reference)
"""Trainium2 Bass kernel for the AttentionBlock problem.

Sharding (8 cores): core = 4*b + qi  (b = batch, qi = query-quarter).
Each core:
  - GroupNorm(8, C) over its batch's full (C=256, N=4096) activations
  - K/V projections for all 4096 tokens (duplicated per batch pair of cores)
  - Q projection for its 1024 queries
  - attention (4 heads) for its 1024 queries against all 4096 keys
  - output projection + bias + residual for its disjoint (256, 1024) slice
Host unshard = pure concatenation of the 8 disjoint output slices.

Softmax uses a constant shift M0 (softmax is invariant to per-row constant
shifts; a global constant is exact in exact arithmetic and fp32-safe here:
scaled scores lie in [-16.5, 13.3] for any plausible input scale, and the
shifted exponentials stay well inside fp32/bf16 range). Row-sums are
obtained by appending a ones-column to V so they fall out of the
attention-value matmul; normalization and the V bias are applied after it.
"""

import os
import sys

# The grading environment may pin JAX_PLATFORMS=cpu for the reference; the
# bass execution path needs the axon/neuron PJRT devices.
if os.environ.get("JAX_PLATFORMS", "").strip() == "cpu":
    del os.environ["JAX_PLATFORMS"]

for _p in ("/opt/trn_rl_repo",):
    if os.path.isdir(_p) and _p not in sys.path:
        sys.path.insert(0, _p)

import numpy as np

B = 2
C = 256
N = 4096
NQ = 1024  # queries per core
NH = 4
HD = 64
G = 8
EPS = 1e-5
SCALE = HD ** -0.5
M0 = 16.0  # constant softmax shift (in scaled-score units)
N_CORES = 8

_CACHE: dict = {}


def _build(debug_taps=False):
    from contextlib import ExitStack

    import concourse.bass as bass
    import concourse.tile as tile
    from concourse import bacc, mybir

    f32 = mybir.dt.float32
    f32r = mybir.dt.float32r
    A = mybir.AluOpType
    AF = mybir.ActivationFunctionType

    nc = bacc.Bacc("TRN2", target_bir_lowering=False, debug=False,
                   num_devices=N_CORES)

    d_xf = nc.dram_tensor("x_full", [C, N], f32r, kind="ExternalInput").ap()
    d_xq = nc.dram_tensor("x_q", [C, NQ], f32, kind="ExternalInput").ap()
    d_wq = nc.dram_tensor("wq_t", [C, C], f32r, kind="ExternalInput").ap()
    d_wk = nc.dram_tensor("wk_t", [C, C], f32r, kind="ExternalInput").ap()
    d_wv = nc.dram_tensor("wv_t", [C, C], f32r, kind="ExternalInput").ap()
    d_wp = nc.dram_tensor("wp_t", [C, C], f32r, kind="ExternalInput").ap()
    d_bq = nc.dram_tensor("bq", [128, 2], f32, kind="ExternalInput").ap()
    d_bk = nc.dram_tensor("bk", [128, 2], f32, kind="ExternalInput").ap()
    d_bv = nc.dram_tensor("bv", [128, 2], f32, kind="ExternalInput").ap()
    d_nw = nc.dram_tensor("nw", [128, 2], f32, kind="ExternalInput").ap()
    d_nb = nc.dram_tensor("nb", [128, 2], f32, kind="ExternalInput").ap()
    d_pb = nc.dram_tensor("pb", [128, 2], f32, kind="ExternalInput").ap()
    d_gm = nc.dram_tensor("gmask", [128, 16], f32, kind="ExternalInput").ap()
    d_gmt = nc.dram_tensor("gmask_t", [8, C], f32, kind="ExternalInput").ap()
    d_out = nc.dram_tensor("out", [C, NQ], f32, kind="ExternalOutput").ap()
    dbg = {}
    if debug_taps:
        dbg["xn0"] = nc.dram_tensor("dbg_xn0", [128, N], f32, kind="ExternalOutput").ap()
        dbg["K0"] = nc.dram_tensor("dbg_K0", [128, N], f32, kind="ExternalOutput").ap()
        dbg["Q0"] = nc.dram_tensor("dbg_Q0", [128, NQ], f32, kind="ExternalOutput").ap()
        dbg["vt"] = nc.dram_tensor("dbg_vt", [128, 32, NH, HD + 1], f32, kind="ExternalOutput").ap()
        dbg["at00"] = nc.dram_tensor("dbg_at00", [128, 2048], f32, kind="ExternalOutput").ap()
        dbg["hA0"] = nc.dram_tensor("dbg_hA0", [65, NQ], f32, kind="ExternalOutput").ap()
        dbg["rsA0"] = nc.dram_tensor("dbg_rsA0", [1, NQ], f32, kind="ExternalOutput").ap()
        dbg["hn0"] = nc.dram_tensor("dbg_hn0", [128, NQ], f32, kind="ExternalOutput").ap()
        dbg["rb0"] = nc.dram_tensor("dbg_rb0", [128, NQ], f32, kind="ExternalOutput").ap()

    def body(ctx: ExitStack, tc: tile.TileContext):
        sing = ctx.enter_context(tc.tile_pool(name="sing", bufs=1))
        wk = ctx.enter_context(tc.tile_pool(name="wk", bufs=2))

        # ---------------- loads ----------------
        xf = []
        xq = []
        for h in range(2):
            t = sing.tile([128, N], f32r, tag=f"xf{h}", name=f"xf{h}")
            nc.sync.dma_start(out=t, in_=d_xf[h * 128:(h + 1) * 128, :])
            xf.append(t)
            t = sing.tile([128, NQ], f32, tag=f"xq{h}", name=f"xq{h}")
            nc.sync.dma_start(out=t, in_=d_xq[h * 128:(h + 1) * 128, :])
            xq.append(t)

        def load_w(name, dram):
            t = sing.tile([128, 2, C], f32r, tag=name, name=name)
            nc.sync.dma_start(out=t, in_=dram.rearrange("(c p) o -> p c o", p=128))
            return t

        wq_sb = load_w("wq_sb", d_wq)
        wk_sb = load_w("wk_sb", d_wk)
        wv_sb = load_w("wv_sb", d_wv)
        wp_sb = load_w("wp_sb", d_wp)

        def load_small(name, dram, shape):
            t = sing.tile(shape, f32, tag=name, name=name)
            nc.sync.dma_start(out=t, in_=dram)
            return t

        bq_sb = load_small("bq_sb", d_bq, [128, 2])
        bk_sb = load_small("bk_sb", d_bk, [128, 2])
        bv_sb = load_small("bv_sb", d_bv, [128, 2])
        nw_sb = load_small("nw_sb", d_nw, [128, 2])
        nb_sb = load_small("nb_sb", d_nb, [128, 2])
        pb_sb = load_small("pb_sb", d_pb, [128, 2])
        gm_sb = load_small("gm_sb", d_gm, [128, 16])
        gmt_sb = load_small("gmt_sb", d_gmt, [8, C])

        # V^T tiles, per-head with an appended ones column for row-sums:
        # vt[:, tt, h, 0:64] = V^T, vt[:, tt, h, 64] = 1.0
        vt = sing.tile([128, 32, NH, HD + 1], f32r, tag="vt", name="vt")
        nc.vector.memset(vt[:, :, :, HD:HD + 1].bitcast(f32), 1.0)

        # activation bias constants (per-partition APs)
        epsc = sing.tile([128, 1], f32, tag="epsc", name="epsc")
        nc.vector.memset(epsc, EPS)
        m0c = sing.tile([128, 1], f32, tag="m0c", name="m0c")
        nc.vector.memset(m0c, -M0)
        ones1 = sing.tile([1, 64], f32, tag="ones1", name="ones1")
        nc.vector.memset(ones1, 1.0)

        K_sb = [sing.tile([128, N], f32r, tag=f"K{hp}", name=f"K{hp}")
                for hp in range(2)]
        Q_sb = [sing.tile([128, NQ], f32r, tag=f"Qs{hp}", name=f"Qs{hp}")
                for hp in range(2)]
        hnT = [sing.tile([128, NQ], f32r, tag=f"hn{hp}", name=f"hn{hp}")
               for hp in range(2)]

        # ---------------- groupnorm + projections ----------------
        with tc.tile_pool(name="ps1", bufs=1, space="PSUM") as ps1:
            # per-channel stats over the 4096 tokens
            st_t = []
            for h in range(2):
                stats = wk.tile([128, 8, 6], f32, tag="stats", name=f"stats{h}")
                for sg in range(8):
                    nc.vector.bn_stats(stats[:, sg, :],
                                       xf[h][:, sg * 512:(sg + 1) * 512])
                mv = wk.tile([128, 2], f32, tag="mv", name=f"mv{h}")
                nc.vector.bn_aggr(mv, stats)
                st = wk.tile([128, 2], f32, tag="st", name=f"st{h}")
                nc.vector.tensor_copy(st[:, 0:1], mv[:, 0:1])
                tmp = wk.tile([128, 1], f32, tag="tmp1", name=f"tmp1_{h}")
                nc.vector.tensor_mul(tmp, mv[:, 0:1], mv[:, 0:1])
                nc.vector.tensor_add(st[:, 1:2], mv[:, 1:2], tmp)
                st_t.append(st)

            # reduce over the 32 channels of each group (mask matmul)
            g_ps = ps1.tile([8, 2], f32, tag="tiny", bufs=2, name="g_ps")
            for h in range(2):
                nc.tensor.matmul(g_ps, gm_sb[:, h * 8:(h + 1) * 8], st_t[h],
                                 start=(h == 0), stop=(h == 1))
            gs2 = wk.tile([8, 2], f32, tag="gs2", name="gs2")
            nc.vector.tensor_scalar_mul(gs2, g_ps, 1.0 / 32.0)
            gt = wk.tile([8, 1], f32, tag="gt", name="gt")
            nc.vector.tensor_mul(gt, gs2[:, 0:1], gs2[:, 0:1])
            vg = wk.tile([8, 1], f32, tag="vg", name="vg")
            nc.vector.tensor_sub(vg, gs2[:, 1:2], gt)
            sq = wk.tile([8, 1], f32, tag="sq", name="sq")
            nc.scalar.activation(sq, vg, AF.Sqrt, bias=epsc[0:8], scale=1.0)
            gsb = wk.tile([8, 2], f32, tag="gsb", name="gsb")
            nc.vector.tensor_copy(gsb[:, 0:1], gs2[:, 0:1])
            nc.vector.reciprocal(gsb[:, 1:2], sq)

            # broadcast group stats back to channels; apply affine in place
            xn = xf  # normalized in place
            xnq = []
            for h in range(2):
                bc_ps = ps1.tile([128, 2], f32, tag="tiny", bufs=2,
                                 name=f"bc_ps{h}")
                nc.tensor.matmul(bc_ps, gmt_sb[:, h * 128:(h + 1) * 128], gsb,
                                 start=True, stop=True)
                ab = wk.tile([128, 2], f32, tag="ab", name=f"ab{h}")
                nc.vector.tensor_mul(ab[:, 0:1], nw_sb[:, h:h + 1], bc_ps[:, 1:2])
                tmp2 = wk.tile([128, 1], f32, tag="tmp2", name=f"tmp2_{h}")
                nc.vector.tensor_mul(tmp2, bc_ps[:, 0:1], ab[:, 0:1])
                nc.vector.tensor_sub(ab[:, 1:2], nb_sb[:, h:h + 1], tmp2)
                nc.vector.tensor_scalar(xf[h], xf[h], ab[:, 0:1], ab[:, 1:2],
                                        A.mult, A.add)
                t = sing.tile([128, NQ], f32r, tag=f"xnq{h}", name=f"xnq{h}")
                nc.vector.tensor_scalar(t, xq[h], ab[:, 0:1], ab[:, 1:2],
                                        A.mult, A.add)
                xnq.append(t)

            # K projection: K_sb[hp] = (2 heads stacked on partitions) x tokens
            for hp in range(2):
                for ch in range(8):
                    pk = ps1.tile([128, 512], f32, tag="pj", bufs=3,
                                  name=f"pk{hp}_{ch}")
                    for cc in range(2):
                        nc.tensor.matmul(
                            pk,
                            wk_sb[:, cc, hp * 128:(hp + 1) * 128],
                            xn[cc][:, ch * 512:(ch + 1) * 512],
                            start=(cc == 0), stop=(cc == 1))
                    nc.vector.tensor_scalar_add(
                        K_sb[hp][:, ch * 512:(ch + 1) * 512], pk,
                        bk_sb[:, hp:hp + 1])
                for ch in range(2):
                    pq = ps1.tile([128, 512], f32, tag="pj", bufs=3,
                                  name=f"pq{hp}_{ch}")
                    for cc in range(2):
                        nc.tensor.matmul(
                            pq,
                            wq_sb[:, cc, hp * 128:(hp + 1) * 128],
                            xnq[cc][:, ch * 512:(ch + 1) * 512],
                            start=(cc == 0), stop=(cc == 1))
                    nc.vector.tensor_scalar_add(
                        Q_sb[hp][:, ch * 512:(ch + 1) * 512], pq,
                        bq_sb[:, hp:hp + 1])

            # V^T projection (token-partition layout), all 4 heads per tile
            for tt in range(32):
                pv = ps1.tile([128, 256], f32, tag="pj", bufs=3, name=f"pv{tt}")
                for cc in range(2):
                    nc.tensor.matmul(
                        pv,
                        xn[cc][:, tt * 128:(tt + 1) * 128],
                        wv_sb[:, cc, :],
                        start=(cc == 0), stop=(cc == 1))
                nc.vector.tensor_copy(
                    vt[:, tt, :, 0:HD],
                    pv.rearrange("p (h e) -> p h e", e=HD))

        if debug_taps:
            nc.sync.dma_start(out=dbg["xn0"], in_=xn[0].bitcast(f32))
            nc.sync.dma_start(out=dbg["K0"], in_=K_sb[0].bitcast(f32))
            nc.sync.dma_start(out=dbg["Q0"], in_=Q_sb[0].bitcast(f32))
            nc.sync.dma_start(out=dbg["vt"], in_=vt.bitcast(f32))

        # ---------------- attention ----------------
        with tc.tile_pool(name="ps2", bufs=1, space="PSUM") as ps2, \
             tc.tile_pool(name="atp", bufs=2) as atp, \
             tc.tile_pool(name="rbp", bufs=2) as rbp:
            for hp in range(2):
                hA = ps2.tile([65, NQ], f32, tag="hA", bufs=1, name=f"hA{hp}")
                hB = ps2.tile([65, NQ], f32, tag="hB", bufs=1, name=f"hB{hp}")
                at_prev = None
                for kt in range(32):
                    at = atp.tile([128, 2048], f32r, tag="at", name=f"at{hp}_{kt}")
                    for qc in range(2):
                        sc = ps2.tile([128, 1024], f32, tag="sc", bufs=2,
                                      name=f"sc{hp}_{kt}_{qc}")
                        for sub in range(2):
                            nc.tensor.matmul(
                                sc[:, sub * 512:(sub + 1) * 512],
                                K_sb[hp][sub * 64:(sub + 1) * 64,
                                         kt * 128:(kt + 1) * 128],
                                Q_sb[hp][sub * 64:(sub + 1) * 64,
                                         qc * 512:(qc + 1) * 512],
                                start=True, stop=True)
                        nc.scalar.activation(at[:, qc * 1024:(qc + 1) * 1024],
                                             sc, AF.Exp, bias=m0c, scale=SCALE)
                    if debug_taps and hp == 0 and kt == 0:
                        nc.sync.dma_start(out=dbg["at00"], in_=at.bitcast(f32))
                    # attention @ [V | 1]  (software-pipelined one tile behind)
                    if at_prev is not None:
                        _av(nc, hA, hB, vt, at_prev, hp, kt - 1)
                    at_prev = at
                _av(nc, hA, hB, vt, at_prev, hp, 31)

                # normalize by row-sums (psum row 64), add v-bias
                if debug_taps and hp == 0:
                    hacp = rbp.tile([65, NQ], f32, tag="hacp", name="hacp", bufs=1)
                    nc.vector.tensor_copy(hacp, hA)
                    nc.sync.dma_start(out=dbg["hA0"], in_=hacp)
                rsA = rbp.tile([1, NQ], f32, tag="rsA", name=f"rsA{hp}")
                nc.vector.tensor_copy(rsA, hA[64:65, :])
                rsB = rbp.tile([1, NQ], f32, tag="rsB", name=f"rsB{hp}")
                nc.vector.tensor_copy(rsB, hB[64:65, :])
                rrA = rbp.tile([1, NQ], f32, tag="rrA", name=f"rrA{hp}")
                nc.vector.reciprocal(rrA, rsA)
                rrB = rbp.tile([1, NQ], f32, tag="rrB", name=f"rrB{hp}")
                nc.vector.reciprocal(rrB, rsB)
                # broadcast 1/rowsum across partitions via K=1 ones-matmul
                bb = ps2.tile([128, NQ], f32, tag="sc", bufs=2, name=f"bb{hp}")
                for qc in range(2):
                    nc.tensor.matmul(bb[0:64, qc * 512:(qc + 1) * 512], ones1,
                                     rrA[:, qc * 512:(qc + 1) * 512],
                                     start=True, stop=True)
                    nc.tensor.matmul(bb[64:128, qc * 512:(qc + 1) * 512], ones1,
                                     rrB[:, qc * 512:(qc + 1) * 512],
                                     start=True, stop=True, tile_position=(0, 64))
                rb = rbp.tile([128, NQ], f32, tag="rb", name=f"rb{hp}")
                nc.vector.tensor_copy(rb, bb)
                nc.vector.tensor_mul(hnT[hp][0:64, :], hA[0:64, :], rb[0:64, :])
                nc.vector.tensor_mul(hnT[hp][64:128, :], hB[0:64, :], rb[64:128, :])
                nc.vector.tensor_scalar_add(hnT[hp], hnT[hp], bv_sb[:, hp:hp + 1])
                if debug_taps and hp == 0:
                    nc.sync.dma_start(out=dbg["rsA0"], in_=rsA)
                    nc.sync.dma_start(out=dbg["rb0"], in_=rb)
                    nc.sync.dma_start(out=dbg["hn0"], in_=hnT[0].bitcast(f32))

        # ---------------- output projection + bias + residual ----------------
        with tc.tile_pool(name="ps3", bufs=1, space="PSUM") as ps3:
            for cc in range(2):
                osb = sing.tile([128, NQ], f32, tag=f"os{cc}", name=f"os{cc}")
                for qc in range(2):
                    op = ps3.tile([128, 512], f32, tag="op", bufs=2,
                                  name=f"op{cc}_{qc}")
                    for hp in range(2):
                        nc.tensor.matmul(
                            op,
                            wp_sb[:, hp, cc * 128:(cc + 1) * 128],
                            hnT[hp][:, qc * 512:(qc + 1) * 512],
                            start=(hp == 0), stop=(hp == 1))
                    nc.vector.scalar_tensor_tensor(
                        osb[:, qc * 512:(qc + 1) * 512], op,
                        pb_sb[:, cc:cc + 1],
                        xq[cc][:, qc * 512:(qc + 1) * 512],
                        A.add, A.add)
                nc.sync.dma_start(out=d_out[cc * 128:(cc + 1) * 128, :], in_=osb)

    def _av(nc, hA, hB, vt, at, hp, kt):
        for qc in range(2):
            nc.tensor.matmul(
                hA[:, qc * 512:(qc + 1) * 512],
                vt[:, kt, 2 * hp, :],
                at[:, qc * 1024:qc * 1024 + 512],
                start=(kt == 0), stop=(kt == 31))
            nc.tensor.matmul(
                hB[:, qc * 512:(qc + 1) * 512],
                vt[:, kt, 2 * hp + 1, :],
                at[:, qc * 1024 + 512:(qc + 1) * 1024],
                start=(kt == 0), stop=(kt == 31))

    with tile.TileContext(nc) as tc:
        with ExitStack() as ctx:
            body(ctx, tc)
    nc.compile()
    return nc


def _prep_in_maps(inputs: dict) -> list:
    x = np.ascontiguousarray(np.asarray(inputs["x"], dtype=np.float32))
    norm_w = np.asarray(inputs["norm_w"], dtype=np.float32)
    norm_b = np.asarray(inputs["norm_b"], dtype=np.float32)
    qkv_w = np.asarray(inputs["qkv_w"], dtype=np.float32)
    qkv_b = np.asarray(inputs["qkv_b"], dtype=np.float32)
    proj_w = np.asarray(inputs["proj_w"], dtype=np.float32)
    proj_b = np.asarray(inputs["proj_b"], dtype=np.float32)

    xr = x.reshape(B, C, N)
    wq_t = np.ascontiguousarray(qkv_w[0:C].T)
    wk_t = np.ascontiguousarray(qkv_w[C:2 * C].T)
    wv_t = np.ascontiguousarray(qkv_w[2 * C:3 * C].T)
    wp_t = np.ascontiguousarray(proj_w.T)
    bq = np.ascontiguousarray(qkv_b[0:C].reshape(2, 128).T)
    bk = np.ascontiguousarray(qkv_b[C:2 * C].reshape(2, 128).T)
    bv = np.ascontiguousarray(qkv_b[2 * C:3 * C].reshape(2, 128).T)
    nw = np.ascontiguousarray(norm_w.reshape(2, 128).T)
    nb = np.ascontiguousarray(norm_b.reshape(2, 128).T)
    pb = np.ascontiguousarray(proj_b.reshape(2, 128).T)

    cgrp = np.arange(C) // (C // G)
    gm3 = (cgrp.reshape(2, 128)[:, :, None] == np.arange(8)[None, None, :])
    gmask = np.ascontiguousarray(
        gm3.transpose(1, 0, 2).reshape(128, 16).astype(np.float32))
    gmask_t = np.ascontiguousarray(
        (np.arange(8)[:, None] == cgrp[None, :]).astype(np.float32))

    shared = dict(wq_t=wq_t, wk_t=wk_t, wv_t=wv_t, wp_t=wp_t,
                  bq=bq, bk=bk, bv=bv, nw=nw, nb=nb, pb=pb,
                  gmask=gmask, gmask_t=gmask_t)
    in_maps = []
    for core in range(N_CORES):
        b = core // 4
        qo = (core % 4) * NQ
        m = dict(shared)
        m["x_full"] = xr[b]
        m["x_q"] = np.ascontiguousarray(xr[b][:, qo:qo + NQ])
        in_maps.append(m)
    return in_maps


def kernel(**inputs) -> np.ndarray:
    from concourse.bass_utils import run_bass_kernel_spmd

    if "nc" not in _CACHE:
        _CACHE["nc"] = _build()
    nc = _CACHE["nc"]

    in_maps = _prep_in_maps(inputs)
    res = run_bass_kernel_spmd(nc, in_maps, core_ids=list(range(N_CORES)))

    out = np.empty((B, C, N), dtype=np.float32)
    for core in range(N_CORES):
        b = core // 4
        qo = (core % 4) * NQ
        out[b][:, qo:qo + NQ] = res.results[core]["out"]
    return out.reshape(B, C, 16, 16, 16)


# revision 35
# speedup vs baseline: 1.2396x; 1.2396x over previous
"""Trainium2 Bass kernel for the AttentionBlock problem.

Sharding (8 cores): core = 4*b + qi  (b = batch, qi = query-quarter).
Each core:
  - GroupNorm(8, C) stats over its batch's full (C=256, N=4096) activations,
    folded into the QKV weights (W' = W @ diag(a), b' = b + W @ beta) so the
    normalized activations are never materialized
  - K/V projections for all 4096 tokens (duplicated per batch pair of cores)
  - Q projection for its 1024 queries
  - attention (4 heads) for its 1024 queries against all 4096 keys
  - output projection + bias + residual for its disjoint (256, 1024) slice
Host unshard = pure concatenation of the 8 disjoint output slices.

Softmax uses a constant shift M0 (softmax is invariant to per-row constant
shifts; a global constant is exact in exact arithmetic and fp32-safe here:
scaled scores lie in [-16.5, 13.3] and the shifted exponentials stay well
inside fp32 range). Row-sums fall out of the attention-value matmul via a
ones-column appended to V; normalization and the V bias are applied after.
"""

import os
import sys

# The grading environment may pin JAX_PLATFORMS=cpu for the reference; the
# bass execution path needs the axon/neuron PJRT devices.
if os.environ.get("JAX_PLATFORMS", "").strip() == "cpu":
    del os.environ["JAX_PLATFORMS"]

for _p in ("/opt/trn_rl_repo",):
    if os.path.isdir(_p) and _p not in sys.path:
        sys.path.insert(0, _p)

import numpy as np

B = 2
C = 256
N = 4096
NQ = 1024  # queries per core
NH = 4
HD = 64
G = 8
EPS = 1e-5
SCALE = HD ** -0.5
M0 = 16.0  # constant softmax shift (in scaled-score units)
N_CORES = 8

_CACHE: dict = {}


def _build(debug_taps=False):
    from contextlib import ExitStack

    import concourse.bass as bass
    import concourse.tile as tile
    from concourse import bacc, mybir

    f32 = mybir.dt.float32
    f32r = mybir.dt.float32r
    A = mybir.AluOpType
    AF = mybir.ActivationFunctionType

    nc = bacc.Bacc("TRN2", target_bir_lowering=False, debug=False,
                   num_devices=N_CORES)

    d_xf = nc.dram_tensor("x_full", [C, N], f32r, kind="ExternalInput").ap()
    d_wq = nc.dram_tensor("wq_t", [C, C], f32r, kind="ExternalInput").ap()
    d_wk = nc.dram_tensor("wk_t", [C, C], f32r, kind="ExternalInput").ap()
    d_wv = nc.dram_tensor("wv_t", [C, C], f32r, kind="ExternalInput").ap()
    d_wp = nc.dram_tensor("wp_t", [C, C], f32r, kind="ExternalInput").ap()
    d_sm = nc.dram_tensor("smalls", [128, 28], f32, kind="ExternalInput").ap()
    d_gmt = nc.dram_tensor("gmask_t", [8, C], f32, kind="ExternalInput").ap()
    d_out = nc.dram_tensor("out", [C, NQ], f32, kind="ExternalOutput").ap()
    dbg = {}
    if debug_taps:
        dbg["K0"] = nc.dram_tensor("dbg_K0", [128, N], f32, kind="ExternalOutput").ap()
        dbg["Q0"] = nc.dram_tensor("dbg_Q0", [128, NQ], f32, kind="ExternalOutput").ap()
        dbg["vt"] = nc.dram_tensor("dbg_vt", [128, 32, NH, HD + 1], f32, kind="ExternalOutput").ap()
        dbg["at00"] = nc.dram_tensor("dbg_at00", [128, 2048], f32, kind="ExternalOutput").ap()
        dbg["hA0"] = nc.dram_tensor("dbg_hA0", [65, NQ], f32, kind="ExternalOutput").ap()
        dbg["rsA0"] = nc.dram_tensor("dbg_rsA0", [1, NQ], f32, kind="ExternalOutput").ap()
        dbg["hn0"] = nc.dram_tensor("dbg_hn0", [128, NQ], f32, kind="ExternalOutput").ap()
        dbg["rb0"] = nc.dram_tensor("dbg_rb0", [128, NQ], f32, kind="ExternalOutput").ap()

    def _av(hA, hB, vt, at, hp, kt):
        nc.tensor.matmul(
            hA, vt[:, kt, 2 * hp, :], at[:, 0:512],
            start=(kt == 0), stop=(kt == 31))
        nc.tensor.matmul(
            hB, vt[:, kt, 2 * hp + 1, :], at[:, 512:1024],
            start=(kt == 0), stop=(kt == 31))

    def body(ctx: ExitStack, tc: tile.TileContext):
        sing = ctx.enter_context(tc.tile_pool(name="sing", bufs=1))
        wk = ctx.enter_context(tc.tile_pool(name="wk", bufs=2))

        # ---------------- loads ----------------
        # DMA order matters (serial HBM bandwidth + ~0.6us HWDGE cost per
        # dma_start): one packed constants transfer, then x (paces the stats
        # chain), then weights in the order the fold needs them.
        sm_sb = sing.tile([128, 28], f32, tag="sm_sb", name="sm_sb")
        nc.sync.dma_start(out=sm_sb, in_=d_sm)
        gmt_sb = sing.tile([8, C], f32, tag="gmt_sb", name="gmt_sb")
        nc.sync.dma_start(out=gmt_sb, in_=d_gmt)
        bq_sb = sm_sb[:, 0:2]
        bk_sb = sm_sb[:, 2:4]
        bv_sb = sm_sb[:, 4:6]
        nw_sb = sm_sb[:, 6:8]
        nb_sb = sm_sb[:, 8:10]
        pb_sb = sm_sb[:, 10:12]
        gm_sb = sm_sb[:, 12:28]

        xf = []
        for h in range(2):
            t = sing.tile([128, N], f32r, tag=f"xf{h}", name=f"xf{h}")
            for chk in range(4):
                nc.sync.dma_start(
                    out=t[:, chk * 1024:(chk + 1) * 1024],
                    in_=d_xf[h * 128:(h + 1) * 128, chk * 1024:(chk + 1) * 1024])
            xf.append(t)
        # queries are token-columns 0:1024 of the (host-rotated) x
        xq = [xf[0][:, 0:NQ], xf[1][:, 0:NQ]]

        def load_w(name, dram):
            t = sing.tile([128, 2, C], f32r, tag=name, name=name)
            nc.sync.dma_start(out=t, in_=dram.rearrange("(c p) o -> p c o", p=128))
            return t

        wq_sb = load_w("wq_sb", d_wq)
        wk_sb = load_w("wk_sb", d_wk)
        wv_sb = load_w("wv_sb", d_wv)
        wp_sb = load_w("wp_sb", d_wp)

        # V^T tiles, per-head with an appended ones column for row-sums
        vt = sing.tile([128, 32, NH, HD + 1], f32r, tag="vt", name="vt")
        nc.vector.memset(vt[:, :, :, HD:HD + 1].bitcast(f32), 1.0)

        epsc = sing.tile([128, 1], f32, tag="epsc", name="epsc")
        nc.vector.memset(epsc, EPS)
        m0c = sing.tile([128, 1], f32, tag="m0c", name="m0c")
        nc.vector.memset(m0c, -M0)
        ones1 = sing.tile([1, 64], f32r, tag="ones1", name="ones1")
        nc.vector.memset(ones1.bitcast(f32), 1.0)
        # preload the sqrt activation table while ACT is idle
        scratch = sing.tile([128, 1], f32, tag="scratch", name="scratch")
        nc.scalar.activation(scratch, epsc, AF.Sqrt, bias=epsc, scale=1.0)

        K_sb = [sing.tile([128, N], f32r, tag=f"K{hp}", name=f"K{hp}")
                for hp in range(2)]
        Q_sb = [sing.tile([128, NQ], f32r, tag=f"Qs{hp}", name=f"Qs{hp}")
                for hp in range(2)]
        hnT = [sing.tile([128, NQ], f32r, tag=f"hn{hp}", name=f"hn{hp}")
               for hp in range(2)]

        # ---------------- groupnorm stats -> folded into weights -----------
        ps = ctx.enter_context(tc.tile_pool(name="ps", bufs=1, space="PSUM"))
        if True:
            st_t = []
            for h in range(2):
                stats = wk.tile([128, 8, 6], f32, tag="stats", name=f"stats{h}")
                for sg in range(8):
                    nc.vector.bn_stats(stats[:, sg, :],
                                       xf[h][:, sg * 512:(sg + 1) * 512])
                mv = wk.tile([128, 2], f32, tag="mv", name=f"mv{h}")
                nc.vector.bn_aggr(mv, stats)
                st = wk.tile([128, 2], f32, tag="st", name=f"st{h}")
                nc.vector.tensor_copy(st[:, 0:1], mv[:, 0:1])
                tmp = wk.tile([128, 1], f32, tag="tmp1", name=f"tmp1_{h}")
                nc.vector.tensor_mul(tmp, mv[:, 0:1], mv[:, 0:1])
                nc.vector.tensor_add(st[:, 1:2], mv[:, 1:2], tmp)
                st_t.append(st)

            # per-group mean / rstd via mask matmul over channels
            g_ps = ps.tile([8, 2], f32, tag="work", bufs=3, name="g_ps")
            for h in range(2):
                nc.tensor.matmul(g_ps, gm_sb[:, h * 8:(h + 1) * 8], st_t[h],
                                 start=(h == 0), stop=(h == 1))
            gs2 = wk.tile([8, 2], f32, tag="gs2", name="gs2")
            nc.vector.tensor_scalar_mul(gs2, g_ps, 1.0 / 32.0)
            gt = wk.tile([8, 1], f32, tag="gt", name="gt")
            nc.vector.tensor_mul(gt, gs2[:, 0:1], gs2[:, 0:1])
            vg = wk.tile([8, 1], f32, tag="vg", name="vg")
            nc.vector.tensor_sub(vg, gs2[:, 1:2], gt)
            sq = wk.tile([8, 1], f32, tag="sq", name="sq")
            nc.scalar.activation(sq, vg, AF.Sqrt, bias=epsc[0:8], scale=1.0)
            # preload the exp table set (input dep on sq keeps it after the
            # real sqrt so the table sets load exactly once each)
            nc.scalar.activation(scratch[0:8], sq, AF.Exp, bias=m0c[0:8],
                                 scale=1.0)
            gsb = wk.tile([8, 2], f32, tag="gsb", name="gsb")
            nc.vector.tensor_copy(gsb[:, 0:1], gs2[:, 0:1])
            nc.vector.reciprocal(gsb[:, 1:2], sq)

            # per-channel affine a, beta (per half), as f32r for the fold
            ab = []
            for h in range(2):
                bc_ps = ps.tile([128, 2], f32, tag="work", bufs=3,
                                name=f"bc_ps{h}")
                nc.tensor.matmul(bc_ps, gmt_sb[:, h * 128:(h + 1) * 128], gsb,
                                 start=True, stop=True)
                abt = wk.tile([128, 2], f32r, tag="ab", name=f"ab{h}")
                nc.vector.tensor_mul(abt[:, 0:1], nw_sb[:, h:h + 1], bc_ps[:, 1:2])
                tmp2 = wk.tile([128, 1], f32, tag="tmp2", name=f"tmp2_{h}")
                nc.vector.tensor_mul(tmp2, bc_ps[:, 0:1], abt[:, 0:1].bitcast(f32))
                nc.vector.tensor_sub(abt[:, 1:2], nb_sb[:, h:h + 1], tmp2)
                ab.append(abt)

            # bias corrections W^T beta (o-channel layout) -> b2 = b + W^T beta
            # (must read the ORIGINAL weights, so emitted before the fold)
            b2 = {}
            for wname, w_sb, b_sb in (("q", wq_sb, bq_sb), ("k", wk_sb, bk_sb),
                                      ("v", wv_sb, bv_sb)):
                b2t = wk.tile([128, 2], f32, tag=f"b2{wname}", name=f"b2{wname}",
                              bufs=1)
                for hp in range(2):
                    wb_ps = ps.tile([128, 1], f32, tag="work", bufs=3,
                                    name=f"wb_{wname}{hp}")
                    for cc in range(2):
                        nc.tensor.matmul(
                            wb_ps,
                            w_sb[:, cc, hp * 128:(hp + 1) * 128].bitcast(f32),
                            ab[cc][:, 1:2].bitcast(f32),
                            start=(cc == 0), stop=(cc == 1))
                    nc.vector.tensor_add(b2t[:, hp:hp + 1], b_sb[:, hp:hp + 1],
                                         wb_ps)
                b2[wname] = b2t
            # fold: W' = diag(a) applied along the contraction (channel) axis
            for w_sb in (wq_sb, wk_sb, wv_sb):
                for cc in range(2):
                    nc.vector.tensor_scalar_mul(w_sb[:, cc, :], w_sb[:, cc, :],
                                                ab[cc][:, 0:1].bitcast(f32))
            # fold the v bias through the output projection:
            # pb2 = pb + Wp^T bv2  (so hnT never needs a bias pass)
            pb2 = wk.tile([128, 2], f32, tag="pb2", name="pb2", bufs=1)
            for cc in range(2):
                pb_ps = ps.tile([128, 1], f32, tag="work", bufs=3,
                                name=f"pb_ps{cc}")
                for hpp in range(2):
                    nc.tensor.matmul(
                        pb_ps,
                        wp_sb[:, hpp, cc * 128:(cc + 1) * 128].bitcast(f32),
                        b2["v"][:, hpp:hpp + 1],
                        start=(hpp == 0), stop=(hpp == 1))
                nc.vector.tensor_add(pb2[:, cc:cc + 1], pb_sb[:, cc:cc + 1],
                                     pb_ps)

            # ---------------- projections (from raw x, folded weights) -----
            # Q first (scores need it for every key tile)
            for hp in range(2):
                for ch in range(2):
                    pq = ps.tile([128, 512], f32, tag="work", bufs=3,
                                 name=f"pq{hp}_{ch}")
                    for cc in range(2):
                        nc.tensor.matmul(
                            pq,
                            wq_sb[:, cc, hp * 128:(hp + 1) * 128],
                            xq[cc][:, ch * 512:(ch + 1) * 512],
                            start=(cc == 0), stop=(cc == 1))
                    nc.vector.tensor_scalar_add(
                        Q_sb[hp][:, ch * 512:(ch + 1) * 512], pq,
                        b2["q"][:, hp:hp + 1])
            def k_chunk(hp, ch):
                pk = ps.tile([128, 512], f32, tag="work", bufs=3,
                             name=f"pk{hp}_{ch}")
                for cc in range(2):
                    nc.tensor.matmul(
                        pk,
                        wk_sb[:, cc, hp * 128:(hp + 1) * 128],
                        xf[cc][:, ch * 512:(ch + 1) * 512],
                        start=(cc == 0), stop=(cc == 1))
                nc.vector.tensor_scalar_add(
                    K_sb[hp][:, ch * 512:(ch + 1) * 512], pk,
                    b2["k"][:, hp:hp + 1])

            def v_chunk(tt):
                pv = ps.tile([128, 256], f32, tag="work", bufs=3, name=f"pv{tt}")
                for cc in range(2):
                    nc.tensor.matmul(
                        pv,
                        xf[cc][:, tt * 128:(tt + 1) * 128],
                        wv_sb[:, cc, :],
                        start=(cc == 0), stop=(cc == 1))
                nc.vector.tensor_copy(
                    vt[:, tt, :, 0:HD],
                    pv.rearrange("p (h e) -> p h e", e=HD))

            k_chunk(0, 0)
            v_chunk(0)

        # ---------------- attention: 4 phases of (head-pair, query-half) ----
        # h accumulators are (65, 512) = 1 PSUM bank each, leaving the shared
        # "work" tag 3 slots -- enough to rotate scores double-buffering plus
        # the just-in-time K/V projection tiles without stalling ACT.
        PHASES = [(0, 0), (0, 1), (1, 0), (1, 1)]
        with tc.tile_pool(name="atp", bufs=3) as atp, \
             tc.tile_pool(name="rbp", bufs=1) as rbp:
            for hp, qc in PHASES:
                qs = slice(qc * 512, (qc + 1) * 512)
                hA = ps.tile([65, 512], f32, tag="hA", bufs=1,
                             name=f"hA{hp}_{qc}")
                hB = ps.tile([65, 512], f32, tag="hB", bufs=1,
                             name=f"hB{hp}_{qc}")
                at_prev = None
                phase_idx = PHASES.index((hp, qc))
                for kt in range(32):
                    if hp == 0 and qc == 0:
                        # just-in-time projection work rides the ACT-bound loop
                        if kt % 4 == 0 and kt // 4 + 1 <= 7:
                            k_chunk(0, kt // 4 + 1)
                        if kt < 31:
                            v_chunk(kt + 1)
                    if hp == 0 and qc == 1 and kt % 4 == 0:
                        k_chunk(1, kt // 4)
                    from contextlib import nullcontext
                    prio = (tc.high_priority(offset=50)
                            if (phase_idx > 0 and kt < 2) else nullcontext())
                    with prio:
                        at = atp.tile([128, 1024], f32r, tag="at",
                                      name=f"at{hp}_{qc}_{kt}")
                        sc = ps.tile([128, 1024], f32, tag="work", bufs=3,
                                     name=f"sc{hp}_{qc}_{kt}")
                        for sub in range(2):
                            nc.tensor.matmul(
                                sc[:, sub * 512:(sub + 1) * 512],
                                K_sb[hp][sub * 64:(sub + 1) * 64,
                                         kt * 128:(kt + 1) * 128],
                                Q_sb[hp][sub * 64:(sub + 1) * 64, qs],
                                start=True, stop=True)
                        nc.scalar.activation(at, sc, AF.Exp, bias=m0c,
                                             scale=SCALE)
                    if debug_taps and hp == 0 and qc == 0 and kt == 0:
                        nc.sync.dma_start(out=dbg["at00"], in_=at.bitcast(f32))
                    # attention @ [V | 1]  (software-pipelined one tile behind)
                    if at_prev is not None:
                        _av(hA, hB, vt, at_prev, hp, kt - 1)
                    at_prev = at
                _av(hA, hB, vt, at_prev, hp, 31)

                # normalize by row-sums (psum row 64), add v-bias
                if debug_taps and hp == 0 and qc == 0:
                    hacp = rbp.tile([65, 512], f32, tag="hacp", name="hacp",
                                    bufs=1)
                    nc.vector.tensor_copy(hacp, hA)
                    nc.sync.dma_start(out=dbg["hA0"][:, 0:512], in_=hacp)
                rsA = rbp.tile([1, 512], f32r, tag="rsA", name=f"rsA{hp}{qc}",
                               bufs=1)
                nc.vector.tensor_copy(rsA, hA[64:65, :])
                rsB = rbp.tile([1, 512], f32r, tag="rsB", name=f"rsB{hp}{qc}",
                               bufs=1)
                nc.vector.tensor_copy(rsB, hB[64:65, :])
                # broadcast raw rowsums across partitions via K=1 ones-matmul,
                # then one reciprocal over all 128 partitions
                bbA = ps.tile([64, 512], f32, tag="work", bufs=3,
                              name=f"bbA{hp}{qc}")
                nc.tensor.matmul(bbA, ones1, rsA, start=True, stop=True)
                bbB = ps.tile([64, 512], f32, tag="work", bufs=3,
                              name=f"bbB{hp}{qc}")
                nc.tensor.matmul(bbB, ones1, rsB, start=True, stop=True)
                rb = rbp.tile([128, 512], f32, tag="rb", name=f"rb{hp}{qc}",
                              bufs=1)
                nc.vector.reciprocal(rb[0:64, :], bbA)
                nc.vector.reciprocal(rb[64:128, :], bbB)
                nc.vector.tensor_mul(hnT[hp][0:64, qs], hA[0:64, :], rb[0:64, :])
                nc.vector.tensor_mul(hnT[hp][64:128, qs], hB[0:64, :],
                                     rb[64:128, :])
                if debug_taps and hp == 0 and qc == 1:
                    nc.sync.dma_start(out=dbg["rsA0"][:, 0:512],
                                      in_=rsA.bitcast(f32))
                    nc.sync.dma_start(out=dbg["rb0"][:, qs], in_=rb)
                    nc.sync.dma_start(out=dbg["hn0"], in_=hnT[0].bitcast(f32))

                # output projection for this query-half once both head-pairs
                # are done (overlaps the next phase's attention loop)
                if hp == 1:
                    for cc in range(2):
                        op = ps.tile([128, 512], f32, tag="work", bufs=3,
                                     name=f"op{cc}_{qc}")
                        for hpp in range(2):
                            nc.tensor.matmul(
                                op,
                                wp_sb[:, hpp, cc * 128:(cc + 1) * 128],
                                hnT[hpp][:, qs],
                                start=(hpp == 0), stop=(hpp == 1))
                        osb = sing.tile([128, NQ], f32, tag=f"os{cc}",
                                        name=f"os{cc}_{qc}")
                        nc.vector.scalar_tensor_tensor(
                            osb[:, qs], op, pb2[:, cc:cc + 1],
                            xq[cc][:, qs].bitcast(f32), A.add, A.add)
                        nc.sync.dma_start(
                            out=d_out[cc * 128:(cc + 1) * 128, qs],
                            in_=osb[:, qs])

        if debug_taps:
            nc.sync.dma_start(out=dbg["K0"], in_=K_sb[0].bitcast(f32))
            nc.sync.dma_start(out=dbg["Q0"], in_=Q_sb[0].bitcast(f32))
            nc.sync.dma_start(out=dbg["vt"], in_=vt.bitcast(f32))

    with tile.TileContext(nc) as tc:
        with ExitStack() as ctx:
            body(ctx, tc)
    nc.compile()
    return nc


def _prep_in_maps(inputs: dict) -> list:
    x = np.ascontiguousarray(np.asarray(inputs["x"], dtype=np.float32))
    norm_w = np.asarray(inputs["norm_w"], dtype=np.float32)
    norm_b = np.asarray(inputs["norm_b"], dtype=np.float32)
    qkv_w = np.asarray(inputs["qkv_w"], dtype=np.float32)
    qkv_b = np.asarray(inputs["qkv_b"], dtype=np.float32)
    proj_w = np.asarray(inputs["proj_w"], dtype=np.float32)
    proj_b = np.asarray(inputs["proj_b"], dtype=np.float32)

    xr = x.reshape(B, C, N)
    wq_t = np.ascontiguousarray(qkv_w[0:C].T)
    wk_t = np.ascontiguousarray(qkv_w[C:2 * C].T)
    wv_t = np.ascontiguousarray(qkv_w[2 * C:3 * C].T)
    wp_t = np.ascontiguousarray(proj_w.T)

    sm = np.zeros((128, 28), np.float32)
    sm[:, 0:2] = qkv_b[0:C].reshape(2, 128).T
    sm[:, 2:4] = qkv_b[C:2 * C].reshape(2, 128).T
    sm[:, 4:6] = qkv_b[2 * C:3 * C].reshape(2, 128).T
    sm[:, 6:8] = norm_w.reshape(2, 128).T
    sm[:, 8:10] = norm_b.reshape(2, 128).T
    sm[:, 10:12] = proj_b.reshape(2, 128).T
    cgrp = np.arange(C) // (C // G)
    gm3 = (cgrp.reshape(2, 128)[:, :, None] == np.arange(8)[None, None, :])
    sm[:, 12:28] = gm3.transpose(1, 0, 2).reshape(128, 16).astype(np.float32)
    gmask_t = np.ascontiguousarray(
        (np.arange(8)[:, None] == cgrp[None, :]).astype(np.float32))

    shared = dict(wq_t=wq_t, wk_t=wk_t, wv_t=wv_t, wp_t=wp_t,
                  smalls=sm, gmask_t=gmask_t)
    in_maps = []
    for core in range(N_CORES):
        b = core // 4
        qo = (core % 4) * NQ
        m = dict(shared)
        # rotate tokens so this core's queries sit at columns 0:NQ --
        # attention is permutation-equivariant over keys, so this is exact
        m["x_full"] = np.ascontiguousarray(np.roll(xr[b], -qo, axis=1))
        in_maps.append(m)
    return in_maps


def kernel(**inputs) -> np.ndarray:
    from concourse.bass_utils import run_bass_kernel_spmd

    if "nc" not in _CACHE:
        _CACHE["nc"] = _build()
    nc = _CACHE["nc"]

    in_maps = _prep_in_maps(inputs)
    res = run_bass_kernel_spmd(nc, in_maps, core_ids=list(range(N_CORES)))

    out = np.empty((B, C, N), dtype=np.float32)
    for core in range(N_CORES):
        b = core // 4
        qo = (core % 4) * NQ
        out[b][:, qo:qo + NQ] = res.results[core]["out"]
    return out.reshape(B, C, 16, 16, 16)


# revision 42
# speedup vs baseline: 1.2638x; 1.0195x over previous
"""Trainium2 Bass kernel for the AttentionBlock problem.

Sharding (8 cores): core = 4*b + qi  (b = batch, qi = query-quarter).
Each core:
  - GroupNorm(8, C) stats over its batch's full (C=256, N=4096) activations,
    folded into the QKV weights (W' = W @ diag(a), b' = b + W @ beta) so the
    normalized activations are never materialized
  - K/V projections for all 4096 tokens (duplicated per batch pair of cores)
  - Q projection for its 1024 queries
  - attention (4 heads) for its 1024 queries against all 4096 keys
  - output projection + bias + residual for its disjoint (256, 1024) slice
Host unshard = pure concatenation of the 8 disjoint output slices.

Softmax uses a constant shift M0 (softmax is invariant to per-row constant
shifts; a global constant is exact in exact arithmetic and fp32-safe here:
scaled scores lie in [-16.5, 13.3] and the shifted exponentials stay well
inside fp32 range). Row-sums fall out of the attention-value matmul via a
ones-column appended to V; normalization and the V bias are applied after.
"""

import os
import sys

# The grading environment may pin JAX_PLATFORMS=cpu for the reference; the
# bass execution path needs the axon/neuron PJRT devices.
if os.environ.get("JAX_PLATFORMS", "").strip() == "cpu":
    del os.environ["JAX_PLATFORMS"]

for _p in ("/opt/trn_rl_repo",):
    if os.path.isdir(_p) and _p not in sys.path:
        sys.path.insert(0, _p)

import numpy as np

B = 2
C = 256
N = 4096
NQ = 1024  # queries per core
NH = 4
HD = 64
G = 8
EPS = 1e-5
SCALE = HD ** -0.5
M0 = 16.0  # constant softmax shift (in scaled-score units)
N_CORES = 8

_CACHE: dict = {}


def _build(debug_taps=False, reps=1):
    from contextlib import ExitStack

    import concourse.bass as bass
    import concourse.tile as tile
    from concourse import bacc, mybir

    f32 = mybir.dt.float32
    f32r = mybir.dt.float32r
    A = mybir.AluOpType
    AF = mybir.ActivationFunctionType

    nc = bacc.Bacc("TRN2", target_bir_lowering=False, debug=False,
                   num_devices=N_CORES)

    d_xf = nc.dram_tensor("x_full", [C, N], f32r, kind="ExternalInput").ap()
    d_wq = nc.dram_tensor("wq_t", [C, C], f32r, kind="ExternalInput").ap()
    d_wk = nc.dram_tensor("wk_t", [C, C], f32r, kind="ExternalInput").ap()
    d_wv = nc.dram_tensor("wv_t", [C, C], f32r, kind="ExternalInput").ap()
    d_wp = nc.dram_tensor("wp_t", [C, C], f32r, kind="ExternalInput").ap()
    d_sm = nc.dram_tensor("smalls", [128, 28], f32, kind="ExternalInput").ap()
    d_gmt = nc.dram_tensor("gmask_t", [8, C], f32, kind="ExternalInput").ap()
    d_out = nc.dram_tensor("out", [C, NQ], f32, kind="ExternalOutput").ap()
    dbg = {}
    if debug_taps:
        dbg["K0"] = nc.dram_tensor("dbg_K0", [128, N], f32, kind="ExternalOutput").ap()
        dbg["Q0"] = nc.dram_tensor("dbg_Q0", [128, NQ], f32, kind="ExternalOutput").ap()
        dbg["vt"] = nc.dram_tensor("dbg_vt", [128, 32, NH, HD + 1], f32, kind="ExternalOutput").ap()
        dbg["at00"] = nc.dram_tensor("dbg_at00", [128, 2048], f32, kind="ExternalOutput").ap()
        dbg["hA0"] = nc.dram_tensor("dbg_hA0", [65, NQ], f32, kind="ExternalOutput").ap()
        dbg["rsA0"] = nc.dram_tensor("dbg_rsA0", [1, NQ], f32, kind="ExternalOutput").ap()
        dbg["hn0"] = nc.dram_tensor("dbg_hn0", [128, NQ], f32, kind="ExternalOutput").ap()
        dbg["rb0"] = nc.dram_tensor("dbg_rb0", [128, NQ], f32, kind="ExternalOutput").ap()

    def _av(hA, hB, vt, at, hp, kt):
        nc.tensor.matmul(
            hA, vt[:, kt, 2 * hp, :], at[:, 0:512],
            start=(kt == 0), stop=(kt == 31))
        nc.tensor.matmul(
            hB, vt[:, kt, 2 * hp + 1, :], at[:, 512:1024],
            start=(kt == 0), stop=(kt == 31))

    def body(ctx: ExitStack, tc: tile.TileContext):
        sing = ctx.enter_context(tc.tile_pool(name="sing", bufs=1))
        wk = ctx.enter_context(tc.tile_pool(name="wk", bufs=2))

        # ---------------- loads ----------------
        # DMA order matters (serial HBM bandwidth + ~0.6us HWDGE cost per
        # dma_start): one packed constants transfer, then x (paces the stats
        # chain), then weights in the order the fold needs them.
        sm_sb = sing.tile([128, 28], f32, tag="sm_sb", name="sm_sb")
        nc.sync.dma_start(out=sm_sb, in_=d_sm)
        gmt_sb = sing.tile([8, C], f32, tag="gmt_sb", name="gmt_sb")
        nc.sync.dma_start(out=gmt_sb, in_=d_gmt)
        bq_sb = sm_sb[:, 0:2]
        bk_sb = sm_sb[:, 2:4]
        bv_sb = sm_sb[:, 4:6]
        nw_sb = sm_sb[:, 6:8]
        nb_sb = sm_sb[:, 8:10]
        pb_sb = sm_sb[:, 10:12]
        gm_sb = sm_sb[:, 12:28]

        xf = []
        for h in range(2):
            t = sing.tile([128, N], f32r, tag=f"xf{h}", name=f"xf{h}")
            for chk in range(4):
                nc.sync.dma_start(
                    out=t[:, chk * 1024:(chk + 1) * 1024],
                    in_=d_xf[h * 128:(h + 1) * 128, chk * 1024:(chk + 1) * 1024])
            xf.append(t)
        # queries are token-columns 0:1024 of the (host-rotated) x
        xq = [xf[0][:, 0:NQ], xf[1][:, 0:NQ]]

        def load_w(name, dram):
            t = sing.tile([128, 2, C], f32r, tag=name, name=name)
            nc.sync.dma_start(out=t, in_=dram.rearrange("(c p) o -> p c o", p=128))
            return t

        wq_sb = load_w("wq_sb", d_wq)
        wk_sb = load_w("wk_sb", d_wk)
        wv_sb = load_w("wv_sb", d_wv)
        wp_sb = load_w("wp_sb", d_wp)

        # V^T tiles, per-head with an appended ones column for row-sums
        vt = sing.tile([128, 32, NH, HD + 1], f32r, tag="vt", name="vt")
        nc.vector.memset(vt[:, :, :, HD:HD + 1].bitcast(f32), 1.0)

        epsc = sing.tile([128, 1], f32, tag="epsc", name="epsc")
        nc.vector.memset(epsc, EPS)
        m0c = sing.tile([128, 1], f32, tag="m0c", name="m0c")
        nc.vector.memset(m0c, -M0)
        ones1 = sing.tile([1, 64], f32r, tag="ones1", name="ones1")
        nc.vector.memset(ones1.bitcast(f32), 1.0)
        # preload the sqrt activation table while ACT is idle
        scratch = sing.tile([128, 1], f32, tag="scratch", name="scratch")
        nc.scalar.activation(scratch, epsc, AF.Sqrt, bias=epsc, scale=1.0)

        K_sb = [sing.tile([128, N], f32r, tag=f"K{hp}", name=f"K{hp}")
                for hp in range(2)]
        Q_sb = [sing.tile([128, NQ], f32r, tag=f"Qs{hp}", name=f"Qs{hp}")
                for hp in range(2)]
        hnT = [sing.tile([128, NQ], f32r, tag=f"hn{hp}", name=f"hn{hp}")
               for hp in range(2)]

        # ---------------- groupnorm stats -> folded into weights -----------
        ps = ctx.enter_context(tc.tile_pool(name="ps", bufs=1, space="PSUM"))
        if True:
            st_t = []
            for h in range(2):
                stats = wk.tile([128, 8, 6], f32, tag="stats", name=f"stats{h}")
                for sg in range(8):
                    nc.vector.bn_stats(stats[:, sg, :],
                                       xf[h][:, sg * 512:(sg + 1) * 512])
                mv = wk.tile([128, 2], f32, tag="mv", name=f"mv{h}")
                nc.vector.bn_aggr(mv, stats)
                st = wk.tile([128, 2], f32, tag="st", name=f"st{h}")
                nc.vector.tensor_copy(st[:, 0:1], mv[:, 0:1])
                tmp = wk.tile([128, 1], f32, tag="tmp1", name=f"tmp1_{h}")
                nc.vector.tensor_mul(tmp, mv[:, 0:1], mv[:, 0:1])
                nc.vector.tensor_add(st[:, 1:2], mv[:, 1:2], tmp)
                st_t.append(st)

            # per-group mean / rstd via mask matmul over channels
            g_ps = ps.tile([8, 2], f32, tag="work", bufs=3, name="g_ps")
            for h in range(2):
                nc.tensor.matmul(g_ps, gm_sb[:, h * 8:(h + 1) * 8], st_t[h],
                                 start=(h == 0), stop=(h == 1))
            gs2 = wk.tile([8, 2], f32, tag="gs2", name="gs2")
            nc.vector.tensor_scalar_mul(gs2, g_ps, 1.0 / 32.0)
            gt = wk.tile([8, 1], f32, tag="gt", name="gt")
            nc.vector.tensor_mul(gt, gs2[:, 0:1], gs2[:, 0:1])
            vg = wk.tile([8, 1], f32, tag="vg", name="vg")
            nc.vector.tensor_sub(vg, gs2[:, 1:2], gt)
            sq = wk.tile([8, 1], f32, tag="sq", name="sq")
            nc.scalar.activation(sq, vg, AF.Sqrt, bias=epsc[0:8], scale=1.0)
            # preload the exp table set (input dep on sq keeps it after the
            # real sqrt so the table sets load exactly once each)
            nc.scalar.activation(scratch[0:8], sq, AF.Exp, bias=m0c[0:8],
                                 scale=1.0)
            gsb = wk.tile([8, 2], f32, tag="gsb", name="gsb")
            nc.vector.tensor_copy(gsb[:, 0:1], gs2[:, 0:1])
            nc.vector.reciprocal(gsb[:, 1:2], sq)

            # per-channel affine a, beta (per half), as f32r for the fold
            ab = []
            for h in range(2):
                bc_ps = ps.tile([128, 2], f32, tag="work", bufs=3,
                                name=f"bc_ps{h}")
                nc.tensor.matmul(bc_ps, gmt_sb[:, h * 128:(h + 1) * 128], gsb,
                                 start=True, stop=True)
                abt = wk.tile([128, 2], f32r, tag="ab", name=f"ab{h}")
                nc.vector.tensor_mul(abt[:, 0:1], nw_sb[:, h:h + 1], bc_ps[:, 1:2])
                tmp2 = wk.tile([128, 1], f32, tag="tmp2", name=f"tmp2_{h}")
                nc.vector.tensor_mul(tmp2, bc_ps[:, 0:1], abt[:, 0:1].bitcast(f32))
                nc.vector.tensor_sub(abt[:, 1:2], nb_sb[:, h:h + 1], tmp2)
                ab.append(abt)

            # fold first (per weight, in the order the projections need
            # them), then bias corrections b2 = b + W'^T (beta/a) -- using the
            # folded weights keeps the fold off the critical path
            for w_sb in (wq_sb, wk_sb, wv_sb):
                for cc in range(2):
                    nc.vector.tensor_scalar_mul(w_sb[:, cc, :], w_sb[:, cc, :],
                                                ab[cc][:, 0:1].bitcast(f32))
            ba = []
            for cc in range(2):
                t = wk.tile([128, 1], f32, tag="ba", name=f"ba{cc}")
                nc.vector.reciprocal(t, ab[cc][:, 0:1].bitcast(f32))
                nc.vector.tensor_mul(t, t, ab[cc][:, 1:2].bitcast(f32))
                ba.append(t)
            b2 = {}
            for wname, w_sb, b_sb in (("q", wq_sb, bq_sb), ("k", wk_sb, bk_sb),
                                      ("v", wv_sb, bv_sb)):
                b2t = wk.tile([128, 2], f32, tag=f"b2{wname}", name=f"b2{wname}",
                              bufs=1)
                for hp in range(2):
                    wb_ps = ps.tile([128, 1], f32, tag="work", bufs=3,
                                    name=f"wb_{wname}{hp}")
                    for cc in range(2):
                        nc.tensor.matmul(
                            wb_ps,
                            w_sb[:, cc, hp * 128:(hp + 1) * 128].bitcast(f32),
                            ba[cc],
                            start=(cc == 0), stop=(cc == 1))
                    nc.vector.tensor_add(b2t[:, hp:hp + 1], b_sb[:, hp:hp + 1],
                                         wb_ps)
                b2[wname] = b2t
            pb2 = wk.tile([128, 2], f32, tag="pb2", name="pb2", bufs=1)
            for cc in range(2):
                pb_ps = ps.tile([128, 1], f32, tag="work", bufs=3,
                                name=f"pb_ps{cc}")
                for hpp in range(2):
                    nc.tensor.matmul(
                        pb_ps,
                        wp_sb[:, hpp, cc * 128:(cc + 1) * 128].bitcast(f32),
                        b2["v"][:, hpp:hpp + 1],
                        start=(hpp == 0), stop=(hpp == 1))
                nc.vector.tensor_add(pb2[:, cc:cc + 1], pb_sb[:, cc:cc + 1],
                                     pb_ps)

            # ---------------- projections (from raw x, folded weights) -----
            # Q first (scores need it for every key tile)
            for hp in range(2):
                for ch in range(2):
                    pq = ps.tile([128, 512], f32, tag="work", bufs=3,
                                 name=f"pq{hp}_{ch}")
                    for cc in range(2):
                        nc.tensor.matmul(
                            pq,
                            wq_sb[:, cc, hp * 128:(hp + 1) * 128],
                            xq[cc][:, ch * 512:(ch + 1) * 512],
                            start=(cc == 0), stop=(cc == 1))
                    nc.vector.tensor_scalar_add(
                        Q_sb[hp][:, ch * 512:(ch + 1) * 512], pq,
                        b2["q"][:, hp:hp + 1])
            def k_chunk(hp, ch):
                pk = ps.tile([128, 512], f32, tag="work", bufs=3,
                             name=f"pk{hp}_{ch}")
                for cc in range(2):
                    nc.tensor.matmul(
                        pk,
                        wk_sb[:, cc, hp * 128:(hp + 1) * 128],
                        xf[cc][:, ch * 512:(ch + 1) * 512],
                        start=(cc == 0), stop=(cc == 1))
                nc.vector.tensor_scalar_add(
                    K_sb[hp][:, ch * 512:(ch + 1) * 512], pk,
                    b2["k"][:, hp:hp + 1])

            def v_chunk2(tt0):
                # two token-tiles per psum tile (halves work-slot pressure)
                pv = ps.tile([128, 512], f32, tag="work", bufs=3,
                             name=f"pv{tt0}")
                for j in range(2):
                    tt = tt0 + j
                    for cc in range(2):
                        nc.tensor.matmul(
                            pv[:, j * 256:(j + 1) * 256],
                            xf[cc][:, tt * 128:(tt + 1) * 128],
                            wv_sb[:, cc, :],
                            start=(cc == 0), stop=(cc == 1))
                nc.vector.tensor_copy(
                    vt[:, tt0:tt0 + 2, :, 0:HD],
                    pv.rearrange("p (t h e) -> p t h e", t=2, e=HD))

            k_chunk(0, 0)
            v_chunk2(0)

        # ---------------- attention: 4 phases of (head-pair, query-half) ----
        # h accumulators are (65, 512) = 1 PSUM bank each, leaving the shared
        # "work" tag 3 slots -- enough to rotate scores double-buffering plus
        # the just-in-time K/V projection tiles without stalling ACT.
        PHASES = [(0, 0), (0, 1), (1, 0), (1, 1)]
        with tc.tile_pool(name="atp", bufs=3) as atp, \
             tc.tile_pool(name="rbp", bufs=1) as rbp:
            for hp, qc in PHASES:
                qs = slice(qc * 512, (qc + 1) * 512)
                hA = ps.tile([65, 512], f32, tag="hA", bufs=1,
                             name=f"hA{hp}_{qc}")
                hB = ps.tile([65, 512], f32, tag="hB", bufs=1,
                             name=f"hB{hp}_{qc}")
                at_prev = None
                phase_idx = PHASES.index((hp, qc))
                for kt in range(32):
                    if hp == 0 and qc == 0 and kt % 2 == 0 and kt < 30:
                        v_chunk2(kt + 2)
                    from contextlib import nullcontext
                    prio = (tc.high_priority(offset=80)
                            if (phase_idx > 0 and kt < 2) else nullcontext())
                    with prio:
                        at = atp.tile([128, 1024], f32r, tag="at",
                                      name=f"at{hp}_{qc}_{kt}")
                        sc = ps.tile([128, 1024], f32, tag="work", bufs=3,
                                     name=f"sc{hp}_{qc}_{kt}")
                        for sub in range(2):
                            nc.tensor.matmul(
                                sc[:, sub * 512:(sub + 1) * 512],
                                K_sb[hp][sub * 64:(sub + 1) * 64,
                                         kt * 128:(kt + 1) * 128],
                                Q_sb[hp][sub * 64:(sub + 1) * 64, qs],
                                start=True, stop=True)
                        nc.scalar.activation(at, sc, AF.Exp, bias=m0c,
                                             scale=SCALE)
                    if debug_taps and hp == 0 and qc == 0 and kt == 0:
                        nc.sync.dma_start(out=dbg["at00"], in_=at.bitcast(f32))
                    # attention @ [V | 1]  (software-pipelined one tile behind)
                    if at_prev is not None:
                        _av(hA, hB, vt, at_prev, hp, kt - 1)
                    at_prev = at
                    # just-in-time K chunks after the AV so the next scores
                    # tile reuses a fast-freed slot
                    if hp == 0 and qc == 0 and kt % 4 == 0 and kt // 4 + 1 <= 7:
                        k_chunk(0, kt // 4 + 1)
                    if hp == 0 and qc == 1 and kt % 4 == 0 and kt // 4 < 8:
                        k_chunk(1, kt // 4)
                _av(hA, hB, vt, at_prev, hp, 31)

                # normalize by row-sums (psum row 64), add v-bias
                if debug_taps and hp == 0 and qc == 0:
                    hacp = rbp.tile([65, 512], f32, tag="hacp", name="hacp",
                                    bufs=1)
                    nc.vector.tensor_copy(hacp, hA)
                    nc.sync.dma_start(out=dbg["hA0"][:, 0:512], in_=hacp)
                rsA = rbp.tile([1, 512], f32r, tag="rsA", name=f"rsA{hp}{qc}",
                               bufs=1)
                nc.vector.tensor_copy(rsA, hA[64:65, :])
                rsB = rbp.tile([1, 512], f32r, tag="rsB", name=f"rsB{hp}{qc}",
                               bufs=1)
                nc.vector.tensor_copy(rsB, hB[64:65, :])
                # broadcast raw rowsums across partitions via K=1 ones-matmul,
                # then one reciprocal over all 128 partitions
                bbA = ps.tile([64, 512], f32, tag="work", bufs=3,
                              name=f"bbA{hp}{qc}")
                nc.tensor.matmul(bbA, ones1, rsA, start=True, stop=True)
                bbB = ps.tile([64, 512], f32, tag="work", bufs=3,
                              name=f"bbB{hp}{qc}")
                nc.tensor.matmul(bbB, ones1, rsB, start=True, stop=True)
                rb = rbp.tile([128, 512], f32, tag="rb", name=f"rb{hp}{qc}",
                              bufs=1)
                nc.vector.reciprocal(rb[0:64, :], bbA)
                nc.vector.reciprocal(rb[64:128, :], bbB)
                nc.vector.tensor_mul(hnT[hp][0:64, qs], hA[0:64, :], rb[0:64, :])
                nc.vector.tensor_mul(hnT[hp][64:128, qs], hB[0:64, :],
                                     rb[64:128, :])
                if debug_taps and hp == 0 and qc == 1:
                    nc.sync.dma_start(out=dbg["rsA0"][:, 0:512],
                                      in_=rsA.bitcast(f32))
                    nc.sync.dma_start(out=dbg["rb0"][:, qs], in_=rb)
                    nc.sync.dma_start(out=dbg["hn0"], in_=hnT[0].bitcast(f32))

                # output projection for this query-half once both head-pairs
                # are done (overlaps the next phase's attention loop)
                if hp == 1:
                    for cc in range(2):
                        op = ps.tile([128, 512], f32, tag="work", bufs=3,
                                     name=f"op{cc}_{qc}")
                        for hpp in range(2):
                            nc.tensor.matmul(
                                op,
                                wp_sb[:, hpp, cc * 128:(cc + 1) * 128],
                                hnT[hpp][:, qs],
                                start=(hpp == 0), stop=(hpp == 1))
                        osb = sing.tile([128, NQ], f32, tag=f"os{cc}",
                                        name=f"os{cc}_{qc}")
                        nc.vector.scalar_tensor_tensor(
                            osb[:, qs], op, pb2[:, cc:cc + 1],
                            xq[cc][:, qs].bitcast(f32), A.add, A.add)
                        nc.sync.dma_start(
                            out=d_out[cc * 128:(cc + 1) * 128, qs],
                            in_=osb[:, qs])

        if debug_taps:
            nc.sync.dma_start(out=dbg["K0"], in_=K_sb[0].bitcast(f32))
            nc.sync.dma_start(out=dbg["Q0"], in_=Q_sb[0].bitcast(f32))
            nc.sync.dma_start(out=dbg["vt"], in_=vt.bitcast(f32))

    with tile.TileContext(nc) as tc:
        for _ in range(reps):
            with ExitStack() as ctx:
                body(ctx, tc)
    nc.compile()
    return nc


def _prep_in_maps(inputs: dict) -> list:
    x = np.ascontiguousarray(np.asarray(inputs["x"], dtype=np.float32))
    norm_w = np.asarray(inputs["norm_w"], dtype=np.float32)
    norm_b = np.asarray(inputs["norm_b"], dtype=np.float32)
    qkv_w = np.asarray(inputs["qkv_w"], dtype=np.float32)
    qkv_b = np.asarray(inputs["qkv_b"], dtype=np.float32)
    proj_w = np.asarray(inputs["proj_w"], dtype=np.float32)
    proj_b = np.asarray(inputs["proj_b"], dtype=np.float32)

    xr = x.reshape(B, C, N)
    wq_t = np.ascontiguousarray(qkv_w[0:C].T)
    wk_t = np.ascontiguousarray(qkv_w[C:2 * C].T)
    wv_t = np.ascontiguousarray(qkv_w[2 * C:3 * C].T)
    wp_t = np.ascontiguousarray(proj_w.T)

    sm = np.zeros((128, 28), np.float32)
    sm[:, 0:2] = qkv_b[0:C].reshape(2, 128).T
    sm[:, 2:4] = qkv_b[C:2 * C].reshape(2, 128).T
    sm[:, 4:6] = qkv_b[2 * C:3 * C].reshape(2, 128).T
    sm[:, 6:8] = norm_w.reshape(2, 128).T
    sm[:, 8:10] = norm_b.reshape(2, 128).T
    sm[:, 10:12] = proj_b.reshape(2, 128).T
    cgrp = np.arange(C) // (C // G)
    gm3 = (cgrp.reshape(2, 128)[:, :, None] == np.arange(8)[None, None, :])
    sm[:, 12:28] = gm3.transpose(1, 0, 2).reshape(128, 16).astype(np.float32)
    gmask_t = np.ascontiguousarray(
        (np.arange(8)[:, None] == cgrp[None, :]).astype(np.float32))

    shared = dict(wq_t=wq_t, wk_t=wk_t, wv_t=wv_t, wp_t=wp_t,
                  smalls=sm, gmask_t=gmask_t)
    in_maps = []
    for core in range(N_CORES):
        b = core // 4
        qo = (core % 4) * NQ
        m = dict(shared)
        # rotate tokens so this core's queries sit at columns 0:NQ --
        # attention is permutation-equivariant over keys, so this is exact
        m["x_full"] = np.ascontiguousarray(np.roll(xr[b], -qo, axis=1))
        in_maps.append(m)
    return in_maps


def kernel(**inputs) -> np.ndarray:
    from concourse.bass_utils import run_bass_kernel_spmd

    if "nc" not in _CACHE:
        _CACHE["nc"] = _build()
    nc = _CACHE["nc"]

    in_maps = _prep_in_maps(inputs)
    res = run_bass_kernel_spmd(nc, in_maps, core_ids=list(range(N_CORES)))

    out = np.empty((B, C, N), dtype=np.float32)
    for core in range(N_CORES):
        b = core // 4
        qo = (core % 4) * NQ
        out[b][:, qo:qo + NQ] = res.results[core]["out"]
    return out.reshape(B, C, 16, 16, 16)


# revision 43
# speedup vs baseline: 1.2952x; 1.0248x over previous
"""Trainium2 Bass kernel for the AttentionBlock problem.

Sharding (8 cores): core = 4*b + qi  (b = batch, qi = query-quarter).
Each core:
  - GroupNorm(8, C) stats over its batch's full (C=256, N=4096) activations,
    folded into the QKV weights (W' = W @ diag(a), b' = b + W @ beta) so the
    normalized activations are never materialized
  - K/V projections for all 4096 tokens (duplicated per batch pair of cores)
  - Q projection for its 1024 queries
  - attention (4 heads) for its 1024 queries against all 4096 keys
  - output projection + bias + residual for its disjoint (256, 1024) slice
Host unshard = pure concatenation of the 8 disjoint output slices.

Softmax uses a constant shift M0 (softmax is invariant to per-row constant
shifts; a global constant is exact in exact arithmetic and fp32-safe here:
scaled scores lie in [-16.5, 13.3] and the shifted exponentials stay well
inside fp32 range). Row-sums fall out of the attention-value matmul via a
ones-column appended to V; normalization and the V bias are applied after.
"""

import os
import sys

# The grading environment may pin JAX_PLATFORMS=cpu for the reference; the
# bass execution path needs the axon/neuron PJRT devices.
if os.environ.get("JAX_PLATFORMS", "").strip() == "cpu":
    del os.environ["JAX_PLATFORMS"]

for _p in ("/opt/trn_rl_repo",):
    if os.path.isdir(_p) and _p not in sys.path:
        sys.path.insert(0, _p)

import numpy as np

B = 2
C = 256
N = 4096
NQ = 1024  # queries per core
NH = 4
HD = 64
G = 8
EPS = 1e-5
SCALE = HD ** -0.5
M0 = 16.0  # constant softmax shift (in scaled-score units)
N_CORES = 8

_CACHE: dict = {}


def _build(debug_taps=False, reps=1):
    from contextlib import ExitStack

    import concourse.bass as bass
    import concourse.tile as tile
    from concourse import bacc, mybir

    f32 = mybir.dt.float32
    f32r = mybir.dt.float32r
    A = mybir.AluOpType
    AF = mybir.ActivationFunctionType

    nc = bacc.Bacc("TRN2", target_bir_lowering=False, debug=False,
                   num_devices=N_CORES)

    d_xf = nc.dram_tensor("x_full", [C, N], f32r, kind="ExternalInput").ap()
    d_wq = nc.dram_tensor("wq_t", [C, C], f32r, kind="ExternalInput").ap()
    d_wk = nc.dram_tensor("wk_t", [C, C], f32r, kind="ExternalInput").ap()
    d_wv = nc.dram_tensor("wv_t", [C, C], f32r, kind="ExternalInput").ap()
    d_wp = nc.dram_tensor("wp_t", [C, C], f32r, kind="ExternalInput").ap()
    d_sm = nc.dram_tensor("smalls", [128, 28], f32, kind="ExternalInput").ap()
    d_gmt = nc.dram_tensor("gmask_t", [8, C], f32, kind="ExternalInput").ap()
    d_out = nc.dram_tensor("out", [C, NQ], f32, kind="ExternalOutput").ap()
    dbg = {}
    if debug_taps:
        dbg["K0"] = nc.dram_tensor("dbg_K0", [128, N], f32, kind="ExternalOutput").ap()
        dbg["Q0"] = nc.dram_tensor("dbg_Q0", [128, NQ], f32, kind="ExternalOutput").ap()
        dbg["vt"] = nc.dram_tensor("dbg_vt", [128, 32, NH, HD + 1], f32, kind="ExternalOutput").ap()
        dbg["at00"] = nc.dram_tensor("dbg_at00", [128, 2048], f32, kind="ExternalOutput").ap()
        dbg["hA0"] = nc.dram_tensor("dbg_hA0", [65, NQ], f32, kind="ExternalOutput").ap()
        dbg["rsA0"] = nc.dram_tensor("dbg_rsA0", [1, NQ], f32, kind="ExternalOutput").ap()
        dbg["hn0"] = nc.dram_tensor("dbg_hn0", [128, NQ], f32, kind="ExternalOutput").ap()
        dbg["rb0"] = nc.dram_tensor("dbg_rb0", [128, NQ], f32, kind="ExternalOutput").ap()

    def _av(hA, hB, vt, at, hp, kt):
        nc.tensor.matmul(
            hA, vt[:, kt, 2 * hp, :], at[:, 0:512],
            start=(kt == 0), stop=(kt == 31))
        nc.tensor.matmul(
            hB, vt[:, kt, 2 * hp + 1, :], at[:, 512:1024],
            start=(kt == 0), stop=(kt == 31))

    def body(ctx: ExitStack, tc: tile.TileContext):
        sing = ctx.enter_context(tc.tile_pool(name="sing", bufs=1))
        wk = ctx.enter_context(tc.tile_pool(name="wk", bufs=2))

        # ---------------- loads ----------------
        # DMA order matters (serial HBM bandwidth + ~0.6us HWDGE cost per
        # dma_start): one packed constants transfer, then x (paces the stats
        # chain), then weights in the order the fold needs them.
        sm_sb = sing.tile([128, 28], f32, tag="sm_sb", name="sm_sb")
        nc.sync.dma_start(out=sm_sb, in_=d_sm)
        gmt_sb = sing.tile([8, C], f32, tag="gmt_sb", name="gmt_sb")
        nc.sync.dma_start(out=gmt_sb, in_=d_gmt)
        bq_sb = sm_sb[:, 0:2]
        bk_sb = sm_sb[:, 2:4]
        bv_sb = sm_sb[:, 4:6]
        nw_sb = sm_sb[:, 6:8]
        nb_sb = sm_sb[:, 8:10]
        pb_sb = sm_sb[:, 10:12]
        gm_sb = sm_sb[:, 12:28]

        xf = []
        for h in range(2):
            t = sing.tile([128, N], f32r, tag=f"xf{h}", name=f"xf{h}")
            for chk in range(4):
                nc.sync.dma_start(
                    out=t[:, chk * 1024:(chk + 1) * 1024],
                    in_=d_xf[h * 128:(h + 1) * 128, chk * 1024:(chk + 1) * 1024])
            xf.append(t)
        # queries are token-columns 0:1024 of the (host-rotated) x
        xq = [xf[0][:, 0:NQ], xf[1][:, 0:NQ]]

        def load_w(name, dram):
            t = sing.tile([128, 2, C], f32r, tag=name, name=name)
            nc.sync.dma_start(out=t, in_=dram.rearrange("(c p) o -> p c o", p=128))
            return t

        wq_sb = load_w("wq_sb", d_wq)
        wk_sb = load_w("wk_sb", d_wk)
        wv_sb = load_w("wv_sb", d_wv)
        wp_sb = load_w("wp_sb", d_wp)

        # V^T tiles, per-head with an appended ones column for row-sums
        vt = sing.tile([128, 32, NH, HD + 1], f32r, tag="vt", name="vt")
        nc.vector.memset(vt[:, :, :, HD:HD + 1].bitcast(f32), 1.0)

        epsc = sing.tile([128, 1], f32, tag="epsc", name="epsc")
        nc.vector.memset(epsc, EPS)
        m0c = sing.tile([128, 1], f32, tag="m0c", name="m0c")
        nc.vector.memset(m0c, -M0)
        ones1 = sing.tile([1, 64], f32r, tag="ones1", name="ones1")
        nc.vector.memset(ones1.bitcast(f32), 1.0)
        # preload the sqrt activation table while ACT is idle
        scratch = sing.tile([128, 1], f32, tag="scratch", name="scratch")
        nc.scalar.activation(scratch, epsc, AF.Sqrt, bias=epsc, scale=1.0)

        K_sb = [sing.tile([128, N], f32r, tag=f"K{hp}", name=f"K{hp}")
                for hp in range(2)]
        Q_sb = [sing.tile([128, NQ], f32r, tag=f"Qs{hp}", name=f"Qs{hp}")
                for hp in range(2)]
        hnT = [sing.tile([128, NQ], f32r, tag=f"hn{hp}", name=f"hn{hp}")
               for hp in range(2)]

        # ---------------- groupnorm stats -> folded into weights -----------
        ps = ctx.enter_context(tc.tile_pool(name="ps", bufs=1, space="PSUM"))
        if True:
            st_t = []
            for h in range(2):
                stats = wk.tile([128, 8, 6], f32, tag="stats", name=f"stats{h}")
                for sg in range(8):
                    nc.vector.bn_stats(stats[:, sg, :],
                                       xf[h][:, sg * 512:(sg + 1) * 512])
                mv = wk.tile([128, 2], f32, tag="mv", name=f"mv{h}")
                nc.vector.bn_aggr(mv, stats)
                st = wk.tile([128, 2], f32, tag="st", name=f"st{h}")
                nc.vector.tensor_copy(st[:, 0:1], mv[:, 0:1])
                tmp = wk.tile([128, 1], f32, tag="tmp1", name=f"tmp1_{h}")
                nc.vector.tensor_mul(tmp, mv[:, 0:1], mv[:, 0:1])
                nc.vector.tensor_add(st[:, 1:2], mv[:, 1:2], tmp)
                st_t.append(st)

            # per-group mean / rstd via mask matmul over channels
            g_ps = ps.tile([8, 2], f32, tag="work", bufs=3, name="g_ps")
            for h in range(2):
                nc.tensor.matmul(g_ps, gm_sb[:, h * 8:(h + 1) * 8], st_t[h],
                                 start=(h == 0), stop=(h == 1))
            gs2 = wk.tile([8, 2], f32, tag="gs2", name="gs2")
            nc.vector.tensor_scalar_mul(gs2, g_ps, 1.0 / 32.0)
            gt = wk.tile([8, 1], f32, tag="gt", name="gt")
            nc.vector.tensor_mul(gt, gs2[:, 0:1], gs2[:, 0:1])
            vg = wk.tile([8, 1], f32, tag="vg", name="vg")
            nc.vector.tensor_sub(vg, gs2[:, 1:2], gt)
            sq = wk.tile([8, 1], f32, tag="sq", name="sq")
            nc.scalar.activation(sq, vg, AF.Sqrt, bias=epsc[0:8], scale=1.0)
            # preload the exp table set (input dep on sq keeps it after the
            # real sqrt so the table sets load exactly once each)
            nc.scalar.activation(scratch[0:8], sq, AF.Exp, bias=m0c[0:8],
                                 scale=1.0)
            gsb = wk.tile([8, 2], f32, tag="gsb", name="gsb")
            nc.vector.tensor_copy(gsb[:, 0:1], gs2[:, 0:1])
            nc.vector.reciprocal(gsb[:, 1:2], sq)

            # per-channel affine a, beta (per half), as f32r for the fold
            ab = []
            for h in range(2):
                bc_ps = ps.tile([128, 2], f32, tag="work", bufs=3,
                                name=f"bc_ps{h}")
                nc.tensor.matmul(bc_ps, gmt_sb[:, h * 128:(h + 1) * 128], gsb,
                                 start=True, stop=True)
                abt = wk.tile([128, 2], f32r, tag="ab", name=f"ab{h}")
                nc.vector.tensor_mul(abt[:, 0:1], nw_sb[:, h:h + 1], bc_ps[:, 1:2])
                tmp2 = wk.tile([128, 1], f32, tag="tmp2", name=f"tmp2_{h}")
                nc.vector.tensor_mul(tmp2, bc_ps[:, 0:1], abt[:, 0:1].bitcast(f32))
                nc.vector.tensor_sub(abt[:, 1:2], nb_sb[:, h:h + 1], tmp2)
                ab.append(abt)

            # fold first (per weight, in the order the projections need
            # them), then bias corrections b2 = b + W'^T (beta/a) -- using the
            # folded weights keeps the fold off the critical path
            for w_sb in (wq_sb, wk_sb, wv_sb):
                for cc in range(2):
                    nc.vector.tensor_scalar_mul(w_sb[:, cc, :], w_sb[:, cc, :],
                                                ab[cc][:, 0:1].bitcast(f32))
            ba = []
            for cc in range(2):
                t = wk.tile([128, 1], f32, tag="ba", name=f"ba{cc}")
                nc.vector.reciprocal(t, ab[cc][:, 0:1].bitcast(f32))
                nc.vector.tensor_mul(t, t, ab[cc][:, 1:2].bitcast(f32))
                ba.append(t)
            b2 = {}
            for wname, w_sb, b_sb in (("q", wq_sb, bq_sb), ("k", wk_sb, bk_sb),
                                      ("v", wv_sb, bv_sb)):
                b2t = wk.tile([128, 2], f32, tag=f"b2{wname}", name=f"b2{wname}",
                              bufs=1)
                for hp in range(2):
                    wb_ps = ps.tile([128, 1], f32, tag="work", bufs=3,
                                    name=f"wb_{wname}{hp}")
                    for cc in range(2):
                        nc.tensor.matmul(
                            wb_ps,
                            w_sb[:, cc, hp * 128:(hp + 1) * 128].bitcast(f32),
                            ba[cc],
                            start=(cc == 0), stop=(cc == 1))
                    nc.vector.tensor_add(b2t[:, hp:hp + 1], b_sb[:, hp:hp + 1],
                                         wb_ps)
                b2[wname] = b2t
            pb2 = wk.tile([128, 2], f32, tag="pb2", name="pb2", bufs=1)
            for cc in range(2):
                pb_ps = ps.tile([128, 1], f32, tag="work", bufs=3,
                                name=f"pb_ps{cc}")
                for hpp in range(2):
                    nc.tensor.matmul(
                        pb_ps,
                        wp_sb[:, hpp, cc * 128:(cc + 1) * 128].bitcast(f32),
                        b2["v"][:, hpp:hpp + 1],
                        start=(hpp == 0), stop=(hpp == 1))
                nc.vector.tensor_add(pb2[:, cc:cc + 1], pb_sb[:, cc:cc + 1],
                                     pb_ps)

            # ---------------- projections (from raw x, folded weights) -----
            # Q first (scores need it for every key tile)
            for hp in range(2):
                for ch in range(2):
                    pq = ps.tile([128, 512], f32, tag="work", bufs=3,
                                 name=f"pq{hp}_{ch}")
                    for cc in range(2):
                        nc.tensor.matmul(
                            pq,
                            wq_sb[:, cc, hp * 128:(hp + 1) * 128],
                            xq[cc][:, ch * 512:(ch + 1) * 512],
                            start=(cc == 0), stop=(cc == 1))
                    nc.vector.tensor_scalar_add(
                        Q_sb[hp][:, ch * 512:(ch + 1) * 512], pq,
                        b2["q"][:, hp:hp + 1])
            def k_chunk(hp, ch):
                pk = ps.tile([128, 512], f32, tag="work", bufs=3,
                             name=f"pk{hp}_{ch}")
                for cc in range(2):
                    nc.tensor.matmul(
                        pk,
                        wk_sb[:, cc, hp * 128:(hp + 1) * 128],
                        xf[cc][:, ch * 512:(ch + 1) * 512],
                        start=(cc == 0), stop=(cc == 1))
                nc.vector.tensor_scalar_add(
                    K_sb[hp][:, ch * 512:(ch + 1) * 512], pk,
                    b2["k"][:, hp:hp + 1])

            def v_chunk2(tt0):
                # two token-tiles per psum tile (halves work-slot pressure)
                pv = ps.tile([128, 512], f32, tag="work", bufs=3,
                             name=f"pv{tt0}")
                for j in range(2):
                    tt = tt0 + j
                    for cc in range(2):
                        nc.tensor.matmul(
                            pv[:, j * 256:(j + 1) * 256],
                            xf[cc][:, tt * 128:(tt + 1) * 128],
                            wv_sb[:, cc, :],
                            start=(cc == 0), stop=(cc == 1))
                nc.vector.tensor_copy(
                    vt[:, tt0:tt0 + 2, :, 0:HD],
                    pv.rearrange("p (t h e) -> p t h e", t=2, e=HD))

            k_chunk(0, 0)
            v_chunk2(0)

        # ---------------- attention: 4 phases of (head-pair, query-half) ----
        # h accumulators are (65, 512) = 1 PSUM bank each, leaving the shared
        # "work" tag 3 slots. Phases are software-pipelined: each phase's
        # drain chain is emitted after the next phase's first two score/exp
        # iterations so ACT never waits on the boundary; AV lags two tiles.
        PHASES = [(0, 0), (0, 1), (1, 0), (1, 1)]
        with tc.tile_pool(name="atp", bufs=3) as atp, \
             tc.tile_pool(name="rbp", bufs=1) as rbp:

            def make_drain(hp, qc, hA, hB):
                def drain():
                    qs = slice(qc * 512, (qc + 1) * 512)
                    rsA = rbp.tile([1, 512], f32r, tag="rsA",
                                   name=f"rsA{hp}{qc}", bufs=1)
                    nc.vector.tensor_copy(rsA, hA[64:65, :])
                    rsB = rbp.tile([1, 512], f32r, tag="rsB",
                                   name=f"rsB{hp}{qc}", bufs=1)
                    nc.vector.tensor_copy(rsB, hB[64:65, :])
                    # broadcast raw rowsums across partitions (K=1 matmul),
                    # then reciprocal over all 128 partitions at once
                    bbA = ps.tile([64, 512], f32, tag="work", bufs=3,
                                  name=f"bbA{hp}{qc}")
                    nc.tensor.matmul(bbA, ones1, rsA, start=True, stop=True)
                    bbB = ps.tile([64, 512], f32, tag="work", bufs=3,
                                  name=f"bbB{hp}{qc}")
                    nc.tensor.matmul(bbB, ones1, rsB, start=True, stop=True)
                    rb = rbp.tile([128, 512], f32, tag="rb",
                                  name=f"rb{hp}{qc}", bufs=1)
                    nc.vector.reciprocal(rb[0:64, :], bbA)
                    nc.vector.reciprocal(rb[64:128, :], bbB)
                    nc.vector.tensor_mul(hnT[hp][0:64, qs], hA[0:64, :],
                                         rb[0:64, :])
                    nc.vector.tensor_mul(hnT[hp][64:128, qs], hB[0:64, :],
                                         rb[64:128, :])
                    if debug_taps and hp == 0 and qc == 1:
                        nc.sync.dma_start(out=dbg["rb0"][:, qs], in_=rb)
                        nc.sync.dma_start(out=dbg["hn0"],
                                          in_=hnT[0].bitcast(f32))
                    # output projection for this query-half once both
                    # head-pairs are done (overlaps the next phase)
                    if hp == 1:
                        for cc in range(2):
                            op = ps.tile([128, 512], f32, tag="work", bufs=3,
                                         name=f"op{cc}_{qc}")
                            for hpp in range(2):
                                nc.tensor.matmul(
                                    op,
                                    wp_sb[:, hpp, cc * 128:(cc + 1) * 128],
                                    hnT[hpp][:, qs],
                                    start=(hpp == 0), stop=(hpp == 1))
                            osb = sing.tile([128, NQ], f32, tag=f"os{cc}",
                                            name=f"os{cc}_{qc}")
                            nc.vector.scalar_tensor_tensor(
                                osb[:, qs], op, pb2[:, cc:cc + 1],
                                xq[cc][:, qs].bitcast(f32), A.add, A.add)
                            nc.sync.dma_start(
                                out=d_out[cc * 128:(cc + 1) * 128, qs],
                                in_=osb[:, qs])
                return drain

            pending = None
            for hp, qc in PHASES:
                qs = slice(qc * 512, (qc + 1) * 512)
                hA = ps.tile([65, 512], f32, tag="hA", bufs=1,
                             name=f"hA{hp}_{qc}")
                hB = ps.tile([65, 512], f32, tag="hB", bufs=1,
                             name=f"hB{hp}_{qc}")
                ats = {}
                for kt in range(32):
                    at = atp.tile([128, 1024], f32r, tag="at",
                                  name=f"at{hp}_{qc}_{kt}")
                    sc = ps.tile([128, 1024], f32, tag="work", bufs=3,
                                 name=f"sc{hp}_{qc}_{kt}")
                    for sub in range(2):
                        nc.tensor.matmul(
                            sc[:, sub * 512:(sub + 1) * 512],
                            K_sb[hp][sub * 64:(sub + 1) * 64,
                                     kt * 128:(kt + 1) * 128],
                            Q_sb[hp][sub * 64:(sub + 1) * 64, qs],
                            start=True, stop=True)
                    nc.scalar.activation(at, sc, AF.Exp, bias=m0c, scale=SCALE)
                    ats[kt] = at
                    if debug_taps and hp == 0 and qc == 0 and kt == 0:
                        nc.sync.dma_start(out=dbg["at00"][:, 0:1024],
                                          in_=at.bitcast(f32))
                    if kt == 1 and pending is not None:
                        pending()
                        pending = None
                    if kt >= 2:
                        _av(hA, hB, vt, ats.pop(kt - 2), hp, kt - 2)
                    # just-in-time projection work rides the ACT-bound loop
                    if hp == 0 and qc == 0:
                        if kt % 2 == 0 and kt < 30:
                            v_chunk2(kt + 2)
                        if kt % 4 == 0 and kt // 4 + 1 <= 7:
                            k_chunk(0, kt // 4 + 1)
                    if hp == 0 and qc == 1 and kt % 4 == 0 and kt // 4 < 8:
                        k_chunk(1, kt // 4)
                _av(hA, hB, vt, ats.pop(30), hp, 30)
                _av(hA, hB, vt, ats.pop(31), hp, 31)
                pending = make_drain(hp, qc, hA, hB)
            pending()

        if debug_taps:
            nc.sync.dma_start(out=dbg["K0"], in_=K_sb[0].bitcast(f32))
            nc.sync.dma_start(out=dbg["Q0"], in_=Q_sb[0].bitcast(f32))
            nc.sync.dma_start(out=dbg["vt"], in_=vt.bitcast(f32))

    with tile.TileContext(nc) as tc:
        for _ in range(reps):
            with ExitStack() as ctx:
                body(ctx, tc)
    nc.compile()
    return nc


def _prep_in_maps(inputs: dict) -> list:
    x = np.ascontiguousarray(np.asarray(inputs["x"], dtype=np.float32))
    norm_w = np.asarray(inputs["norm_w"], dtype=np.float32)
    norm_b = np.asarray(inputs["norm_b"], dtype=np.float32)
    qkv_w = np.asarray(inputs["qkv_w"], dtype=np.float32)
    qkv_b = np.asarray(inputs["qkv_b"], dtype=np.float32)
    proj_w = np.asarray(inputs["proj_w"], dtype=np.float32)
    proj_b = np.asarray(inputs["proj_b"], dtype=np.float32)

    xr = x.reshape(B, C, N)
    wq_t = np.ascontiguousarray(qkv_w[0:C].T)
    wk_t = np.ascontiguousarray(qkv_w[C:2 * C].T)
    wv_t = np.ascontiguousarray(qkv_w[2 * C:3 * C].T)
    wp_t = np.ascontiguousarray(proj_w.T)

    sm = np.zeros((128, 28), np.float32)
    sm[:, 0:2] = qkv_b[0:C].reshape(2, 128).T
    sm[:, 2:4] = qkv_b[C:2 * C].reshape(2, 128).T
    sm[:, 4:6] = qkv_b[2 * C:3 * C].reshape(2, 128).T
    sm[:, 6:8] = norm_w.reshape(2, 128).T
    sm[:, 8:10] = norm_b.reshape(2, 128).T
    sm[:, 10:12] = proj_b.reshape(2, 128).T
    cgrp = np.arange(C) // (C // G)
    gm3 = (cgrp.reshape(2, 128)[:, :, None] == np.arange(8)[None, None, :])
    sm[:, 12:28] = gm3.transpose(1, 0, 2).reshape(128, 16).astype(np.float32)
    gmask_t = np.ascontiguousarray(
        (np.arange(8)[:, None] == cgrp[None, :]).astype(np.float32))

    shared = dict(wq_t=wq_t, wk_t=wk_t, wv_t=wv_t, wp_t=wp_t,
                  smalls=sm, gmask_t=gmask_t)
    in_maps = []
    for core in range(N_CORES):
        b = core // 4
        qo = (core % 4) * NQ
        m = dict(shared)
        # rotate tokens so this core's queries sit at columns 0:NQ --
        # attention is permutation-equivariant over keys, so this is exact
        m["x_full"] = np.ascontiguousarray(np.roll(xr[b], -qo, axis=1))
        in_maps.append(m)
    return in_maps


def kernel(**inputs) -> np.ndarray:
    from concourse.bass_utils import run_bass_kernel_spmd

    if "nc" not in _CACHE:
        _CACHE["nc"] = _build()
    nc = _CACHE["nc"]

    in_maps = _prep_in_maps(inputs)
    res = run_bass_kernel_spmd(nc, in_maps, core_ids=list(range(N_CORES)))

    out = np.empty((B, C, N), dtype=np.float32)
    for core in range(N_CORES):
        b = core // 4
        qo = (core % 4) * NQ
        out[b][:, qo:qo + NQ] = res.results[core]["out"]
    return out.reshape(B, C, 16, 16, 16)


# revision 44
# speedup vs baseline: 1.3082x; 1.0100x over previous
"""Trainium2 Bass kernel for the AttentionBlock problem.

Sharding (8 cores): core = 4*b + qi  (b = batch, qi = query-quarter).
Each core:
  - GroupNorm(8, C) stats over its batch's full (C=256, N=4096) activations,
    folded into the QKV weights (W' = W @ diag(a), b' = b + W @ beta) so the
    normalized activations are never materialized
  - K/V projections for all 4096 tokens (duplicated per batch pair of cores)
  - Q projection for its 1024 queries
  - attention (4 heads) for its 1024 queries against all 4096 keys
  - output projection + bias + residual for its disjoint (256, 1024) slice
Host unshard = pure concatenation of the 8 disjoint output slices.

Softmax uses a constant shift M0 (softmax is invariant to per-row constant
shifts; a global constant is exact in exact arithmetic and fp32-safe here:
scaled scores lie in [-16.5, 13.3] and the shifted exponentials stay well
inside fp32 range). Row-sums fall out of the attention-value matmul via a
ones-column appended to V; normalization and the V bias are applied after.
"""

import os
import sys

# The grading environment may pin JAX_PLATFORMS=cpu for the reference; the
# bass execution path needs the axon/neuron PJRT devices.
if os.environ.get("JAX_PLATFORMS", "").strip() == "cpu":
    del os.environ["JAX_PLATFORMS"]

for _p in ("/opt/trn_rl_repo",):
    if os.path.isdir(_p) and _p not in sys.path:
        sys.path.insert(0, _p)

import numpy as np

B = 2
C = 256
N = 4096
NQ = 1024  # queries per core
NH = 4
HD = 64
G = 8
EPS = 1e-5
SCALE = HD ** -0.5
M0 = 16.0  # constant softmax shift (in scaled-score units)
N_CORES = 8

_CACHE: dict = {}


def _build(debug_taps=False, reps=1):
    from contextlib import ExitStack

    import concourse.bass as bass
    import concourse.tile as tile
    from concourse import bacc, mybir

    f32 = mybir.dt.float32
    f32r = mybir.dt.float32r
    A = mybir.AluOpType
    AF = mybir.ActivationFunctionType

    nc = bacc.Bacc("TRN2", target_bir_lowering=False, debug=False,
                   num_devices=N_CORES)

    d_xf = nc.dram_tensor("x_full", [C, N], f32r, kind="ExternalInput").ap()
    d_wq = nc.dram_tensor("wq_t", [C, C], f32r, kind="ExternalInput").ap()
    d_wk = nc.dram_tensor("wk_t", [C, C], f32r, kind="ExternalInput").ap()
    d_wv = nc.dram_tensor("wv_t", [C, C], f32r, kind="ExternalInput").ap()
    d_wp = nc.dram_tensor("wp_t", [C, C], f32r, kind="ExternalInput").ap()
    d_sm = nc.dram_tensor("smalls", [128, 28], f32, kind="ExternalInput").ap()
    d_gmt = nc.dram_tensor("gmask_t", [8, C], f32, kind="ExternalInput").ap()
    d_out = nc.dram_tensor("out", [C, NQ], f32, kind="ExternalOutput").ap()
    dbg = {}
    if debug_taps:
        dbg["K0"] = nc.dram_tensor("dbg_K0", [128, N], f32, kind="ExternalOutput").ap()
        dbg["Q0"] = nc.dram_tensor("dbg_Q0", [128, NQ], f32, kind="ExternalOutput").ap()
        dbg["vt"] = nc.dram_tensor("dbg_vt", [128, 32, NH, HD + 1], f32, kind="ExternalOutput").ap()
        dbg["at00"] = nc.dram_tensor("dbg_at00", [128, 2048], f32, kind="ExternalOutput").ap()
        dbg["hA0"] = nc.dram_tensor("dbg_hA0", [65, NQ], f32, kind="ExternalOutput").ap()
        dbg["rsA0"] = nc.dram_tensor("dbg_rsA0", [1, NQ], f32, kind="ExternalOutput").ap()
        dbg["hn0"] = nc.dram_tensor("dbg_hn0", [128, NQ], f32, kind="ExternalOutput").ap()
        dbg["rb0"] = nc.dram_tensor("dbg_rb0", [128, NQ], f32, kind="ExternalOutput").ap()

    def _av(hA, hB, vt, at, hp, kt):
        nc.tensor.matmul(
            hA, vt[:, kt, 2 * hp, :], at[:, 0:512],
            start=(kt == 0), stop=(kt == 31))
        nc.tensor.matmul(
            hB, vt[:, kt, 2 * hp + 1, :], at[:, 512:1024],
            start=(kt == 0), stop=(kt == 31))

    def body(ctx: ExitStack, tc: tile.TileContext):
        sing = ctx.enter_context(tc.tile_pool(name="sing", bufs=1))
        wk = ctx.enter_context(tc.tile_pool(name="wk", bufs=2))

        # ---------------- loads ----------------
        # DMA order matters (serial HBM bandwidth + ~0.6us HWDGE cost per
        # dma_start): one packed constants transfer, then x (paces the stats
        # chain), then weights in the order the fold needs them.
        sm_sb = sing.tile([128, 28], f32, tag="sm_sb", name="sm_sb")
        nc.sync.dma_start(out=sm_sb, in_=d_sm)
        gmt_sb = sing.tile([8, C], f32, tag="gmt_sb", name="gmt_sb")
        nc.sync.dma_start(out=gmt_sb, in_=d_gmt)
        bq_sb = sm_sb[:, 0:2]
        bk_sb = sm_sb[:, 2:4]
        bv_sb = sm_sb[:, 4:6]
        nw_sb = sm_sb[:, 6:8]
        nb_sb = sm_sb[:, 8:10]
        pb_sb = sm_sb[:, 10:12]
        gm_sb = sm_sb[:, 12:28]

        xf = []
        for h in range(2):
            t = sing.tile([128, N], f32r, tag=f"xf{h}", name=f"xf{h}")
            for chk in range(4):
                nc.sync.dma_start(
                    out=t[:, chk * 1024:(chk + 1) * 1024],
                    in_=d_xf[h * 128:(h + 1) * 128, chk * 1024:(chk + 1) * 1024])
            xf.append(t)
        # queries are token-columns 0:1024 of the (host-rotated) x
        xq = [xf[0][:, 0:NQ], xf[1][:, 0:NQ]]

        def load_w(name, dram):
            t = sing.tile([128, 2, C], f32r, tag=name, name=name)
            nc.sync.dma_start(out=t, in_=dram.rearrange("(c p) o -> p c o", p=128))
            return t

        wq_sb = load_w("wq_sb", d_wq)
        wk_sb = load_w("wk_sb", d_wk)
        wv_sb = load_w("wv_sb", d_wv)
        wp_sb = load_w("wp_sb", d_wp)

        # V^T tiles, per-head with an appended ones column for row-sums
        vt = sing.tile([128, 32, NH, HD + 1], f32r, tag="vt", name="vt")
        nc.vector.memset(vt[:, :, :, HD:HD + 1].bitcast(f32), 1.0)

        epsc = sing.tile([128, 1], f32, tag="epsc", name="epsc")
        nc.vector.memset(epsc, EPS)
        m0c = sing.tile([128, 1], f32, tag="m0c", name="m0c")
        nc.vector.memset(m0c, -M0)
        ones1 = sing.tile([1, 64], f32r, tag="ones1", name="ones1")
        nc.vector.memset(ones1.bitcast(f32), 1.0)
        # preload the sqrt activation table while ACT is idle
        scratch = sing.tile([128, 1], f32, tag="scratch", name="scratch")
        nc.scalar.activation(scratch, epsc, AF.Sqrt, bias=epsc, scale=1.0)

        K_sb = [sing.tile([128, N], f32r, tag=f"K{hp}", name=f"K{hp}")
                for hp in range(2)]
        Q_sb = [sing.tile([128, NQ], f32r, tag=f"Qs{hp}", name=f"Qs{hp}")
                for hp in range(2)]
        hnT = [sing.tile([128, NQ], f32r, tag=f"hn{hp}", name=f"hn{hp}")
               for hp in range(2)]

        # ---------------- groupnorm stats -> folded into weights -----------
        ps = ctx.enter_context(tc.tile_pool(name="ps", bufs=1, space="PSUM"))
        if True:
            st_t = []
            for h in range(2):
                stats = wk.tile([128, 8, 6], f32, tag="stats", name=f"stats{h}")
                for sg in range(8):
                    nc.vector.bn_stats(stats[:, sg, :],
                                       xf[h][:, sg * 512:(sg + 1) * 512])
                mv = wk.tile([128, 2], f32, tag="mv", name=f"mv{h}")
                nc.vector.bn_aggr(mv, stats)
                st = wk.tile([128, 2], f32, tag="st", name=f"st{h}")
                nc.vector.tensor_copy(st[:, 0:1], mv[:, 0:1])
                tmp = wk.tile([128, 1], f32, tag="tmp1", name=f"tmp1_{h}")
                nc.vector.tensor_mul(tmp, mv[:, 0:1], mv[:, 0:1])
                nc.vector.tensor_add(st[:, 1:2], mv[:, 1:2], tmp)
                st_t.append(st)

            # per-group mean / rstd via mask matmul over channels
            g_ps = ps.tile([8, 2], f32, tag="work", bufs=3, name="g_ps")
            for h in range(2):
                nc.tensor.matmul(g_ps, gm_sb[:, h * 8:(h + 1) * 8], st_t[h],
                                 start=(h == 0), stop=(h == 1))
            gs2 = wk.tile([8, 2], f32, tag="gs2", name="gs2")
            nc.vector.tensor_scalar_mul(gs2, g_ps, 1.0 / 32.0)
            gt = wk.tile([8, 1], f32, tag="gt", name="gt")
            nc.vector.tensor_mul(gt, gs2[:, 0:1], gs2[:, 0:1])
            vg = wk.tile([8, 1], f32, tag="vg", name="vg")
            nc.vector.tensor_sub(vg, gs2[:, 1:2], gt)
            sq = wk.tile([8, 1], f32, tag="sq", name="sq")
            nc.scalar.activation(sq, vg, AF.Sqrt, bias=epsc[0:8], scale=1.0)
            # preload the exp table set (input dep on sq keeps it after the
            # real sqrt so the table sets load exactly once each)
            nc.scalar.activation(scratch[0:8], sq, AF.Exp, bias=m0c[0:8],
                                 scale=1.0)
            gsb = wk.tile([8, 2], f32, tag="gsb", name="gsb")
            nc.vector.tensor_copy(gsb[:, 0:1], gs2[:, 0:1])
            nc.vector.reciprocal(gsb[:, 1:2], sq)

            # per-channel affine a, beta (per half), as f32r for the fold
            ab = []
            for h in range(2):
                bc_ps = ps.tile([128, 2], f32, tag="work", bufs=3,
                                name=f"bc_ps{h}")
                nc.tensor.matmul(bc_ps, gmt_sb[:, h * 128:(h + 1) * 128], gsb,
                                 start=True, stop=True)
                abt = wk.tile([128, 2], f32r, tag="ab", name=f"ab{h}")
                nc.vector.tensor_mul(abt[:, 0:1], nw_sb[:, h:h + 1], bc_ps[:, 1:2])
                tmp2 = wk.tile([128, 1], f32, tag="tmp2", name=f"tmp2_{h}")
                nc.vector.tensor_mul(tmp2, bc_ps[:, 0:1], abt[:, 0:1].bitcast(f32))
                nc.vector.tensor_sub(abt[:, 1:2], nb_sb[:, h:h + 1], tmp2)
                ab.append(abt)

            # fold first (per weight, in the order the projections need
            # them), then bias corrections b2 = b + W'^T (beta/a) -- using the
            # folded weights keeps the fold off the critical path
            for w_sb in (wq_sb, wk_sb, wv_sb):
                for cc in range(2):
                    nc.vector.tensor_scalar_mul(w_sb[:, cc, :], w_sb[:, cc, :],
                                                ab[cc][:, 0:1].bitcast(f32))
            ba = []
            for cc in range(2):
                t = wk.tile([128, 1], f32, tag="ba", name=f"ba{cc}")
                nc.vector.reciprocal(t, ab[cc][:, 0:1].bitcast(f32))
                nc.vector.tensor_mul(t, t, ab[cc][:, 1:2].bitcast(f32))
                ba.append(t)
            b2 = {}
            for wname, w_sb, b_sb in (("q", wq_sb, bq_sb), ("k", wk_sb, bk_sb),
                                      ("v", wv_sb, bv_sb)):
                b2t = wk.tile([128, 2], f32, tag=f"b2{wname}", name=f"b2{wname}",
                              bufs=1)
                for hp in range(2):
                    wb_ps = ps.tile([128, 1], f32, tag="work", bufs=3,
                                    name=f"wb_{wname}{hp}")
                    for cc in range(2):
                        nc.tensor.matmul(
                            wb_ps,
                            w_sb[:, cc, hp * 128:(hp + 1) * 128].bitcast(f32),
                            ba[cc],
                            start=(cc == 0), stop=(cc == 1))
                    nc.vector.tensor_add(b2t[:, hp:hp + 1], b_sb[:, hp:hp + 1],
                                         wb_ps)
                b2[wname] = b2t
            pb2 = wk.tile([128, 2], f32, tag="pb2", name="pb2", bufs=1)
            for cc in range(2):
                pb_ps = ps.tile([128, 1], f32, tag="work", bufs=3,
                                name=f"pb_ps{cc}")
                for hpp in range(2):
                    nc.tensor.matmul(
                        pb_ps,
                        wp_sb[:, hpp, cc * 128:(cc + 1) * 128].bitcast(f32),
                        b2["v"][:, hpp:hpp + 1],
                        start=(hpp == 0), stop=(hpp == 1))
                nc.vector.tensor_add(pb2[:, cc:cc + 1], pb_sb[:, cc:cc + 1],
                                     pb_ps)

            # ---------------- projections (from raw x, folded weights) -----
            # Q first (scores need it for every key tile)
            for hp in range(2):
                for ch in range(2):
                    pq = ps.tile([128, 512], f32, tag="work", bufs=3,
                                 name=f"pq{hp}_{ch}")
                    for cc in range(2):
                        nc.tensor.matmul(
                            pq,
                            wq_sb[:, cc, hp * 128:(hp + 1) * 128],
                            xq[cc][:, ch * 512:(ch + 1) * 512],
                            start=(cc == 0), stop=(cc == 1))
                    nc.vector.tensor_scalar_add(
                        Q_sb[hp][:, ch * 512:(ch + 1) * 512], pq,
                        b2["q"][:, hp:hp + 1])
            def k_chunk(hp, ch):
                pk = ps.tile([128, 512], f32, tag="work", bufs=3,
                             name=f"pk{hp}_{ch}")
                for cc in range(2):
                    nc.tensor.matmul(
                        pk,
                        wk_sb[:, cc, hp * 128:(hp + 1) * 128],
                        xf[cc][:, ch * 512:(ch + 1) * 512],
                        start=(cc == 0), stop=(cc == 1))
                nc.vector.tensor_scalar_add(
                    K_sb[hp][:, ch * 512:(ch + 1) * 512], pk,
                    b2["k"][:, hp:hp + 1])

            def v_chunk2(tt0):
                # two token-tiles per psum tile (halves work-slot pressure)
                pv = ps.tile([128, 512], f32, tag="work", bufs=3,
                             name=f"pv{tt0}")
                for j in range(2):
                    tt = tt0 + j
                    for cc in range(2):
                        nc.tensor.matmul(
                            pv[:, j * 256:(j + 1) * 256],
                            xf[cc][:, tt * 128:(tt + 1) * 128],
                            wv_sb[:, cc, :],
                            start=(cc == 0), stop=(cc == 1))
                nc.vector.tensor_copy(
                    vt[:, tt0:tt0 + 2, :, 0:HD],
                    pv.rearrange("p (t h e) -> p t h e", t=2, e=HD))

            k_chunk(0, 0)
            v_chunk2(0)

        # ---------------- attention: 4 phases of (head-pair, query-half) ----
        # h accumulators are (65, 512) = 1 PSUM bank each, leaving the shared
        # "work" tag 3 slots. Phases are software-pipelined: each phase's
        # drain chain is emitted after the next phase's first two score/exp
        # iterations so ACT never waits on the boundary; AV lags two tiles.
        PHASES = [(0, 0), (0, 1), (1, 0), (1, 1)]
        with tc.tile_pool(name="atp", bufs=3) as atp, \
             tc.tile_pool(name="rbp", bufs=1) as rbp:

            def make_drain(hp, qc, hA, hB):
                def drain():
                    qs = slice(qc * 512, (qc + 1) * 512)
                    rsA = rbp.tile([1, 512], f32r, tag="rsA",
                                   name=f"rsA{hp}{qc}", bufs=1)
                    nc.vector.tensor_copy(rsA, hA[64:65, :])
                    rsB = rbp.tile([1, 512], f32r, tag="rsB",
                                   name=f"rsB{hp}{qc}", bufs=1)
                    nc.vector.tensor_copy(rsB, hB[64:65, :])
                    # broadcast raw rowsums across partitions (K=1 matmul),
                    # then reciprocal over all 128 partitions at once
                    bbA = ps.tile([64, 512], f32, tag="work", bufs=3,
                                  name=f"bbA{hp}{qc}")
                    nc.tensor.matmul(bbA, ones1, rsA, start=True, stop=True)
                    bbB = ps.tile([64, 512], f32, tag="work", bufs=3,
                                  name=f"bbB{hp}{qc}")
                    nc.tensor.matmul(bbB, ones1, rsB, start=True, stop=True)
                    rb = rbp.tile([128, 512], f32, tag="rb",
                                  name=f"rb{hp}{qc}", bufs=1)
                    nc.vector.reciprocal(rb[0:64, :], bbA)
                    nc.vector.reciprocal(rb[64:128, :], bbB)
                    nc.vector.tensor_mul(hnT[hp][0:64, qs], hA[0:64, :],
                                         rb[0:64, :])
                    nc.vector.tensor_mul(hnT[hp][64:128, qs], hB[0:64, :],
                                         rb[64:128, :])
                    if debug_taps and hp == 0 and qc == 1:
                        nc.sync.dma_start(out=dbg["rb0"][:, qs], in_=rb)
                        nc.sync.dma_start(out=dbg["hn0"],
                                          in_=hnT[0].bitcast(f32))
                    # output projection for this query-half once both
                    # head-pairs are done (overlaps the next phase)
                    if hp == 1:
                        for cc in range(2):
                            op = ps.tile([128, 512], f32, tag="work", bufs=3,
                                         name=f"op{cc}_{qc}")
                            for hpp in range(2):
                                nc.tensor.matmul(
                                    op,
                                    wp_sb[:, hpp, cc * 128:(cc + 1) * 128],
                                    hnT[hpp][:, qs],
                                    start=(hpp == 0), stop=(hpp == 1))
                            osb = sing.tile([128, NQ], f32, tag=f"os{cc}",
                                            name=f"os{cc}_{qc}")
                            nc.vector.scalar_tensor_tensor(
                                osb[:, qs], op, pb2[:, cc:cc + 1],
                                xq[cc][:, qs].bitcast(f32), A.add, A.add)
                            nc.sync.dma_start(
                                out=d_out[cc * 128:(cc + 1) * 128, qs],
                                in_=osb[:, qs])
                return drain

            pending = None
            for hp, qc in PHASES:
                qs = slice(qc * 512, (qc + 1) * 512)
                hA = ps.tile([65, 512], f32, tag="hA", bufs=1,
                             name=f"hA{hp}_{qc}")
                hB = ps.tile([65, 512], f32, tag="hB", bufs=1,
                             name=f"hB{hp}_{qc}")
                ats = {}
                for kt in range(32):
                    at = atp.tile([128, 1024], f32r, tag="at",
                                  name=f"at{hp}_{qc}_{kt}")
                    sc = ps.tile([128, 1024], f32, tag="work", bufs=3,
                                 name=f"sc{hp}_{qc}_{kt}")
                    for sub in range(2):
                        nc.tensor.matmul(
                            sc[:, sub * 512:(sub + 1) * 512],
                            K_sb[hp][sub * 64:(sub + 1) * 64,
                                     kt * 128:(kt + 1) * 128],
                            Q_sb[hp][sub * 64:(sub + 1) * 64, qs],
                            start=True, stop=True)
                    nc.scalar.activation(at, sc, AF.Exp, bias=m0c, scale=SCALE)
                    ats[kt] = at
                    if debug_taps and hp == 0 and qc == 0 and kt == 0:
                        nc.sync.dma_start(out=dbg["at00"][:, 0:1024],
                                          in_=at.bitcast(f32))
                    if kt == 1 and pending is not None:
                        pending()
                        pending = None
                    if kt >= 2:
                        _av(hA, hB, vt, ats.pop(kt - 2), hp, kt - 2)
                    # just-in-time projection work rides the ACT-bound loop
                    if hp == 0 and qc == 0:
                        if kt % 2 == 0 and kt < 30:
                            v_chunk2(kt + 2)
                        if kt % 4 == 1 and kt // 4 + 1 <= 7:
                            k_chunk(0, kt // 4 + 1)
                    if hp == 0 and qc == 1 and kt % 4 == 1 and kt // 4 < 8:
                        k_chunk(1, kt // 4)
                _av(hA, hB, vt, ats.pop(30), hp, 30)
                _av(hA, hB, vt, ats.pop(31), hp, 31)
                pending = make_drain(hp, qc, hA, hB)
            pending()

        if debug_taps:
            nc.sync.dma_start(out=dbg["K0"], in_=K_sb[0].bitcast(f32))
            nc.sync.dma_start(out=dbg["Q0"], in_=Q_sb[0].bitcast(f32))
            nc.sync.dma_start(out=dbg["vt"], in_=vt.bitcast(f32))

    with tile.TileContext(nc) as tc:
        for _ in range(reps):
            with ExitStack() as ctx:
                body(ctx, tc)
    nc.compile()
    return nc


def _prep_in_maps(inputs: dict) -> list:
    x = np.ascontiguousarray(np.asarray(inputs["x"], dtype=np.float32))
    norm_w = np.asarray(inputs["norm_w"], dtype=np.float32)
    norm_b = np.asarray(inputs["norm_b"], dtype=np.float32)
    qkv_w = np.asarray(inputs["qkv_w"], dtype=np.float32)
    qkv_b = np.asarray(inputs["qkv_b"], dtype=np.float32)
    proj_w = np.asarray(inputs["proj_w"], dtype=np.float32)
    proj_b = np.asarray(inputs["proj_b"], dtype=np.float32)

    xr = x.reshape(B, C, N)
    wq_t = np.ascontiguousarray(qkv_w[0:C].T)
    wk_t = np.ascontiguousarray(qkv_w[C:2 * C].T)
    wv_t = np.ascontiguousarray(qkv_w[2 * C:3 * C].T)
    wp_t = np.ascontiguousarray(proj_w.T)

    sm = np.zeros((128, 28), np.float32)
    sm[:, 0:2] = qkv_b[0:C].reshape(2, 128).T
    sm[:, 2:4] = qkv_b[C:2 * C].reshape(2, 128).T
    sm[:, 4:6] = qkv_b[2 * C:3 * C].reshape(2, 128).T
    sm[:, 6:8] = norm_w.reshape(2, 128).T
    sm[:, 8:10] = norm_b.reshape(2, 128).T
    sm[:, 10:12] = proj_b.reshape(2, 128).T
    cgrp = np.arange(C) // (C // G)
    gm3 = (cgrp.reshape(2, 128)[:, :, None] == np.arange(8)[None, None, :])
    sm[:, 12:28] = gm3.transpose(1, 0, 2).reshape(128, 16).astype(np.float32)
    gmask_t = np.ascontiguousarray(
        (np.arange(8)[:, None] == cgrp[None, :]).astype(np.float32))

    shared = dict(wq_t=wq_t, wk_t=wk_t, wv_t=wv_t, wp_t=wp_t,
                  smalls=sm, gmask_t=gmask_t)
    in_maps = []
    for core in range(N_CORES):
        b = core // 4
        qo = (core % 4) * NQ
        m = dict(shared)
        # rotate tokens so this core's queries sit at columns 0:NQ --
        # attention is permutation-equivariant over keys, so this is exact
        m["x_full"] = np.ascontiguousarray(np.roll(xr[b], -qo, axis=1))
        in_maps.append(m)
    return in_maps


def kernel(**inputs) -> np.ndarray:
    from concourse.bass_utils import run_bass_kernel_spmd

    if "nc" not in _CACHE:
        _CACHE["nc"] = _build()
    nc = _CACHE["nc"]

    in_maps = _prep_in_maps(inputs)
    res = run_bass_kernel_spmd(nc, in_maps, core_ids=list(range(N_CORES)))

    out = np.empty((B, C, N), dtype=np.float32)
    for core in range(N_CORES):
        b = core // 4
        qo = (core % 4) * NQ
        out[b][:, qo:qo + NQ] = res.results[core]["out"]
    return out.reshape(B, C, 16, 16, 16)


# revision 45
# speedup vs baseline: 1.3094x; 1.0010x over previous
"""Trainium2 Bass kernel for the AttentionBlock problem.

Sharding (8 cores): core = 4*b + qi  (b = batch, qi = query-quarter).
Each core:
  - GroupNorm(8, C) stats over its batch's full (C=256, N=4096) activations,
    folded into the QKV weights (W' = W @ diag(a), b' = b + W @ beta) so the
    normalized activations are never materialized
  - K/V projections for all 4096 tokens (duplicated per batch pair of cores)
  - Q projection for its 1024 queries
  - attention (4 heads) for its 1024 queries against all 4096 keys
  - output projection + bias + residual for its disjoint (256, 1024) slice
Host unshard = pure concatenation of the 8 disjoint output slices.

Softmax uses a constant shift M0 (softmax is invariant to per-row constant
shifts; a global constant is exact in exact arithmetic and fp32-safe here:
scaled scores lie in [-16.5, 13.3] and the shifted exponentials stay well
inside fp32 range). Row-sums fall out of the attention-value matmul via a
ones-column appended to V; normalization and the V bias are applied after.
"""

import os
import sys

# The grading environment may pin JAX_PLATFORMS=cpu for the reference; the
# bass execution path needs the axon/neuron PJRT devices.
if os.environ.get("JAX_PLATFORMS", "").strip() == "cpu":
    del os.environ["JAX_PLATFORMS"]

for _p in ("/opt/trn_rl_repo",):
    if os.path.isdir(_p) and _p not in sys.path:
        sys.path.insert(0, _p)

import numpy as np

B = 2
C = 256
N = 4096
NQ = 1024  # queries per core
NH = 4
HD = 64
G = 8
EPS = 1e-5
SCALE = HD ** -0.5
M0 = 16.0  # constant softmax shift (in scaled-score units)
N_CORES = 8

_CACHE: dict = {}


def _build(debug_taps=False, reps=1):
    from contextlib import ExitStack

    import concourse.bass as bass
    import concourse.tile as tile
    from concourse import bacc, mybir

    f32 = mybir.dt.float32
    f32r = mybir.dt.float32r
    A = mybir.AluOpType
    AF = mybir.ActivationFunctionType

    nc = bacc.Bacc("TRN2", target_bir_lowering=False, debug=False,
                   num_devices=N_CORES)

    d_xf = nc.dram_tensor("x_full", [C, N], f32r, kind="ExternalInput").ap()
    d_wq = nc.dram_tensor("wq_t", [C, C], f32r, kind="ExternalInput").ap()
    d_wk = nc.dram_tensor("wk_t", [C, C], f32r, kind="ExternalInput").ap()
    d_wv = nc.dram_tensor("wv_t", [C, C], f32r, kind="ExternalInput").ap()
    d_wp = nc.dram_tensor("wp_t", [C, C], f32r, kind="ExternalInput").ap()
    d_sm = nc.dram_tensor("smalls", [128, 28], f32, kind="ExternalInput").ap()
    d_gmt = nc.dram_tensor("gmask_t", [8, C], f32, kind="ExternalInput").ap()
    d_out = nc.dram_tensor("out", [C, NQ], f32, kind="ExternalOutput").ap()
    dbg = {}
    if debug_taps:
        dbg["K0"] = nc.dram_tensor("dbg_K0", [128, N], f32, kind="ExternalOutput").ap()
        dbg["Q0"] = nc.dram_tensor("dbg_Q0", [128, NQ], f32, kind="ExternalOutput").ap()
        dbg["vt"] = nc.dram_tensor("dbg_vt", [128, 32, NH, HD + 1], f32, kind="ExternalOutput").ap()
        dbg["at00"] = nc.dram_tensor("dbg_at00", [128, 2048], f32, kind="ExternalOutput").ap()
        dbg["hA0"] = nc.dram_tensor("dbg_hA0", [65, NQ], f32, kind="ExternalOutput").ap()
        dbg["rsA0"] = nc.dram_tensor("dbg_rsA0", [1, NQ], f32, kind="ExternalOutput").ap()
        dbg["hn0"] = nc.dram_tensor("dbg_hn0", [128, NQ], f32, kind="ExternalOutput").ap()
        dbg["rb0"] = nc.dram_tensor("dbg_rb0", [128, NQ], f32, kind="ExternalOutput").ap()

    def _av(hA, hB, vt, at, hp, kt):
        nc.tensor.matmul(
            hA, vt[:, kt, 2 * hp, :], at[:, 0:512],
            start=(kt == 0), stop=(kt == 31))
        nc.tensor.matmul(
            hB, vt[:, kt, 2 * hp + 1, :], at[:, 512:1024],
            start=(kt == 0), stop=(kt == 31))

    def body(ctx: ExitStack, tc: tile.TileContext):
        sing = ctx.enter_context(tc.tile_pool(name="sing", bufs=1))
        wk = ctx.enter_context(tc.tile_pool(name="wk", bufs=2))

        # ---------------- loads ----------------
        # DMA order matters (serial HBM bandwidth + ~0.6us HWDGE cost per
        # dma_start): one packed constants transfer, then x (paces the stats
        # chain), then weights in the order the fold needs them.
        sm_sb = sing.tile([128, 28], f32, tag="sm_sb", name="sm_sb")
        nc.sync.dma_start(out=sm_sb, in_=d_sm)
        gmt_sb = sing.tile([8, C], f32, tag="gmt_sb", name="gmt_sb")
        nc.sync.dma_start(out=gmt_sb, in_=d_gmt)
        bq_sb = sm_sb[:, 0:2]
        bk_sb = sm_sb[:, 2:4]
        bv_sb = sm_sb[:, 4:6]
        nw_sb = sm_sb[:, 6:8]
        nb_sb = sm_sb[:, 8:10]
        pb_sb = sm_sb[:, 10:12]
        gm_sb = sm_sb[:, 12:28]

        xf = []
        for h in range(2):
            t = sing.tile([128, N], f32r, tag=f"xf{h}", name=f"xf{h}")
            for chk in range(4):
                nc.sync.dma_start(
                    out=t[:, chk * 1024:(chk + 1) * 1024],
                    in_=d_xf[h * 128:(h + 1) * 128, chk * 1024:(chk + 1) * 1024])
            xf.append(t)
        # queries are token-columns 0:1024 of the (host-rotated) x
        xq = [xf[0][:, 0:NQ], xf[1][:, 0:NQ]]

        def load_w(name, dram):
            t = sing.tile([128, 2, C], f32r, tag=name, name=name)
            nc.sync.dma_start(out=t, in_=dram.rearrange("(c p) o -> p c o", p=128))
            return t

        wq_sb = load_w("wq_sb", d_wq)
        wk_sb = load_w("wk_sb", d_wk)
        wv_sb = load_w("wv_sb", d_wv)
        wp_sb = load_w("wp_sb", d_wp)

        # V^T tiles, per-head with an appended ones column for row-sums
        vt = sing.tile([128, 32, NH, HD + 1], f32r, tag="vt", name="vt")
        nc.vector.memset(vt[:, :, :, HD:HD + 1].bitcast(f32), 1.0)

        epsc = sing.tile([128, 1], f32, tag="epsc", name="epsc")
        nc.vector.memset(epsc, EPS)
        m0c = sing.tile([128, 1], f32, tag="m0c", name="m0c")
        nc.vector.memset(m0c, -M0)
        ones1 = sing.tile([1, 64], f32r, tag="ones1", name="ones1")
        nc.vector.memset(ones1.bitcast(f32), 1.0)
        # preload the sqrt activation table while ACT is idle
        scratch = sing.tile([128, 1], f32, tag="scratch", name="scratch")
        nc.scalar.activation(scratch, epsc, AF.Sqrt, bias=epsc, scale=1.0)

        K_sb = [sing.tile([128, N], f32r, tag=f"K{hp}", name=f"K{hp}")
                for hp in range(2)]
        Q_sb = [sing.tile([128, NQ], f32r, tag=f"Qs{hp}", name=f"Qs{hp}")
                for hp in range(2)]
        hnT = [sing.tile([128, NQ], f32r, tag=f"hn{hp}", name=f"hn{hp}")
               for hp in range(2)]

        # ---------------- groupnorm stats -> folded into weights -----------
        ps = ctx.enter_context(tc.tile_pool(name="ps", bufs=1, space="PSUM"))
        if True:
            st_t = []
            for h in range(2):
                stats = wk.tile([128, 8, 6], f32, tag="stats", name=f"stats{h}")
                for sg in range(8):
                    nc.vector.bn_stats(stats[:, sg, :],
                                       xf[h][:, sg * 512:(sg + 1) * 512])
                mv = wk.tile([128, 2], f32, tag="mv", name=f"mv{h}")
                nc.vector.bn_aggr(mv, stats)
                st = wk.tile([128, 2], f32, tag="st", name=f"st{h}")
                nc.vector.tensor_copy(st[:, 0:1], mv[:, 0:1])
                tmp = wk.tile([128, 1], f32, tag="tmp1", name=f"tmp1_{h}")
                nc.vector.tensor_mul(tmp, mv[:, 0:1], mv[:, 0:1])
                nc.vector.tensor_add(st[:, 1:2], mv[:, 1:2], tmp)
                st_t.append(st)

            # per-group mean / rstd via mask matmul over channels
            g_ps = ps.tile([8, 2], f32, tag="work", bufs=3, name="g_ps")
            for h in range(2):
                nc.tensor.matmul(g_ps, gm_sb[:, h * 8:(h + 1) * 8], st_t[h],
                                 start=(h == 0), stop=(h == 1))
            gs2 = wk.tile([8, 2], f32, tag="gs2", name="gs2")
            nc.vector.tensor_scalar_mul(gs2, g_ps, 1.0 / 32.0)
            gt = wk.tile([8, 1], f32, tag="gt", name="gt")
            nc.vector.tensor_mul(gt, gs2[:, 0:1], gs2[:, 0:1])
            vg = wk.tile([8, 1], f32, tag="vg", name="vg")
            nc.vector.tensor_sub(vg, gs2[:, 1:2], gt)
            sq = wk.tile([8, 1], f32, tag="sq", name="sq")
            nc.scalar.activation(sq, vg, AF.Sqrt, bias=epsc[0:8], scale=1.0)
            # preload the exp table set (input dep on sq keeps it after the
            # real sqrt so the table sets load exactly once each)
            nc.scalar.activation(scratch[0:8], sq, AF.Exp, bias=m0c[0:8],
                                 scale=1.0)
            gsb = wk.tile([8, 2], f32, tag="gsb", name="gsb")
            nc.vector.tensor_copy(gsb[:, 0:1], gs2[:, 0:1])
            nc.vector.reciprocal(gsb[:, 1:2], sq)

            # per-channel affine a, beta (per half), as f32r for the fold
            ab = []
            for h in range(2):
                bc_ps = ps.tile([128, 2], f32, tag="work", bufs=3,
                                name=f"bc_ps{h}")
                nc.tensor.matmul(bc_ps, gmt_sb[:, h * 128:(h + 1) * 128], gsb,
                                 start=True, stop=True)
                abt = wk.tile([128, 2], f32r, tag="ab", name=f"ab{h}")
                nc.vector.tensor_mul(abt[:, 0:1], nw_sb[:, h:h + 1], bc_ps[:, 1:2])
                tmp2 = wk.tile([128, 1], f32, tag="tmp2", name=f"tmp2_{h}")
                nc.vector.tensor_mul(tmp2, bc_ps[:, 0:1], abt[:, 0:1].bitcast(f32))
                nc.vector.tensor_sub(abt[:, 1:2], nb_sb[:, h:h + 1], tmp2)
                ab.append(abt)

            # fold first (per weight, in the order the projections need
            # them), then bias corrections b2 = b + W'^T (beta/a) -- using the
            # folded weights keeps the fold off the critical path
            for w_sb in (wq_sb, wk_sb, wv_sb):
                for cc in range(2):
                    nc.vector.tensor_scalar_mul(w_sb[:, cc, :], w_sb[:, cc, :],
                                                ab[cc][:, 0:1].bitcast(f32))
            ba = []
            for cc in range(2):
                t = wk.tile([128, 1], f32, tag="ba", name=f"ba{cc}")
                nc.vector.reciprocal(t, ab[cc][:, 0:1].bitcast(f32))
                nc.vector.tensor_mul(t, t, ab[cc][:, 1:2].bitcast(f32))
                ba.append(t)
            b2 = {}
            for wname, w_sb, b_sb in (("q", wq_sb, bq_sb), ("k", wk_sb, bk_sb),
                                      ("v", wv_sb, bv_sb)):
                b2t = wk.tile([128, 2], f32, tag=f"b2{wname}", name=f"b2{wname}",
                              bufs=1)
                for hp in range(2):
                    wb_ps = ps.tile([128, 1], f32, tag="work", bufs=3,
                                    name=f"wb_{wname}{hp}")
                    for cc in range(2):
                        nc.tensor.matmul(
                            wb_ps,
                            w_sb[:, cc, hp * 128:(hp + 1) * 128].bitcast(f32),
                            ba[cc],
                            start=(cc == 0), stop=(cc == 1))
                    nc.vector.tensor_add(b2t[:, hp:hp + 1], b_sb[:, hp:hp + 1],
                                         wb_ps)
                b2[wname] = b2t
            pb2 = wk.tile([128, 2], f32, tag="pb2", name="pb2", bufs=1)
            for cc in range(2):
                pb_ps = ps.tile([128, 1], f32, tag="work", bufs=3,
                                name=f"pb_ps{cc}")
                for hpp in range(2):
                    nc.tensor.matmul(
                        pb_ps,
                        wp_sb[:, hpp, cc * 128:(cc + 1) * 128].bitcast(f32),
                        b2["v"][:, hpp:hpp + 1],
                        start=(hpp == 0), stop=(hpp == 1))
                nc.vector.tensor_add(pb2[:, cc:cc + 1], pb_sb[:, cc:cc + 1],
                                     pb_ps)

            # ---------------- projections (from raw x, folded weights) -----
            # Q first (scores need it for every key tile)
            for hp in range(2):
                for ch in range(2):
                    pq = ps.tile([128, 512], f32, tag="work", bufs=3,
                                 name=f"pq{hp}_{ch}")
                    for cc in range(2):
                        nc.tensor.matmul(
                            pq,
                            wq_sb[:, cc, hp * 128:(hp + 1) * 128],
                            xq[cc][:, ch * 512:(ch + 1) * 512],
                            start=(cc == 0), stop=(cc == 1))
                    nc.scalar.activation(
                        Q_sb[hp][:, ch * 512:(ch + 1) * 512], pq, AF.Identity,
                        bias=b2["q"][:, hp:hp + 1], scale=1.0)
            def k_chunk(hp, ch, on_act=False):
                pk = ps.tile([128, 512], f32, tag="work", bufs=3,
                             name=f"pk{hp}_{ch}")
                for cc in range(2):
                    nc.tensor.matmul(
                        pk,
                        wk_sb[:, cc, hp * 128:(hp + 1) * 128],
                        xf[cc][:, ch * 512:(ch + 1) * 512],
                        start=(cc == 0), stop=(cc == 1))
                if on_act:
                    nc.scalar.activation(
                        K_sb[hp][:, ch * 512:(ch + 1) * 512], pk, AF.Identity,
                        bias=b2["k"][:, hp:hp + 1], scale=1.0)
                else:
                    nc.vector.tensor_scalar_add(
                        K_sb[hp][:, ch * 512:(ch + 1) * 512], pk,
                        b2["k"][:, hp:hp + 1])

            def v_chunk2(tt0):
                # two token-tiles per psum tile (halves work-slot pressure)
                pv = ps.tile([128, 512], f32, tag="work", bufs=3,
                             name=f"pv{tt0}")
                for j in range(2):
                    tt = tt0 + j
                    for cc in range(2):
                        nc.tensor.matmul(
                            pv[:, j * 256:(j + 1) * 256],
                            xf[cc][:, tt * 128:(tt + 1) * 128],
                            wv_sb[:, cc, :],
                            start=(cc == 0), stop=(cc == 1))
                nc.vector.tensor_copy(
                    vt[:, tt0:tt0 + 2, :, 0:HD],
                    pv.rearrange("p (t h e) -> p t h e", t=2, e=HD))

            k_chunk(0, 0, on_act=True)
            v_chunk2(0)

        # ---------------- attention: 4 phases of (head-pair, query-half) ----
        # h accumulators are (65, 512) = 1 PSUM bank each, leaving the shared
        # "work" tag 3 slots. Phases are software-pipelined: each phase's
        # drain chain is emitted after the next phase's first two score/exp
        # iterations so ACT never waits on the boundary; AV lags two tiles.
        PHASES = [(0, 0), (0, 1), (1, 0), (1, 1)]
        with tc.tile_pool(name="atp", bufs=3) as atp, \
             tc.tile_pool(name="rbp", bufs=1) as rbp:

            def make_drain(hp, qc, hA, hB):
                def drain():
                    qs = slice(qc * 512, (qc + 1) * 512)
                    rsA = rbp.tile([1, 512], f32r, tag="rsA",
                                   name=f"rsA{hp}{qc}", bufs=1)
                    nc.vector.tensor_copy(rsA, hA[64:65, :])
                    rsB = rbp.tile([1, 512], f32r, tag="rsB",
                                   name=f"rsB{hp}{qc}", bufs=1)
                    nc.vector.tensor_copy(rsB, hB[64:65, :])
                    # broadcast raw rowsums across partitions (K=1 matmul),
                    # then reciprocal over all 128 partitions at once
                    bbA = ps.tile([64, 512], f32, tag="work", bufs=3,
                                  name=f"bbA{hp}{qc}")
                    nc.tensor.matmul(bbA, ones1, rsA, start=True, stop=True)
                    bbB = ps.tile([64, 512], f32, tag="work", bufs=3,
                                  name=f"bbB{hp}{qc}")
                    nc.tensor.matmul(bbB, ones1, rsB, start=True, stop=True)
                    rb = rbp.tile([128, 512], f32, tag="rb",
                                  name=f"rb{hp}{qc}", bufs=1)
                    nc.vector.reciprocal(rb[0:64, :], bbA)
                    nc.vector.reciprocal(rb[64:128, :], bbB)
                    nc.vector.tensor_mul(hnT[hp][0:64, qs], hA[0:64, :],
                                         rb[0:64, :])
                    nc.vector.tensor_mul(hnT[hp][64:128, qs], hB[0:64, :],
                                         rb[64:128, :])
                    if debug_taps and hp == 0 and qc == 1:
                        nc.sync.dma_start(out=dbg["rb0"][:, qs], in_=rb)
                        nc.sync.dma_start(out=dbg["hn0"],
                                          in_=hnT[0].bitcast(f32))
                    # output projection for this query-half once both
                    # head-pairs are done (overlaps the next phase)
                    if hp == 1:
                        for cc in range(2):
                            op = ps.tile([128, 512], f32, tag="work", bufs=3,
                                         name=f"op{cc}_{qc}")
                            for hpp in range(2):
                                nc.tensor.matmul(
                                    op,
                                    wp_sb[:, hpp, cc * 128:(cc + 1) * 128],
                                    hnT[hpp][:, qs],
                                    start=(hpp == 0), stop=(hpp == 1))
                            osb = sing.tile([128, NQ], f32, tag=f"os{cc}",
                                            name=f"os{cc}_{qc}")
                            nc.vector.scalar_tensor_tensor(
                                osb[:, qs], op, pb2[:, cc:cc + 1],
                                xq[cc][:, qs].bitcast(f32), A.add, A.add)
                            nc.sync.dma_start(
                                out=d_out[cc * 128:(cc + 1) * 128, qs],
                                in_=osb[:, qs])
                return drain

            pending = None
            for hp, qc in PHASES:
                qs = slice(qc * 512, (qc + 1) * 512)
                hA = ps.tile([65, 512], f32, tag="hA", bufs=1,
                             name=f"hA{hp}_{qc}")
                hB = ps.tile([65, 512], f32, tag="hB", bufs=1,
                             name=f"hB{hp}_{qc}")
                ats = {}
                for kt in range(32):
                    at = atp.tile([128, 1024], f32r, tag="at",
                                  name=f"at{hp}_{qc}_{kt}")
                    sc = ps.tile([128, 1024], f32, tag="work", bufs=3,
                                 name=f"sc{hp}_{qc}_{kt}")
                    for sub in range(2):
                        nc.tensor.matmul(
                            sc[:, sub * 512:(sub + 1) * 512],
                            K_sb[hp][sub * 64:(sub + 1) * 64,
                                     kt * 128:(kt + 1) * 128],
                            Q_sb[hp][sub * 64:(sub + 1) * 64, qs],
                            start=True, stop=True)
                    nc.scalar.activation(at, sc, AF.Exp, bias=m0c, scale=SCALE)
                    ats[kt] = at
                    if debug_taps and hp == 0 and qc == 0 and kt == 0:
                        nc.sync.dma_start(out=dbg["at00"][:, 0:1024],
                                          in_=at.bitcast(f32))
                    if kt == 1 and pending is not None:
                        pending()
                        pending = None
                    if kt >= 2:
                        _av(hA, hB, vt, ats.pop(kt - 2), hp, kt - 2)
                    # just-in-time projection work rides the ACT-bound loop
                    if hp == 0 and qc == 0:
                        if kt % 2 == 0 and kt < 30:
                            v_chunk2(kt + 2)
                        if kt % 4 == 1 and kt // 4 + 1 <= 7:
                            k_chunk(0, kt // 4 + 1)
                    if hp == 0 and qc == 1 and kt % 4 == 1 and kt // 4 < 8:
                        k_chunk(1, kt // 4)
                _av(hA, hB, vt, ats.pop(30), hp, 30)
                _av(hA, hB, vt, ats.pop(31), hp, 31)
                pending = make_drain(hp, qc, hA, hB)
            pending()

        if debug_taps:
            nc.sync.dma_start(out=dbg["K0"], in_=K_sb[0].bitcast(f32))
            nc.sync.dma_start(out=dbg["Q0"], in_=Q_sb[0].bitcast(f32))
            nc.sync.dma_start(out=dbg["vt"], in_=vt.bitcast(f32))

    with tile.TileContext(nc) as tc:
        for _ in range(reps):
            with ExitStack() as ctx:
                body(ctx, tc)
    nc.compile()
    return nc


def _prep_in_maps(inputs: dict) -> list:
    x = np.ascontiguousarray(np.asarray(inputs["x"], dtype=np.float32))
    norm_w = np.asarray(inputs["norm_w"], dtype=np.float32)
    norm_b = np.asarray(inputs["norm_b"], dtype=np.float32)
    qkv_w = np.asarray(inputs["qkv_w"], dtype=np.float32)
    qkv_b = np.asarray(inputs["qkv_b"], dtype=np.float32)
    proj_w = np.asarray(inputs["proj_w"], dtype=np.float32)
    proj_b = np.asarray(inputs["proj_b"], dtype=np.float32)

    xr = x.reshape(B, C, N)
    wq_t = np.ascontiguousarray(qkv_w[0:C].T)
    wk_t = np.ascontiguousarray(qkv_w[C:2 * C].T)
    wv_t = np.ascontiguousarray(qkv_w[2 * C:3 * C].T)
    wp_t = np.ascontiguousarray(proj_w.T)

    sm = np.zeros((128, 28), np.float32)
    sm[:, 0:2] = qkv_b[0:C].reshape(2, 128).T
    sm[:, 2:4] = qkv_b[C:2 * C].reshape(2, 128).T
    sm[:, 4:6] = qkv_b[2 * C:3 * C].reshape(2, 128).T
    sm[:, 6:8] = norm_w.reshape(2, 128).T
    sm[:, 8:10] = norm_b.reshape(2, 128).T
    sm[:, 10:12] = proj_b.reshape(2, 128).T
    cgrp = np.arange(C) // (C // G)
    gm3 = (cgrp.reshape(2, 128)[:, :, None] == np.arange(8)[None, None, :])
    sm[:, 12:28] = gm3.transpose(1, 0, 2).reshape(128, 16).astype(np.float32)
    gmask_t = np.ascontiguousarray(
        (np.arange(8)[:, None] == cgrp[None, :]).astype(np.float32))

    shared = dict(wq_t=wq_t, wk_t=wk_t, wv_t=wv_t, wp_t=wp_t,
                  smalls=sm, gmask_t=gmask_t)
    in_maps = []
    for core in range(N_CORES):
        b = core // 4
        qo = (core % 4) * NQ
        m = dict(shared)
        # rotate tokens so this core's queries sit at columns 0:NQ --
        # attention is permutation-equivariant over keys, so this is exact
        m["x_full"] = np.ascontiguousarray(np.roll(xr[b], -qo, axis=1))
        in_maps.append(m)
    return in_maps


def kernel(**inputs) -> np.ndarray:
    from concourse.bass_utils import run_bass_kernel_spmd

    if "nc" not in _CACHE:
        _CACHE["nc"] = _build()
    nc = _CACHE["nc"]

    in_maps = _prep_in_maps(inputs)
    res = run_bass_kernel_spmd(nc, in_maps, core_ids=list(range(N_CORES)))

    out = np.empty((B, C, N), dtype=np.float32)
    for core in range(N_CORES):
        b = core // 4
        qo = (core % 4) * NQ
        out[b][:, qo:qo + NQ] = res.results[core]["out"]
    return out.reshape(B, C, 16, 16, 16)


# revision 53
# speedup vs baseline: 1.3313x; 1.0167x over previous
"""Trainium2 Bass kernel for the AttentionBlock problem.

Sharding (8 cores): core = 4*b + qi  (b = batch, qi = query-quarter).
Each core:
  - GroupNorm(8, C) stats over its batch's full (C=256, N=4096) activations,
    folded into the QKV weights (W' = W @ diag(a), b' = b + W @ beta) so the
    normalized activations are never materialized
  - K/V projections for all 4096 tokens (duplicated per batch pair of cores)
  - Q projection for its 1024 queries
  - attention (4 heads) for its 1024 queries against all 4096 keys
  - output projection + bias + residual for its disjoint (256, 1024) slice
Host unshard = pure concatenation of the 8 disjoint output slices.

Softmax uses a constant shift M0 (softmax is invariant to per-row constant
shifts; a global constant is exact in exact arithmetic and fp32-safe here:
scaled scores lie in [-16.5, 13.3] and the shifted exponentials stay well
inside fp32 range). Row-sums fall out of the attention-value matmul via a
ones-column appended to V; normalization and the V bias are applied after.
"""

import os
import sys

# The grading environment may pin JAX_PLATFORMS=cpu for the reference; the
# bass execution path needs the axon/neuron PJRT devices.
if os.environ.get("JAX_PLATFORMS", "").strip() == "cpu":
    del os.environ["JAX_PLATFORMS"]

for _p in ("/opt/trn_rl_repo",):
    if os.path.isdir(_p) and _p not in sys.path:
        sys.path.insert(0, _p)

import numpy as np

B = 2
C = 256
N = 4096
NQ = 1024  # queries per core
NH = 4
HD = 64
G = 8
EPS = 1e-5
SCALE = HD ** -0.5
M0 = 16.0  # constant softmax shift (in scaled-score units)
N_CORES = 8

_CACHE: dict = {}


def _build(debug_taps=False, reps=1):
    from contextlib import ExitStack

    import concourse.bass as bass
    import concourse.tile as tile
    from concourse import bacc, mybir

    f32 = mybir.dt.float32
    f32r = mybir.dt.float32r
    A = mybir.AluOpType
    AF = mybir.ActivationFunctionType

    nc = bacc.Bacc("TRN2", target_bir_lowering=False, debug=False,
                   num_devices=N_CORES)

    d_xf = nc.dram_tensor("x_full", [C, N], f32r, kind="ExternalInput").ap()
    d_wq = nc.dram_tensor("wq_t", [C, C], f32r, kind="ExternalInput").ap()
    d_wk = nc.dram_tensor("wk_t", [C, C], f32r, kind="ExternalInput").ap()
    d_wv = nc.dram_tensor("wv_t", [C, C], f32r, kind="ExternalInput").ap()
    d_wp = nc.dram_tensor("wp_t", [C, C], f32r, kind="ExternalInput").ap()
    d_sm = nc.dram_tensor("smalls", [128, 28], f32, kind="ExternalInput").ap()
    d_gmt = nc.dram_tensor("gmask_t", [8, C], f32, kind="ExternalInput").ap()
    d_out = nc.dram_tensor("out", [C, NQ], f32, kind="ExternalOutput").ap()
    dbg = {}
    if debug_taps:
        dbg["K0"] = nc.dram_tensor("dbg_K0", [128, N], f32, kind="ExternalOutput").ap()
        dbg["Q0"] = nc.dram_tensor("dbg_Q0", [128, NQ], f32, kind="ExternalOutput").ap()
        dbg["vt"] = nc.dram_tensor("dbg_vt", [128, 32, NH, HD + 1], f32, kind="ExternalOutput").ap()
        dbg["at00"] = nc.dram_tensor("dbg_at00", [128, 2048], f32, kind="ExternalOutput").ap()
        dbg["hA0"] = nc.dram_tensor("dbg_hA0", [65, NQ], f32, kind="ExternalOutput").ap()
        dbg["rsA0"] = nc.dram_tensor("dbg_rsA0", [1, NQ], f32, kind="ExternalOutput").ap()
        dbg["hn0"] = nc.dram_tensor("dbg_hn0", [128, NQ], f32, kind="ExternalOutput").ap()
        dbg["rb0"] = nc.dram_tensor("dbg_rb0", [128, NQ], f32, kind="ExternalOutput").ap()

    def _av(hA, hB, vt, at, hp, kt):
        nc.tensor.matmul(
            hA, vt[:, kt, 2 * hp, :], at[:, 0:512],
            start=(kt == 0), stop=(kt == 31))
        nc.tensor.matmul(
            hB, vt[:, kt, 2 * hp + 1, :], at[:, 512:1024],
            start=(kt == 0), stop=(kt == 31))

    def body(ctx: ExitStack, tc: tile.TileContext):
        sing = ctx.enter_context(tc.tile_pool(name="sing", bufs=1))
        wk = ctx.enter_context(tc.tile_pool(name="wk", bufs=2))

        # ---------------- loads ----------------
        # DMA order matters (serial HBM bandwidth + ~0.6us HWDGE cost per
        # dma_start): one packed constants transfer, then x (paces the stats
        # chain), then weights in the order the fold needs them.
        sm_sb = sing.tile([128, 28], f32, tag="sm_sb", name="sm_sb")
        nc.sync.dma_start(out=sm_sb, in_=d_sm)
        gmt_sb = sing.tile([8, C], f32, tag="gmt_sb", name="gmt_sb")
        nc.sync.dma_start(out=gmt_sb, in_=d_gmt)
        bq_sb = sm_sb[:, 0:2]
        bk_sb = sm_sb[:, 2:4]
        bv_sb = sm_sb[:, 4:6]
        nw_sb = sm_sb[:, 6:8]
        nb_sb = sm_sb[:, 8:10]
        pb_sb = sm_sb[:, 10:12]
        gm_sb = sm_sb[:, 12:28]

        xf = []
        for h in range(2):
            t = sing.tile([128, N], f32r, tag=f"xf{h}", name=f"xf{h}")
            for chk in range(4):
                nc.sync.dma_start(
                    out=t[:, chk * 1024:(chk + 1) * 1024],
                    in_=d_xf[h * 128:(h + 1) * 128, chk * 1024:(chk + 1) * 1024])
            xf.append(t)
        # queries are token-columns 0:1024 of the (host-rotated) x
        xq = [xf[0][:, 0:NQ], xf[1][:, 0:NQ]]

        def load_w(name, dram):
            t = sing.tile([128, 2, C], f32r, tag=name, name=name)
            nc.sync.dma_start(out=t, in_=dram.rearrange("(c p) o -> p c o", p=128))
            return t

        wq_sb = load_w("wq_sb", d_wq)
        wk_sb = load_w("wk_sb", d_wk)
        wv_sb = load_w("wv_sb", d_wv)
        wp_sb = load_w("wp_sb", d_wp)

        # V^T tiles, per-head with an appended ones column for row-sums
        vt = sing.tile([128, 32, NH, HD + 1], f32r, tag="vt", name="vt")
        nc.vector.memset(vt[:, :, :, HD:HD + 1].bitcast(f32), 1.0)

        epsc = sing.tile([128, 1], f32, tag="epsc", name="epsc")
        nc.vector.memset(epsc, EPS)
        m0c = sing.tile([128, 1], f32, tag="m0c", name="m0c")
        nc.vector.memset(m0c, -M0)
        ones1 = sing.tile([1, 64], f32r, tag="ones1", name="ones1")
        nc.vector.memset(ones1.bitcast(f32), 1.0)
        # preload the sqrt activation table while ACT is idle
        scratch = sing.tile([128, 1], f32, tag="scratch", name="scratch")
        nc.scalar.activation(scratch, epsc, AF.Sqrt, bias=epsc, scale=1.0)

        K_sb = [sing.tile([128, N], f32r, tag=f"K{hp}", name=f"K{hp}")
                for hp in range(2)]
        Q_sb = [sing.tile([128, NQ], f32r, tag=f"Qs{hp}", name=f"Qs{hp}")
                for hp in range(2)]
        hnT = [sing.tile([128, NQ], f32r, tag=f"hn{hp}", name=f"hn{hp}")
               for hp in range(2)]

        # ---------------- groupnorm stats -> folded into weights -----------
        ps = ctx.enter_context(tc.tile_pool(name="ps", bufs=1, space="PSUM"))
        if True:
            st_t = []
            for h in range(2):
                stats = wk.tile([128, 8, 6], f32, tag="stats", name=f"stats{h}")
                for sg in range(8):
                    nc.vector.bn_stats(stats[:, sg, :],
                                       xf[h][:, sg * 512:(sg + 1) * 512])
                mv = wk.tile([128, 2], f32, tag="mv", name=f"mv{h}")
                nc.vector.bn_aggr(mv, stats)
                st = wk.tile([128, 2], f32, tag="st", name=f"st{h}")
                nc.vector.tensor_copy(st[:, 0:1], mv[:, 0:1])
                tmp = wk.tile([128, 1], f32, tag="tmp1", name=f"tmp1_{h}")
                nc.vector.tensor_mul(tmp, mv[:, 0:1], mv[:, 0:1])
                nc.vector.tensor_add(st[:, 1:2], mv[:, 1:2], tmp)
                st_t.append(st)

            # per-group mean / rstd via mask matmul over channels
            g_ps = ps.tile([8, 2], f32, tag="work", bufs=3, name="g_ps")
            for h in range(2):
                nc.tensor.matmul(g_ps, gm_sb[:, h * 8:(h + 1) * 8], st_t[h],
                                 start=(h == 0), stop=(h == 1))
            gs2 = wk.tile([8, 2], f32, tag="gs2", name="gs2")
            nc.vector.tensor_scalar_mul(gs2, g_ps, 1.0 / 32.0)
            gt = wk.tile([8, 1], f32, tag="gt", name="gt")
            nc.vector.tensor_mul(gt, gs2[:, 0:1], gs2[:, 0:1])
            vg = wk.tile([8, 1], f32, tag="vg", name="vg")
            nc.vector.tensor_sub(vg, gs2[:, 1:2], gt)
            sq = wk.tile([8, 1], f32, tag="sq", name="sq")
            nc.scalar.activation(sq, vg, AF.Sqrt, bias=epsc[0:8], scale=1.0)
            # preload the exp table set (input dep on sq keeps it after the
            # real sqrt so the table sets load exactly once each)
            nc.scalar.activation(scratch[0:8], sq, AF.Exp, bias=m0c[0:8],
                                 scale=1.0)
            gsb = wk.tile([8, 2], f32, tag="gsb", name="gsb")
            nc.vector.tensor_copy(gsb[:, 0:1], gs2[:, 0:1])
            nc.vector.reciprocal(gsb[:, 1:2], sq)

            # per-channel affine a, beta (per half), as f32r for the fold
            ab = []
            for h in range(2):
                bc_ps = ps.tile([128, 2], f32, tag="work", bufs=3,
                                name=f"bc_ps{h}")
                nc.tensor.matmul(bc_ps, gmt_sb[:, h * 128:(h + 1) * 128], gsb,
                                 start=True, stop=True)
                abt = wk.tile([128, 2], f32r, tag="ab", name=f"ab{h}")
                nc.vector.tensor_mul(abt[:, 0:1], nw_sb[:, h:h + 1], bc_ps[:, 1:2])
                tmp2 = wk.tile([128, 1], f32, tag="tmp2", name=f"tmp2_{h}")
                nc.vector.tensor_mul(tmp2, bc_ps[:, 0:1], abt[:, 0:1].bitcast(f32))
                nc.vector.tensor_sub(abt[:, 1:2], nb_sb[:, h:h + 1], tmp2)
                ab.append(abt)

            # fold first (per weight, in the order the projections need
            # them), then bias corrections b2 = b + W'^T (beta/a) -- using the
            # folded weights keeps the fold off the critical path
            for w_sb in (wq_sb, wk_sb, wv_sb):
                for cc in range(2):
                    nc.vector.tensor_scalar_mul(w_sb[:, cc, :], w_sb[:, cc, :],
                                                ab[cc][:, 0:1].bitcast(f32))
            ba = []
            for cc in range(2):
                t = wk.tile([128, 1], f32, tag="ba", name=f"ba{cc}")
                nc.vector.reciprocal(t, ab[cc][:, 0:1].bitcast(f32))
                nc.vector.tensor_mul(t, t, ab[cc][:, 1:2].bitcast(f32))
                ba.append(t)
            b2 = {}
            for wname, w_sb, b_sb in (("q", wq_sb, bq_sb), ("k", wk_sb, bk_sb),
                                      ("v", wv_sb, bv_sb)):
                b2t = wk.tile([128, 2], f32, tag=f"b2{wname}", name=f"b2{wname}",
                              bufs=1)
                for hp in range(2):
                    wb_ps = ps.tile([128, 1], f32, tag="work", bufs=3,
                                    name=f"wb_{wname}{hp}")
                    for cc in range(2):
                        nc.tensor.matmul(
                            wb_ps,
                            w_sb[:, cc, hp * 128:(hp + 1) * 128].bitcast(f32),
                            ba[cc],
                            start=(cc == 0), stop=(cc == 1))
                    nc.vector.tensor_add(b2t[:, hp:hp + 1], b_sb[:, hp:hp + 1],
                                         wb_ps)
                b2[wname] = b2t
            pb2 = wk.tile([128, 2], f32, tag="pb2", name="pb2", bufs=1)
            for cc in range(2):
                pb_ps = ps.tile([128, 1], f32, tag="work", bufs=3,
                                name=f"pb_ps{cc}")
                for hpp in range(2):
                    nc.tensor.matmul(
                        pb_ps,
                        wp_sb[:, hpp, cc * 128:(cc + 1) * 128].bitcast(f32),
                        b2["v"][:, hpp:hpp + 1],
                        start=(hpp == 0), stop=(hpp == 1))
                nc.vector.tensor_add(pb2[:, cc:cc + 1], pb_sb[:, cc:cc + 1],
                                     pb_ps)

            # ---------------- projections (from raw x, folded weights) -----
            # Q first (scores need it for every key tile)
            for hp in range(2):
                for ch in range(2):
                    pq = ps.tile([128, 512], f32, tag="work", bufs=3,
                                 name=f"pq{hp}_{ch}")
                    for cc in range(2):
                        nc.tensor.matmul(
                            pq,
                            wq_sb[:, cc, hp * 128:(hp + 1) * 128],
                            xq[cc][:, ch * 512:(ch + 1) * 512],
                            start=(cc == 0), stop=(cc == 1))
                    nc.scalar.activation(
                        Q_sb[hp][:, ch * 512:(ch + 1) * 512], pq, AF.Identity,
                        bias=b2["q"][:, hp:hp + 1], scale=1.0)
            def k_chunk(hp, ch, on_act=False):
                pk = ps.tile([128, 512], f32, tag="work", bufs=3,
                             name=f"pk{hp}_{ch}")
                for cc in range(2):
                    nc.tensor.matmul(
                        pk,
                        wk_sb[:, cc, hp * 128:(hp + 1) * 128],
                        xf[cc][:, ch * 512:(ch + 1) * 512],
                        start=(cc == 0), stop=(cc == 1))
                if on_act:
                    nc.scalar.activation(
                        K_sb[hp][:, ch * 512:(ch + 1) * 512], pk, AF.Identity,
                        bias=b2["k"][:, hp:hp + 1], scale=1.0)
                else:
                    nc.vector.tensor_scalar_add(
                        K_sb[hp][:, ch * 512:(ch + 1) * 512], pk,
                        b2["k"][:, hp:hp + 1])

            def v_chunk2(tt0):
                # two token-tiles per psum tile (halves work-slot pressure)
                pv = ps.tile([128, 512], f32, tag="work", bufs=3,
                             name=f"pv{tt0}")
                for j in range(2):
                    tt = tt0 + j
                    for cc in range(2):
                        nc.tensor.matmul(
                            pv[:, j * 256:(j + 1) * 256],
                            xf[cc][:, tt * 128:(tt + 1) * 128],
                            wv_sb[:, cc, :],
                            start=(cc == 0), stop=(cc == 1))
                nc.vector.tensor_copy(
                    vt[:, tt0:tt0 + 2, :, 0:HD],
                    pv.rearrange("p (t h e) -> p t h e", t=2, e=HD))

            k_chunk(0, 0, on_act=True)
            v_chunk2(0)

        # ---------------- attention: 4 phases of (head-pair, query-half) ----
        # h accumulators are (65, 512) = 1 PSUM bank each, leaving the shared
        # "work" tag 3 slots. Phases are software-pipelined: each phase's
        # drain chain is emitted after the next phase's first two score/exp
        # iterations so ACT never waits on the boundary; AV lags two tiles.
        PHASES = [(0, 0), (0, 1), (1, 0), (1, 1)]
        with tc.tile_pool(name="atp", bufs=4) as atp, \
             tc.tile_pool(name="rbp", bufs=1) as rbp:

            def make_drain(hp, qc, hA, hB, at30, at31, last=False):
                def drain():
                    qs = slice(qc * 512, (qc + 1) * 512)
                    _av(hA, hB, vt, at30, hp, 30)
                    _av(hA, hB, vt, at31, hp, 31)
                    rsA = rbp.tile([1, 512], f32r, tag="rsA",
                                   name=f"rsA{hp}{qc}", bufs=1)
                    if last:
                        # ACT is idle after the final exp; copy in parallel
                        nc.scalar.activation(rsA, hA[64:65, :], AF.Copy)
                    else:
                        nc.vector.tensor_copy(rsA, hA[64:65, :])
                    rsB = rbp.tile([1, 512], f32r, tag="rsB",
                                   name=f"rsB{hp}{qc}", bufs=1)
                    nc.vector.tensor_copy(rsB, hB[64:65, :])
                    # broadcast raw rowsums across partitions (K=1 matmul),
                    # then reciprocal over all 128 partitions at once
                    bbA = ps.tile([64, 512], f32, tag="work", bufs=3,
                                  name=f"bbA{hp}{qc}")
                    nc.tensor.matmul(bbA, ones1, rsA, start=True, stop=True)
                    bbB = ps.tile([64, 512], f32, tag="work", bufs=3,
                                  name=f"bbB{hp}{qc}")
                    nc.tensor.matmul(bbB, ones1, rsB, start=True, stop=True)
                    rb = rbp.tile([128, 512], f32, tag="rb",
                                  name=f"rb{hp}{qc}", bufs=1)
                    nc.vector.reciprocal(rb[0:64, :], bbA)
                    nc.vector.reciprocal(rb[64:128, :], bbB)
                    nc.vector.tensor_mul(hnT[hp][0:64, qs], hA[0:64, :],
                                         rb[0:64, :])
                    nc.vector.tensor_mul(hnT[hp][64:128, qs], hB[0:64, :],
                                         rb[64:128, :])
                    if debug_taps and hp == 0 and qc == 1:
                        nc.sync.dma_start(out=dbg["rb0"][:, qs], in_=rb)
                        nc.sync.dma_start(out=dbg["hn0"],
                                          in_=hnT[0].bitcast(f32))
                    return

                def proj_part():
                    qs = slice(qc * 512, (qc + 1) * 512)
                    if hp == 1:
                        for cc in range(2):
                            op = ps.tile([128, 512], f32, tag="work", bufs=3,
                                         name=f"op{cc}_{qc}")
                            for hpp in range(2):
                                nc.tensor.matmul(
                                    op,
                                    wp_sb[:, hpp, cc * 128:(cc + 1) * 128],
                                    hnT[hpp][:, qs],
                                    start=(hpp == 0), stop=(hpp == 1))
                            osb = sing.tile([128, NQ], f32, tag=f"os{cc}",
                                            name=f"os{cc}_{qc}")
                            nc.vector.scalar_tensor_tensor(
                                osb[:, qs], op, pb2[:, cc:cc + 1],
                                xq[cc][:, qs].bitcast(f32), A.add, A.add)
                            nc.sync.dma_start(
                                out=d_out[cc * 128:(cc + 1) * 128, qs],
                                in_=osb[:, qs])
                return drain, proj_part

            pending = None
            for hp, qc in PHASES:
                qs = slice(qc * 512, (qc + 1) * 512)
                hA = ps.tile([65, 512], f32, tag="hA", bufs=1,
                             name=f"hA{hp}_{qc}")
                hB = ps.tile([65, 512], f32, tag="hB", bufs=1,
                             name=f"hB{hp}_{qc}")
                ats = {}
                for kt in range(32):
                    at = atp.tile([128, 1024], f32r, tag="at",
                                  name=f"at{hp}_{qc}_{kt}")
                    sc = ps.tile([128, 1024], f32, tag="work", bufs=3,
                                 name=f"sc{hp}_{qc}_{kt}")
                    for sub in range(2):
                        nc.tensor.matmul(
                            sc[:, sub * 512:(sub + 1) * 512],
                            K_sb[hp][sub * 64:(sub + 1) * 64,
                                     kt * 128:(kt + 1) * 128],
                            Q_sb[hp][sub * 64:(sub + 1) * 64, qs],
                            start=True, stop=True)
                    nc.scalar.activation(at, sc, AF.Exp, bias=m0c, scale=SCALE)
                    ats[kt] = at
                    if debug_taps and hp == 0 and qc == 0 and kt == 0:
                        nc.sync.dma_start(out=dbg["at00"][:, 0:1024],
                                          in_=at.bitcast(f32))
                    if kt == 1 and pending is not None:
                        pending[0]()
                    if kt == 4 and pending is not None:
                        pending[1]()
                        pending = None
                    if kt >= 2:
                        _av(hA, hB, vt, ats.pop(kt - 2), hp, kt - 2)
                    # just-in-time projection work rides the ACT-bound loop
                    if hp == 0 and qc == 0:
                        if kt % 2 == 0 and kt < 30:
                            v_chunk2(kt + 2)
                        if kt % 4 == 1 and kt // 4 + 1 <= 7:
                            k_chunk(0, kt // 4 + 1)
                    if hp == 0 and qc == 1 and kt % 4 == 1 and kt // 4 < 8:
                        k_chunk(1, kt // 4)
                pending = make_drain(hp, qc, hA, hB, ats.pop(30),
                                     ats.pop(31), last=(hp, qc) == PHASES[-1])
            pending[0]()
            pending[1]()

        if debug_taps:
            nc.sync.dma_start(out=dbg["K0"], in_=K_sb[0].bitcast(f32))
            nc.sync.dma_start(out=dbg["Q0"], in_=Q_sb[0].bitcast(f32))
            nc.sync.dma_start(out=dbg["vt"], in_=vt.bitcast(f32))

    with tile.TileContext(nc) as tc:
        for _ in range(reps):
            with ExitStack() as ctx:
                body(ctx, tc)
    nc.compile()
    return nc


def _prep_in_maps(inputs: dict) -> list:
    x = np.ascontiguousarray(np.asarray(inputs["x"], dtype=np.float32))
    norm_w = np.asarray(inputs["norm_w"], dtype=np.float32)
    norm_b = np.asarray(inputs["norm_b"], dtype=np.float32)
    qkv_w = np.asarray(inputs["qkv_w"], dtype=np.float32)
    qkv_b = np.asarray(inputs["qkv_b"], dtype=np.float32)
    proj_w = np.asarray(inputs["proj_w"], dtype=np.float32)
    proj_b = np.asarray(inputs["proj_b"], dtype=np.float32)

    xr = x.reshape(B, C, N)
    wq_t = np.ascontiguousarray(qkv_w[0:C].T)
    wk_t = np.ascontiguousarray(qkv_w[C:2 * C].T)
    wv_t = np.ascontiguousarray(qkv_w[2 * C:3 * C].T)
    wp_t = np.ascontiguousarray(proj_w.T)

    sm = np.zeros((128, 28), np.float32)
    sm[:, 0:2] = qkv_b[0:C].reshape(2, 128).T
    sm[:, 2:4] = qkv_b[C:2 * C].reshape(2, 128).T
    sm[:, 4:6] = qkv_b[2 * C:3 * C].reshape(2, 128).T
    sm[:, 6:8] = norm_w.reshape(2, 128).T
    sm[:, 8:10] = norm_b.reshape(2, 128).T
    sm[:, 10:12] = proj_b.reshape(2, 128).T
    cgrp = np.arange(C) // (C // G)
    gm3 = (cgrp.reshape(2, 128)[:, :, None] == np.arange(8)[None, None, :])
    sm[:, 12:28] = gm3.transpose(1, 0, 2).reshape(128, 16).astype(np.float32)
    gmask_t = np.ascontiguousarray(
        (np.arange(8)[:, None] == cgrp[None, :]).astype(np.float32))

    shared = dict(wq_t=wq_t, wk_t=wk_t, wv_t=wv_t, wp_t=wp_t,
                  smalls=sm, gmask_t=gmask_t)
    in_maps = []
    for core in range(N_CORES):
        b = core // 4
        qo = (core % 4) * NQ
        m = dict(shared)
        # rotate tokens so this core's queries sit at columns 0:NQ --
        # attention is permutation-equivariant over keys, so this is exact
        m["x_full"] = np.ascontiguousarray(np.roll(xr[b], -qo, axis=1))
        in_maps.append(m)
    return in_maps


def kernel(**inputs) -> np.ndarray:
    from concourse.bass_utils import run_bass_kernel_spmd

    if "nc" not in _CACHE:
        _CACHE["nc"] = _build()
    nc = _CACHE["nc"]

    in_maps = _prep_in_maps(inputs)
    res = run_bass_kernel_spmd(nc, in_maps, core_ids=list(range(N_CORES)))

    out = np.empty((B, C, N), dtype=np.float32)
    for core in range(N_CORES):
        b = core // 4
        qo = (core % 4) * NQ
        out[b][:, qo:qo + NQ] = res.results[core]["out"]
    return out.reshape(B, C, 16, 16, 16)


# revision 59
# speedup vs baseline: 1.3533x; 1.0165x over previous
"""Trainium2 Bass kernel for the AttentionBlock problem.

Sharding (8 cores): core = 4*b + qi  (b = batch, qi = query-quarter).
Each core:
  - GroupNorm(8, C) stats over its batch's full (C=256, N=4096) activations,
    folded into the QKV weights (W' = W @ diag(a), b' = b + W @ beta) so the
    normalized activations are never materialized
  - K/V projections for all 4096 tokens (duplicated per batch pair of cores)
  - Q projection for its 1024 queries
  - attention (4 heads) for its 1024 queries against all 4096 keys
  - output projection + bias + residual for its disjoint (256, 1024) slice
Host unshard = pure concatenation of the 8 disjoint output slices.

Softmax uses a constant shift M0 (softmax is invariant to per-row constant
shifts; a global constant is exact in exact arithmetic and fp32-safe here:
scaled scores lie in [-16.5, 13.3] and the shifted exponentials stay well
inside fp32 range). Row-sums fall out of the attention-value matmul via a
ones-column appended to V; normalization and the V bias are applied after.
"""

import os
import sys

# The grading environment may pin JAX_PLATFORMS=cpu for the reference; the
# bass execution path needs the axon/neuron PJRT devices.
if os.environ.get("JAX_PLATFORMS", "").strip() == "cpu":
    del os.environ["JAX_PLATFORMS"]

for _p in ("/opt/trn_rl_repo",):
    if os.path.isdir(_p) and _p not in sys.path:
        sys.path.insert(0, _p)

import numpy as np

B = 2
C = 256
N = 4096
NQ = 1024  # queries per core
NH = 4
HD = 64
G = 8
EPS = 1e-5
SCALE = HD ** -0.5
M0 = 16.0  # constant softmax shift (in scaled-score units)
N_CORES = 8

_CACHE: dict = {}


def _build(debug_taps=False, reps=1):
    from contextlib import ExitStack

    import concourse.bass as bass
    import concourse.tile as tile
    from concourse import bacc, mybir

    f32 = mybir.dt.float32
    f32r = mybir.dt.float32r
    f16 = mybir.dt.float16
    A = mybir.AluOpType
    AF = mybir.ActivationFunctionType

    nc = bacc.Bacc("TRN2", target_bir_lowering=False, debug=False,
                   num_devices=N_CORES)

    d_xf = nc.dram_tensor("x_full", [C, N], f16, kind="ExternalInput").ap()
    d_xq = nc.dram_tensor("x_q", [C, NQ], f32, kind="ExternalInput").ap()
    d_wq = nc.dram_tensor("wq_t", [C, C], f16, kind="ExternalInput").ap()
    d_wk = nc.dram_tensor("wk_t", [C, C], f16, kind="ExternalInput").ap()
    d_wv = nc.dram_tensor("wv_t", [C, C], f16, kind="ExternalInput").ap()
    d_wp = nc.dram_tensor("wp_t", [C, C], f32r, kind="ExternalInput").ap()
    d_sm = nc.dram_tensor("smalls", [128, 28], f32, kind="ExternalInput").ap()
    d_gmt = nc.dram_tensor("gmask_t", [8, C], f32, kind="ExternalInput").ap()
    d_out = nc.dram_tensor("out", [C, NQ], f32, kind="ExternalOutput").ap()
    dbg = {}
    if debug_taps:
        dbg["K0"] = nc.dram_tensor("dbg_K0", [128, N], f32, kind="ExternalOutput").ap()
        dbg["Q0"] = nc.dram_tensor("dbg_Q0", [128, NQ], f32, kind="ExternalOutput").ap()
        dbg["vt"] = nc.dram_tensor("dbg_vt", [128, 32, NH, HD + 1], f32, kind="ExternalOutput").ap()
        dbg["at00"] = nc.dram_tensor("dbg_at00", [128, 2048], f32, kind="ExternalOutput").ap()
        dbg["hA0"] = nc.dram_tensor("dbg_hA0", [65, NQ], f32, kind="ExternalOutput").ap()
        dbg["rsA0"] = nc.dram_tensor("dbg_rsA0", [1, NQ], f32, kind="ExternalOutput").ap()
        dbg["hn0"] = nc.dram_tensor("dbg_hn0", [128, NQ], f32, kind="ExternalOutput").ap()
        dbg["rb0"] = nc.dram_tensor("dbg_rb0", [128, NQ], f32, kind="ExternalOutput").ap()

    def _av(hA, hB, vt, at, hp, kt):
        nc.tensor.matmul(
            hA, vt[:, kt, 2 * hp, :], at[:, 0:512],
            start=(kt == 0), stop=(kt == 31))
        nc.tensor.matmul(
            hB, vt[:, kt, 2 * hp + 1, :], at[:, 512:1024],
            start=(kt == 0), stop=(kt == 31))

    def body(ctx: ExitStack, tc: tile.TileContext):
        sing = ctx.enter_context(tc.tile_pool(name="sing", bufs=1))
        wk = ctx.enter_context(tc.tile_pool(name="wk", bufs=2))

        # ---------------- loads ----------------
        # DMA order matters (serial HBM bandwidth + ~0.6us HWDGE cost per
        # dma_start): one packed constants transfer, then x (paces the stats
        # chain), then weights in the order the fold needs them.
        sm_sb = sing.tile([128, 28], f32, tag="sm_sb", name="sm_sb")
        nc.sync.dma_start(out=sm_sb, in_=d_sm)
        gmt_sb = sing.tile([8, C], f32, tag="gmt_sb", name="gmt_sb")
        nc.sync.dma_start(out=gmt_sb, in_=d_gmt)
        bq_sb = sm_sb[:, 0:2]
        bk_sb = sm_sb[:, 2:4]
        bv_sb = sm_sb[:, 4:6]
        nw_sb = sm_sb[:, 6:8]
        nb_sb = sm_sb[:, 8:10]
        pb_sb = sm_sb[:, 10:12]
        gm_sb = sm_sb[:, 12:28]

        xf = []
        for h in range(2):
            t = sing.tile([128, N], f16, tag=f"xf{h}", name=f"xf{h}")
            for chk in range(4):
                nc.sync.dma_start(
                    out=t[:, chk * 1024:(chk + 1) * 1024],
                    in_=d_xf[h * 128:(h + 1) * 128, chk * 1024:(chk + 1) * 1024])
            xf.append(t)
        # queries are token-columns 0:1024 of the (host-rotated) x
        xq = [xf[0][:, 0:NQ], xf[1][:, 0:NQ]]

        def load_w(name, dram, dt_):
            t = sing.tile([128, 2, C], dt_, tag=name, name=name)
            nc.sync.dma_start(out=t, in_=dram.rearrange("(c p) o -> p c o", p=128))
            return t

        wq_sb = load_w("wq_sb", d_wq, f16)
        wk_sb = load_w("wk_sb", d_wk, f16)
        wv_sb = load_w("wv_sb", d_wv, f16)
        wp_sb = load_w("wp_sb", d_wp, f32r)
        # fp32 residual slice, only needed at the very end
        xq32 = []
        for h in range(2):
            t = sing.tile([128, NQ], f32, tag=f"xq32_{h}", name=f"xq32_{h}")
            nc.sync.dma_start(out=t, in_=d_xq[h * 128:(h + 1) * 128, :])
            xq32.append(t)

        # V^T tiles, per-head with an appended ones column for row-sums
        vt = sing.tile([128, 32, NH, HD + 1], f32r, tag="vt", name="vt")
        nc.vector.memset(vt[:, :, :, HD:HD + 1].bitcast(f32), 1.0)

        epsc = sing.tile([128, 1], f32, tag="epsc", name="epsc")
        nc.vector.memset(epsc, EPS)
        m0c = sing.tile([128, 1], f32, tag="m0c", name="m0c")
        nc.vector.memset(m0c, -M0)
        ones1 = sing.tile([1, 64], f32r, tag="ones1", name="ones1")
        nc.vector.memset(ones1.bitcast(f32), 1.0)
        # preload the sqrt activation table while ACT is idle
        scratch = sing.tile([128, 1], f32, tag="scratch", name="scratch")
        nc.scalar.activation(scratch, epsc, AF.Sqrt, bias=epsc, scale=1.0)

        K_sb = [sing.tile([128, N], f16, tag=f"K{hp}", name=f"K{hp}")
                for hp in range(2)]
        Q_sb = [sing.tile([128, NQ], f16, tag=f"Qs{hp}", name=f"Qs{hp}")
                for hp in range(2)]
        hnT = [sing.tile([128, NQ], f32r, tag=f"hn{hp}", name=f"hn{hp}")
               for hp in range(2)]

        # ---------------- groupnorm stats -> folded into weights -----------
        ps = ctx.enter_context(tc.tile_pool(name="ps", bufs=1, space="PSUM"))
        if True:
            st_t = []
            for h in range(2):
                stats = wk.tile([128, 8, 6], f32, tag="stats", name=f"stats{h}")
                for sg in range(8):
                    nc.vector.bn_stats(stats[:, sg, :],
                                       xf[h][:, sg * 512:(sg + 1) * 512])
                mv = wk.tile([128, 2], f32, tag="mv", name=f"mv{h}")
                nc.vector.bn_aggr(mv, stats)
                st = wk.tile([128, 2], f32, tag="st", name=f"st{h}")
                nc.vector.tensor_copy(st[:, 0:1], mv[:, 0:1])
                tmp = wk.tile([128, 1], f32, tag="tmp1", name=f"tmp1_{h}")
                nc.vector.tensor_mul(tmp, mv[:, 0:1], mv[:, 0:1])
                nc.vector.tensor_add(st[:, 1:2], mv[:, 1:2], tmp)
                st_t.append(st)

            # per-group mean / rstd via mask matmul over channels
            g_ps = ps.tile([8, 2], f32, tag="work", bufs=3, name="g_ps")
            for h in range(2):
                nc.tensor.matmul(g_ps, gm_sb[:, h * 8:(h + 1) * 8], st_t[h],
                                 start=(h == 0), stop=(h == 1))
            gs2 = wk.tile([8, 2], f32, tag="gs2", name="gs2")
            nc.vector.tensor_scalar_mul(gs2, g_ps, 1.0 / 32.0)
            gt = wk.tile([8, 1], f32, tag="gt", name="gt")
            nc.vector.tensor_mul(gt, gs2[:, 0:1], gs2[:, 0:1])
            vg = wk.tile([8, 1], f32, tag="vg", name="vg")
            nc.vector.tensor_sub(vg, gs2[:, 1:2], gt)
            sq = wk.tile([8, 1], f32, tag="sq", name="sq")
            nc.scalar.activation(sq, vg, AF.Sqrt, bias=epsc[0:8], scale=1.0)
            # preload the exp table set (input dep on sq keeps it after the
            # real sqrt so the table sets load exactly once each)
            nc.scalar.activation(scratch[0:8], sq, AF.Exp, bias=m0c[0:8],
                                 scale=1.0)
            gsb = wk.tile([8, 2], f32, tag="gsb", name="gsb")
            nc.vector.tensor_copy(gsb[:, 0:1], gs2[:, 0:1])
            nc.vector.reciprocal(gsb[:, 1:2], sq)

            # per-channel affine a, beta (per half), as f32r for the fold
            ab = []
            for h in range(2):
                bc_ps = ps.tile([128, 2], f32, tag="work", bufs=3,
                                name=f"bc_ps{h}")
                nc.tensor.matmul(bc_ps, gmt_sb[:, h * 128:(h + 1) * 128], gsb,
                                 start=True, stop=True)
                abt = wk.tile([128, 2], f32r, tag="ab", name=f"ab{h}")
                nc.vector.tensor_mul(abt[:, 0:1], nw_sb[:, h:h + 1], bc_ps[:, 1:2])
                tmp2 = wk.tile([128, 1], f32, tag="tmp2", name=f"tmp2_{h}")
                nc.vector.tensor_mul(tmp2, bc_ps[:, 0:1], abt[:, 0:1].bitcast(f32))
                nc.vector.tensor_sub(abt[:, 1:2], nb_sb[:, h:h + 1], tmp2)
                ab.append(abt)

            # fold first (per weight, in the order the projections need
            # them), then bias corrections b2 = b + W'^T (beta/a) -- using the
            # folded weights keeps the fold off the critical path
            for w_sb in (wq_sb, wk_sb, wv_sb):
                for cc in range(2):
                    nc.vector.tensor_scalar_mul(w_sb[:, cc, :], w_sb[:, cc, :],
                                                ab[cc][:, 0:1].bitcast(f32))
            ba = []
            for cc in range(2):
                tr = wk.tile([128, 1], f32, tag="bar", name=f"bar{cc}")
                nc.vector.reciprocal(tr, ab[cc][:, 0:1].bitcast(f32))
                t = wk.tile([128, 1], f16, tag="ba", name=f"ba{cc}")
                nc.vector.tensor_mul(t, tr, ab[cc][:, 1:2].bitcast(f32))
                ba.append(t)
            b2 = {}
            for wname, w_sb, b_sb in (("q", wq_sb, bq_sb), ("k", wk_sb, bk_sb),
                                      ("v", wv_sb, bv_sb)):
                b2t = wk.tile([128, 2], f32, tag=f"b2{wname}", name=f"b2{wname}",
                              bufs=1)
                for hp in range(2):
                    wb_ps = ps.tile([128, 1], f32, tag="work", bufs=3,
                                    name=f"wb_{wname}{hp}")
                    for cc in range(2):
                        nc.tensor.matmul(
                            wb_ps,
                            w_sb[:, cc, hp * 128:(hp + 1) * 128],
                            ba[cc],
                            start=(cc == 0), stop=(cc == 1))
                    nc.vector.tensor_add(b2t[:, hp:hp + 1], b_sb[:, hp:hp + 1],
                                         wb_ps)
                b2[wname] = b2t
            pb2 = wk.tile([128, 2], f32, tag="pb2", name="pb2", bufs=1)
            for cc in range(2):
                pb_ps = ps.tile([128, 1], f32, tag="work", bufs=3,
                                name=f"pb_ps{cc}")
                for hpp in range(2):
                    nc.tensor.matmul(
                        pb_ps,
                        wp_sb[:, hpp, cc * 128:(cc + 1) * 128].bitcast(f32),
                        b2["v"][:, hpp:hpp + 1],
                        start=(hpp == 0), stop=(hpp == 1))
                nc.vector.tensor_add(pb2[:, cc:cc + 1], pb_sb[:, cc:cc + 1],
                                     pb_ps)

            # ---------------- projections (from raw x, folded weights) -----
            # Q first (scores need it for every key tile)
            for hp in range(2):
                for ch in range(2):
                    pq = ps.tile([128, 512], f32, tag="work", bufs=3,
                                 name=f"pq{hp}_{ch}")
                    for cc in range(2):
                        nc.tensor.matmul(
                            pq,
                            wq_sb[:, cc, hp * 128:(hp + 1) * 128],
                            xq[cc][:, ch * 512:(ch + 1) * 512],
                            start=(cc == 0), stop=(cc == 1))
                    nc.scalar.activation(
                        Q_sb[hp][:, ch * 512:(ch + 1) * 512], pq, AF.Identity,
                        bias=b2["q"][:, hp:hp + 1], scale=1.0)
            def k_chunk(hp, ch, on_act=False):
                pk = ps.tile([128, 512], f32, tag="work", bufs=3,
                             name=f"pk{hp}_{ch}")
                for cc in range(2):
                    nc.tensor.matmul(
                        pk,
                        wk_sb[:, cc, hp * 128:(hp + 1) * 128],
                        xf[cc][:, ch * 512:(ch + 1) * 512],
                        start=(cc == 0), stop=(cc == 1))
                if on_act:
                    nc.scalar.activation(
                        K_sb[hp][:, ch * 512:(ch + 1) * 512], pk, AF.Identity,
                        bias=b2["k"][:, hp:hp + 1], scale=1.0)
                else:
                    nc.vector.tensor_scalar_add(
                        K_sb[hp][:, ch * 512:(ch + 1) * 512], pk,
                        b2["k"][:, hp:hp + 1])

            def v_chunk2(tt0):
                # two token-tiles per psum tile (halves work-slot pressure)
                pv = ps.tile([128, 512], f32, tag="work", bufs=3,
                             name=f"pv{tt0}")
                for j in range(2):
                    tt = tt0 + j
                    for cc in range(2):
                        nc.tensor.matmul(
                            pv[:, j * 256:(j + 1) * 256],
                            xf[cc][:, tt * 128:(tt + 1) * 128],
                            wv_sb[:, cc, :],
                            start=(cc == 0), stop=(cc == 1))
                nc.vector.tensor_copy(
                    vt[:, tt0:tt0 + 2, :, 0:HD],
                    pv.rearrange("p (t h e) -> p t h e", t=2, e=HD))

            k_chunk(0, 0, on_act=True)
            v_chunk2(0)

        # ---------------- attention: 4 phases of (head-pair, query-half) ----
        # h accumulators are (65, 512) = 1 PSUM bank each, leaving the shared
        # "work" tag 3 slots. Phases are software-pipelined: each phase's
        # drain chain is emitted after the next phase's first two score/exp
        # iterations so ACT never waits on the boundary; AV lags two tiles.
        PHASES = [(0, 0), (0, 1), (1, 0), (1, 1)]
        with tc.tile_pool(name="atp", bufs=4) as atp, \
             tc.tile_pool(name="rbp", bufs=1) as rbp:

            def make_drain(hp, qc, hA, hB, at30, at31, last=False):
                def drain():
                    qs = slice(qc * 512, (qc + 1) * 512)
                    _av(hA, hB, vt, at30, hp, 30)
                    _av(hA, hB, vt, at31, hp, 31)
                    rsA = rbp.tile([1, 512], f32r, tag="rsA",
                                   name=f"rsA{hp}{qc}", bufs=1)
                    if last:
                        # ACT is idle after the final exp; copy in parallel
                        nc.scalar.activation(rsA, hA[64:65, :], AF.Copy)
                    else:
                        nc.vector.tensor_copy(rsA, hA[64:65, :])
                    rsB = rbp.tile([1, 512], f32r, tag="rsB",
                                   name=f"rsB{hp}{qc}", bufs=1)
                    nc.vector.tensor_copy(rsB, hB[64:65, :])
                    # broadcast raw rowsums across partitions (K=1 matmul),
                    # then reciprocal over all 128 partitions at once
                    bbA = ps.tile([64, 512], f32, tag="work", bufs=3,
                                  name=f"bbA{hp}{qc}")
                    nc.tensor.matmul(bbA, ones1, rsA, start=True, stop=True)
                    bbB = ps.tile([64, 512], f32, tag="work", bufs=3,
                                  name=f"bbB{hp}{qc}")
                    nc.tensor.matmul(bbB, ones1, rsB, start=True, stop=True)
                    rb = rbp.tile([128, 512], f32, tag="rb",
                                  name=f"rb{hp}{qc}", bufs=1)
                    nc.vector.reciprocal(rb[0:64, :], bbA)
                    nc.vector.reciprocal(rb[64:128, :], bbB)
                    nc.vector.tensor_mul(hnT[hp][0:64, qs], hA[0:64, :],
                                         rb[0:64, :])
                    nc.vector.tensor_mul(hnT[hp][64:128, qs], hB[0:64, :],
                                         rb[64:128, :])
                    if debug_taps and hp == 0 and qc == 1:
                        nc.sync.dma_start(out=dbg["rb0"][:, qs], in_=rb)
                        nc.sync.dma_start(out=dbg["hn0"],
                                          in_=hnT[0].bitcast(f32))
                    return

                def proj_part():
                    qs = slice(qc * 512, (qc + 1) * 512)
                    if hp == 1:
                        for cc in range(2):
                            op = ps.tile([128, 512], f32, tag="work", bufs=3,
                                         name=f"op{cc}_{qc}")
                            for hpp in range(2):
                                nc.tensor.matmul(
                                    op,
                                    wp_sb[:, hpp, cc * 128:(cc + 1) * 128],
                                    hnT[hpp][:, qs],
                                    start=(hpp == 0), stop=(hpp == 1))
                            osb = sing.tile([128, NQ], f32, tag=f"os{cc}",
                                            name=f"os{cc}_{qc}")
                            nc.vector.scalar_tensor_tensor(
                                osb[:, qs], op, pb2[:, cc:cc + 1],
                                xq32[cc][:, qs], A.add, A.add)
                            nc.sync.dma_start(
                                out=d_out[cc * 128:(cc + 1) * 128, qs],
                                in_=osb[:, qs])
                return drain, proj_part

            pending = None
            for hp, qc in PHASES:
                qs = slice(qc * 512, (qc + 1) * 512)
                hA = ps.tile([65, 512], f32, tag="hA", bufs=1,
                             name=f"hA{hp}_{qc}")
                hB = ps.tile([65, 512], f32, tag="hB", bufs=1,
                             name=f"hB{hp}_{qc}")
                ats = {}
                for kt in range(32):
                    at = atp.tile([128, 1024], f32r, tag="at",
                                  name=f"at{hp}_{qc}_{kt}")
                    sc = ps.tile([128, 1024], f32, tag="work", bufs=3,
                                 name=f"sc{hp}_{qc}_{kt}")
                    for sub in range(2):
                        nc.tensor.matmul(
                            sc[:, sub * 512:(sub + 1) * 512],
                            K_sb[hp][sub * 64:(sub + 1) * 64,
                                     kt * 128:(kt + 1) * 128],
                            Q_sb[hp][sub * 64:(sub + 1) * 64, qs],
                            start=True, stop=True)
                    nc.scalar.activation(at, sc, AF.Exp, bias=m0c, scale=SCALE)
                    ats[kt] = at
                    if debug_taps and hp == 0 and qc == 0 and kt == 0:
                        nc.sync.dma_start(out=dbg["at00"][:, 0:1024],
                                          in_=at.bitcast(f32))
                    if kt == 1 and pending is not None:
                        pending[0]()
                    if kt == 4 and pending is not None:
                        pending[1]()
                        pending = None
                    if kt >= 2:
                        _av(hA, hB, vt, ats.pop(kt - 2), hp, kt - 2)
                    # just-in-time projection work rides the ACT-bound loop
                    if hp == 0 and qc == 0:
                        if kt % 2 == 0 and kt < 30:
                            v_chunk2(kt + 2)
                        if kt % 4 == 1 and kt // 4 + 1 <= 7:
                            k_chunk(0, kt // 4 + 1)
                    if hp == 0 and qc == 1 and kt % 4 == 1 and kt // 4 < 8:
                        k_chunk(1, kt // 4)
                pending = make_drain(hp, qc, hA, hB, ats.pop(30),
                                     ats.pop(31), last=(hp, qc) == PHASES[-1])
            pending[0]()
            pending[1]()

        if debug_taps:
            nc.sync.dma_start(out=dbg["K0"], in_=K_sb[0].bitcast(f32))
            nc.sync.dma_start(out=dbg["Q0"], in_=Q_sb[0].bitcast(f32))
            nc.sync.dma_start(out=dbg["vt"], in_=vt.bitcast(f32))

    with tile.TileContext(nc) as tc:
        for _ in range(reps):
            with ExitStack() as ctx:
                body(ctx, tc)
    nc.compile()
    return nc


def _prep_in_maps(inputs: dict) -> list:
    x = np.ascontiguousarray(np.asarray(inputs["x"], dtype=np.float32))
    norm_w = np.asarray(inputs["norm_w"], dtype=np.float32)
    norm_b = np.asarray(inputs["norm_b"], dtype=np.float32)
    qkv_w = np.asarray(inputs["qkv_w"], dtype=np.float32)
    qkv_b = np.asarray(inputs["qkv_b"], dtype=np.float32)
    proj_w = np.asarray(inputs["proj_w"], dtype=np.float32)
    proj_b = np.asarray(inputs["proj_b"], dtype=np.float32)

    xr = x.reshape(B, C, N)
    wq_t = np.ascontiguousarray(qkv_w[0:C].T).astype(np.float16)
    wk_t = np.ascontiguousarray(qkv_w[C:2 * C].T).astype(np.float16)
    wv_t = np.ascontiguousarray(qkv_w[2 * C:3 * C].T).astype(np.float16)
    wp_t = np.ascontiguousarray(proj_w.T)

    sm = np.zeros((128, 28), np.float32)
    sm[:, 0:2] = qkv_b[0:C].reshape(2, 128).T
    sm[:, 2:4] = qkv_b[C:2 * C].reshape(2, 128).T
    sm[:, 4:6] = qkv_b[2 * C:3 * C].reshape(2, 128).T
    sm[:, 6:8] = norm_w.reshape(2, 128).T
    sm[:, 8:10] = norm_b.reshape(2, 128).T
    sm[:, 10:12] = proj_b.reshape(2, 128).T
    cgrp = np.arange(C) // (C // G)
    gm3 = (cgrp.reshape(2, 128)[:, :, None] == np.arange(8)[None, None, :])
    sm[:, 12:28] = gm3.transpose(1, 0, 2).reshape(128, 16).astype(np.float32)
    gmask_t = np.ascontiguousarray(
        (np.arange(8)[:, None] == cgrp[None, :]).astype(np.float32))

    shared = dict(wq_t=wq_t, wk_t=wk_t, wv_t=wv_t, wp_t=wp_t,
                  smalls=sm, gmask_t=gmask_t)
    in_maps = []
    for core in range(N_CORES):
        b = core // 4
        qo = (core % 4) * NQ
        m = dict(shared)
        # rotate tokens so this core's queries sit at columns 0:NQ --
        # attention is permutation-equivariant over keys, so this is exact
        xrot = np.ascontiguousarray(np.roll(xr[b], -qo, axis=1))
        m["x_full"] = xrot.astype(np.float16)
        m["x_q"] = np.ascontiguousarray(xrot[:, 0:NQ])
        in_maps.append(m)
    return in_maps


def kernel(**inputs) -> np.ndarray:
    from concourse.bass_utils import run_bass_kernel_spmd

    if "nc" not in _CACHE:
        _CACHE["nc"] = _build()
    nc = _CACHE["nc"]

    in_maps = _prep_in_maps(inputs)
    res = run_bass_kernel_spmd(nc, in_maps, core_ids=list(range(N_CORES)))

    out = np.empty((B, C, N), dtype=np.float32)
    for core in range(N_CORES):
        b = core // 4
        qo = (core % 4) * NQ
        out[b][:, qo:qo + NQ] = res.results[core]["out"]
    return out.reshape(B, C, 16, 16, 16)


# revision 63
# speedup vs baseline: 1.3582x; 1.0036x over previous
"""Trainium2 Bass kernel for the AttentionBlock problem.

Sharding (8 cores): core = 4*b + qi  (b = batch, qi = query-quarter).
Each core:
  - GroupNorm(8, C) stats over its batch's full (C=256, N=4096) activations,
    folded into the QKV weights (W' = W @ diag(a), b' = b + W @ beta) so the
    normalized activations are never materialized
  - K/V projections for all 4096 tokens (duplicated per batch pair of cores)
  - Q projection for its 1024 queries
  - attention (4 heads) for its 1024 queries against all 4096 keys
  - output projection + bias + residual for its disjoint (256, 1024) slice
Host unshard = pure concatenation of the 8 disjoint output slices.

Softmax uses a constant shift M0 (softmax is invariant to per-row constant
shifts; a global constant is exact in exact arithmetic and fp32-safe here:
scaled scores lie in [-16.5, 13.3] and the shifted exponentials stay well
inside fp32 range). Row-sums fall out of the attention-value matmul via a
ones-column appended to V; normalization and the V bias are applied after.
"""

import os
import sys

# The grading environment may pin JAX_PLATFORMS=cpu for the reference; the
# bass execution path needs the axon/neuron PJRT devices.
if os.environ.get("JAX_PLATFORMS", "").strip() == "cpu":
    del os.environ["JAX_PLATFORMS"]

for _p in ("/opt/trn_rl_repo",):
    if os.path.isdir(_p) and _p not in sys.path:
        sys.path.insert(0, _p)

import numpy as np

B = 2
C = 256
N = 4096
NQ = 1024  # queries per core
NH = 4
HD = 64
G = 8
EPS = 1e-5
SCALE = HD ** -0.5
M0 = 16.0  # constant softmax shift (in scaled-score units)
N_CORES = 8

_CACHE: dict = {}


def _build(debug_taps=False, reps=1):
    from contextlib import ExitStack

    import concourse.bass as bass
    import concourse.tile as tile
    from concourse import bacc, mybir

    f32 = mybir.dt.float32
    f32r = mybir.dt.float32r
    f16 = mybir.dt.float16
    A = mybir.AluOpType
    AF = mybir.ActivationFunctionType

    nc = bacc.Bacc("TRN2", target_bir_lowering=False, debug=False,
                   num_devices=N_CORES)

    d_xf = nc.dram_tensor("x_full", [C, N], f16, kind="ExternalInput").ap()
    d_xq = nc.dram_tensor("x_q", [C, NQ], f32, kind="ExternalInput").ap()
    d_wq = nc.dram_tensor("wq_t", [C, C], f16, kind="ExternalInput").ap()
    d_wk = nc.dram_tensor("wk_t", [C, C], f16, kind="ExternalInput").ap()
    d_wv = nc.dram_tensor("wv_t", [C, C], f16, kind="ExternalInput").ap()
    d_wp = nc.dram_tensor("wp_t", [C, C], f32r, kind="ExternalInput").ap()
    d_sm = nc.dram_tensor("smalls", [128, 28], f32, kind="ExternalInput").ap()
    d_gmt = nc.dram_tensor("gmask_t", [8, C], f32, kind="ExternalInput").ap()
    d_out = nc.dram_tensor("out", [C, NQ], f32, kind="ExternalOutput").ap()
    dbg = {}
    if debug_taps:
        dbg["K0"] = nc.dram_tensor("dbg_K0", [128, N], f32, kind="ExternalOutput").ap()
        dbg["Q0"] = nc.dram_tensor("dbg_Q0", [128, NQ], f32, kind="ExternalOutput").ap()
        dbg["vt"] = nc.dram_tensor("dbg_vt", [128, 32, NH, HD + 1], f32, kind="ExternalOutput").ap()
        dbg["at00"] = nc.dram_tensor("dbg_at00", [128, 2048], f32, kind="ExternalOutput").ap()
        dbg["hA0"] = nc.dram_tensor("dbg_hA0", [65, NQ], f32, kind="ExternalOutput").ap()
        dbg["rsA0"] = nc.dram_tensor("dbg_rsA0", [1, NQ], f32, kind="ExternalOutput").ap()
        dbg["hn0"] = nc.dram_tensor("dbg_hn0", [128, NQ], f32, kind="ExternalOutput").ap()
        dbg["rb0"] = nc.dram_tensor("dbg_rb0", [128, NQ], f32, kind="ExternalOutput").ap()

    def _av(hA, hB, vt, at, hp, kt):
        nc.tensor.matmul(
            hA, vt[:, kt, 2 * hp, :], at[:, 0:512],
            start=(kt == 0), stop=(kt == 31))
        nc.tensor.matmul(
            hB, vt[:, kt, 2 * hp + 1, :], at[:, 512:1024],
            start=(kt == 0), stop=(kt == 31))

    def body(ctx: ExitStack, tc: tile.TileContext):
        sing = ctx.enter_context(tc.tile_pool(name="sing", bufs=1))
        wk = ctx.enter_context(tc.tile_pool(name="wk", bufs=2))

        # ---------------- loads ----------------
        # DMA order matters (serial HBM bandwidth + ~0.6us HWDGE cost per
        # dma_start): one packed constants transfer, then x (paces the stats
        # chain), then weights in the order the fold needs them.
        sm_sb = sing.tile([128, 28], f32, tag="sm_sb", name="sm_sb")
        nc.sync.dma_start(out=sm_sb, in_=d_sm)
        gmt_sb = sing.tile([8, C], f32, tag="gmt_sb", name="gmt_sb")
        nc.sync.dma_start(out=gmt_sb, in_=d_gmt)
        bq_sb = sm_sb[:, 0:2]
        bk_sb = sm_sb[:, 2:4]
        bv_sb = sm_sb[:, 4:6]
        nw_sb = sm_sb[:, 6:8]
        nb_sb = sm_sb[:, 8:10]
        pb_sb = sm_sb[:, 10:12]
        gm_sb = sm_sb[:, 12:28]

        xf = []
        for h in range(2):
            t = sing.tile([128, N], f16, tag=f"xf{h}", name=f"xf{h}")
            for chk in range(4):
                nc.sync.dma_start(
                    out=t[:, chk * 1024:(chk + 1) * 1024],
                    in_=d_xf[h * 128:(h + 1) * 128, chk * 1024:(chk + 1) * 1024])
            xf.append(t)
        # queries are token-columns 0:1024 of the (host-rotated) x
        xq = [xf[0][:, 0:NQ], xf[1][:, 0:NQ]]

        def load_w(name, dram, dt_):
            t = sing.tile([128, 2, C], dt_, tag=name, name=name)
            nc.sync.dma_start(out=t, in_=dram.rearrange("(c p) o -> p c o", p=128))
            return t

        wq_sb = load_w("wq_sb", d_wq, f16)
        wk_sb = load_w("wk_sb", d_wk, f16)
        wv_sb = load_w("wv_sb", d_wv, f16)
        wp_sb = load_w("wp_sb", d_wp, f32r)
        # fp32 residual slice, only needed at the very end
        xq32 = []
        for h in range(2):
            t = sing.tile([128, NQ], f32, tag=f"xq32_{h}", name=f"xq32_{h}")
            nc.sync.dma_start(out=t, in_=d_xq[h * 128:(h + 1) * 128, :])
            xq32.append(t)

        # V^T tiles, per-head with an appended ones column for row-sums
        vt = sing.tile([128, 32, NH, HD + 1], f32r, tag="vt", name="vt")
        nc.vector.memset(vt[:, :, :, HD:HD + 1].bitcast(f32), 1.0)

        epsc = sing.tile([128, 1], f32, tag="epsc", name="epsc")
        nc.vector.memset(epsc, EPS)
        m0c = sing.tile([128, 1], f32, tag="m0c", name="m0c")
        nc.vector.memset(m0c, -M0)
        ones1 = sing.tile([1, 64], f32r, tag="ones1", name="ones1")
        nc.vector.memset(ones1.bitcast(f32), 1.0)
        # preload the sqrt activation table while ACT is idle
        scratch = sing.tile([128, 1], f32, tag="scratch", name="scratch")
        nc.scalar.activation(scratch, epsc, AF.Sqrt, bias=epsc, scale=1.0)

        K_sb = [sing.tile([128, N], f16, tag=f"K{hp}", name=f"K{hp}")
                for hp in range(2)]
        Q_sb = [sing.tile([128, NQ], f16, tag=f"Qs{hp}", name=f"Qs{hp}")
                for hp in range(2)]
        hnT = [sing.tile([128, NQ], f32r, tag=f"hn{hp}", name=f"hn{hp}")
               for hp in range(2)]

        # ---------------- groupnorm stats -> folded into weights -----------
        ps = ctx.enter_context(tc.tile_pool(name="ps", bufs=1, space="PSUM"))
        if True:
            st_t = []
            for h in range(2):
                stats = wk.tile([128, 8, 6], f32, tag="stats", name=f"stats{h}")
                for sg in range(8):
                    nc.vector.bn_stats(stats[:, sg, :],
                                       xf[h][:, sg * 512:(sg + 1) * 512])
                mv = wk.tile([128, 2], f32, tag="mv", name=f"mv{h}")
                nc.vector.bn_aggr(mv, stats)
                st = wk.tile([128, 2], f32, tag="st", name=f"st{h}")
                nc.vector.tensor_copy(st[:, 0:1], mv[:, 0:1])
                tmp = wk.tile([128, 1], f32, tag="tmp1", name=f"tmp1_{h}")
                nc.vector.tensor_mul(tmp, mv[:, 0:1], mv[:, 0:1])
                nc.vector.tensor_add(st[:, 1:2], mv[:, 1:2], tmp)
                st_t.append(st)

            # per-group mean / rstd via mask matmul over channels
            g_ps = ps.tile([8, 2], f32, tag="work", bufs=3, name="g_ps")
            for h in range(2):
                nc.tensor.matmul(g_ps, gm_sb[:, h * 8:(h + 1) * 8], st_t[h],
                                 start=(h == 0), stop=(h == 1))
            gs2 = wk.tile([8, 2], f32, tag="gs2", name="gs2")
            nc.vector.tensor_scalar_mul(gs2, g_ps, 1.0 / 32.0)
            gt = wk.tile([8, 1], f32, tag="gt", name="gt")
            nc.vector.tensor_mul(gt, gs2[:, 0:1], gs2[:, 0:1])
            vg = wk.tile([8, 1], f32, tag="vg", name="vg")
            nc.vector.tensor_sub(vg, gs2[:, 1:2], gt)
            sq = wk.tile([8, 1], f32, tag="sq", name="sq")
            nc.scalar.activation(sq, vg, AF.Sqrt, bias=epsc[0:8], scale=1.0)
            # preload the exp table set (input dep on sq keeps it after the
            # real sqrt so the table sets load exactly once each)
            nc.scalar.activation(scratch[0:8], sq, AF.Exp, bias=m0c[0:8],
                                 scale=1.0)
            gsb = wk.tile([8, 2], f32, tag="gsb", name="gsb")
            nc.vector.tensor_copy(gsb[:, 0:1], gs2[:, 0:1])
            nc.vector.reciprocal(gsb[:, 1:2], sq)

            # per-channel affine a, beta (per half), as f32r for the fold
            ab = []
            for h in range(2):
                bc_ps = ps.tile([128, 2], f32, tag="work", bufs=3,
                                name=f"bc_ps{h}")
                nc.tensor.matmul(bc_ps, gmt_sb[:, h * 128:(h + 1) * 128], gsb,
                                 start=True, stop=True)
                abt = wk.tile([128, 2], f32r, tag="ab", name=f"ab{h}")
                nc.vector.tensor_mul(abt[:, 0:1], nw_sb[:, h:h + 1], bc_ps[:, 1:2])
                tmp2 = wk.tile([128, 1], f32, tag="tmp2", name=f"tmp2_{h}")
                nc.vector.tensor_mul(tmp2, bc_ps[:, 0:1], abt[:, 0:1].bitcast(f32))
                nc.vector.tensor_sub(abt[:, 1:2], nb_sb[:, h:h + 1], tmp2)
                ab.append(abt)

            # fold first (per weight, in the order the projections need
            # them), then bias corrections b2 = b + W'^T (beta/a) -- using the
            # folded weights keeps the fold off the critical path
            for w_sb in (wq_sb, wk_sb, wv_sb):
                for cc in range(2):
                    nc.vector.tensor_scalar_mul(w_sb[:, cc, :], w_sb[:, cc, :],
                                                ab[cc][:, 0:1].bitcast(f32))
            ba = []
            for cc in range(2):
                tr = wk.tile([128, 1], f32, tag="bar", name=f"bar{cc}")
                nc.vector.reciprocal(tr, ab[cc][:, 0:1].bitcast(f32))
                t = wk.tile([128, 1], f16, tag="ba", name=f"ba{cc}")
                nc.vector.tensor_mul(t, tr, ab[cc][:, 1:2].bitcast(f32))
                ba.append(t)
            b2 = {}
            for wname, w_sb, b_sb in (("q", wq_sb, bq_sb), ("k", wk_sb, bk_sb),
                                      ("v", wv_sb, bv_sb)):
                b2t = wk.tile([128, 2], f32, tag=f"b2{wname}", name=f"b2{wname}",
                              bufs=1)
                for hp in range(2):
                    wb_ps = ps.tile([128, 1], f32, tag="work", bufs=3,
                                    name=f"wb_{wname}{hp}")
                    for cc in range(2):
                        nc.tensor.matmul(
                            wb_ps,
                            w_sb[:, cc, hp * 128:(hp + 1) * 128],
                            ba[cc],
                            start=(cc == 0), stop=(cc == 1))
                    nc.vector.tensor_add(b2t[:, hp:hp + 1], b_sb[:, hp:hp + 1],
                                         wb_ps)
                b2[wname] = b2t
            pb2 = wk.tile([128, 2], f32, tag="pb2", name="pb2", bufs=1)
            for cc in range(2):
                pb_ps = ps.tile([128, 1], f32, tag="work", bufs=3,
                                name=f"pb_ps{cc}")
                for hpp in range(2):
                    nc.tensor.matmul(
                        pb_ps,
                        wp_sb[:, hpp, cc * 128:(cc + 1) * 128].bitcast(f32),
                        b2["v"][:, hpp:hpp + 1],
                        start=(hpp == 0), stop=(hpp == 1))
                nc.vector.tensor_add(pb2[:, cc:cc + 1], pb_sb[:, cc:cc + 1],
                                     pb_ps)

            # ---------------- projections (from raw x, folded weights) -----
            # Q first (scores need it for every key tile)
            for hp in range(2):
                for ch in range(2):
                    pq = ps.tile([128, 512], f32, tag="work", bufs=3,
                                 name=f"pq{hp}_{ch}")
                    for cc in range(2):
                        nc.tensor.matmul(
                            pq,
                            wq_sb[:, cc, hp * 128:(hp + 1) * 128],
                            xq[cc][:, ch * 512:(ch + 1) * 512],
                            start=(cc == 0), stop=(cc == 1))
                    nc.scalar.activation(
                        Q_sb[hp][:, ch * 512:(ch + 1) * 512], pq, AF.Identity,
                        bias=b2["q"][:, hp:hp + 1], scale=1.0)
            def k_chunk(hp, ch, on_act=False):
                pk = ps.tile([128, 512], f32, tag="work", bufs=3,
                             name=f"pk{hp}_{ch}")
                for cc in range(2):
                    nc.tensor.matmul(
                        pk,
                        wk_sb[:, cc, hp * 128:(hp + 1) * 128],
                        xf[cc][:, ch * 512:(ch + 1) * 512],
                        start=(cc == 0), stop=(cc == 1))
                if on_act:
                    nc.scalar.activation(
                        K_sb[hp][:, ch * 512:(ch + 1) * 512], pk, AF.Identity,
                        bias=b2["k"][:, hp:hp + 1], scale=1.0)
                else:
                    nc.vector.tensor_scalar_add(
                        K_sb[hp][:, ch * 512:(ch + 1) * 512], pk,
                        b2["k"][:, hp:hp + 1])

            def v_chunk2(tt0):
                # two token-tiles per psum tile (halves work-slot pressure)
                pv = ps.tile([128, 512], f32, tag="work", bufs=3,
                             name=f"pv{tt0}")
                for j in range(2):
                    tt = tt0 + j
                    for cc in range(2):
                        nc.tensor.matmul(
                            pv[:, j * 256:(j + 1) * 256],
                            xf[cc][:, tt * 128:(tt + 1) * 128],
                            wv_sb[:, cc, :],
                            start=(cc == 0), stop=(cc == 1))
                nc.vector.tensor_copy(
                    vt[:, tt0:tt0 + 2, :, 0:HD],
                    pv.rearrange("p (t h e) -> p t h e", t=2, e=HD))

            k_chunk(0, 0, on_act=True)
            v_chunk2(0)

        # ---------------- attention: 4 phases of (head-pair, query-half) ----
        # h accumulators are (65, 512) = 1 PSUM bank each, leaving the shared
        # "work" tag 3 slots. Phases are software-pipelined: each phase's
        # drain chain is emitted after the next phase's first two score/exp
        # iterations so ACT never waits on the boundary; AV lags two tiles.
        PHASES = [(0, 0), (0, 1), (1, 0), (1, 1)]
        with tc.tile_pool(name="atp", bufs=4) as atp, \
             tc.tile_pool(name="rbp", bufs=1) as rbp:

            def make_drain(hp, qc, hA, hB, at30, at31, last=False):
                def drain():
                    qs = slice(qc * 512, (qc + 1) * 512)
                    _av(hA, hB, vt, at30, hp, 30)
                    _av(hA, hB, vt, at31, hp, 31)
                    rsA = rbp.tile([1, 512], f32r, tag="rsA",
                                   name=f"rsA{hp}{qc}", bufs=1)
                    if last:
                        # ACT is idle after the final exp; copy in parallel
                        nc.scalar.activation(rsA, hA[64:65, :], AF.Copy)
                    else:
                        nc.vector.tensor_copy(rsA, hA[64:65, :])
                    rsB = rbp.tile([1, 512], f32r, tag="rsB",
                                   name=f"rsB{hp}{qc}", bufs=1)
                    nc.vector.tensor_copy(rsB, hB[64:65, :])
                    # broadcast raw rowsums across partitions (K=1 matmul),
                    # then reciprocal over all 128 partitions at once
                    bbA = ps.tile([64, 512], f32, tag="work", bufs=3,
                                  name=f"bbA{hp}{qc}")
                    nc.tensor.matmul(bbA, ones1, rsA, start=True, stop=True)
                    bbB = ps.tile([64, 512], f32, tag="work", bufs=3,
                                  name=f"bbB{hp}{qc}")
                    nc.tensor.matmul(bbB, ones1, rsB, start=True, stop=True)
                    rb = rbp.tile([128, 512], f32, tag="rb",
                                  name=f"rb{hp}{qc}", bufs=1)
                    nc.vector.reciprocal(rb[0:64, :], bbA)
                    nc.vector.reciprocal(rb[64:128, :], bbB)
                    nc.vector.tensor_mul(hnT[hp][0:64, qs], hA[0:64, :],
                                         rb[0:64, :])
                    nc.vector.tensor_mul(hnT[hp][64:128, qs], hB[0:64, :],
                                         rb[64:128, :])
                    if debug_taps and hp == 0 and qc == 1:
                        nc.sync.dma_start(out=dbg["rb0"][:, qs], in_=rb)
                        nc.sync.dma_start(out=dbg["hn0"],
                                          in_=hnT[0].bitcast(f32))
                    return

                def proj_part():
                    qs = slice(qc * 512, (qc + 1) * 512)
                    if hp == 1:
                        for cc in range(2):
                            op = ps.tile([128, 512], f32, tag="work", bufs=3,
                                         name=f"op{cc}_{qc}")
                            for hpp in range(2):
                                nc.tensor.matmul(
                                    op,
                                    wp_sb[:, hpp, cc * 128:(cc + 1) * 128],
                                    hnT[hpp][:, qs],
                                    start=(hpp == 0), stop=(hpp == 1))
                            osb = sing.tile([128, NQ], f32, tag=f"os{cc}",
                                            name=f"os{cc}_{qc}")
                            nc.vector.scalar_tensor_tensor(
                                osb[:, qs], op, pb2[:, cc:cc + 1],
                                xq32[cc][:, qs], A.add, A.add)
                            nc.sync.dma_start(
                                out=d_out[cc * 128:(cc + 1) * 128, qs],
                                in_=osb[:, qs])
                return drain, proj_part

            pending = None
            for hp, qc in PHASES:
                qs = slice(qc * 512, (qc + 1) * 512)
                hA = ps.tile([65, 512], f32, tag="hA", bufs=1,
                             name=f"hA{hp}_{qc}")
                hB = ps.tile([65, 512], f32, tag="hB", bufs=1,
                             name=f"hB{hp}_{qc}")
                ats = {}
                for kt in range(32):
                    at = atp.tile([128, 1024], f32r, tag="at",
                                  name=f"at{hp}_{qc}_{kt}")
                    sc = ps.tile([128, 1024], f32, tag="work", bufs=3,
                                 name=f"sc{hp}_{qc}_{kt}")
                    for sub in range(2):
                        nc.tensor.matmul(
                            sc[:, sub * 512:(sub + 1) * 512],
                            K_sb[hp][sub * 64:(sub + 1) * 64,
                                     kt * 128:(kt + 1) * 128],
                            Q_sb[hp][sub * 64:(sub + 1) * 64, qs],
                            start=True, stop=True)
                    nc.scalar.activation(at, sc, AF.Exp, bias=m0c, scale=SCALE)
                    ats[kt] = at
                    if debug_taps and hp == 0 and qc == 0 and kt == 0:
                        nc.sync.dma_start(out=dbg["at00"][:, 0:1024],
                                          in_=at.bitcast(f32))
                    if kt == 1 and pending is not None:
                        pending[0]()
                    if kt == 7 and pending is not None:
                        pending[1]()
                        pending = None
                    if kt >= 2:
                        _av(hA, hB, vt, ats.pop(kt - 2), hp, kt - 2)
                    # just-in-time projection work rides the ACT-bound loop
                    if hp == 0 and qc == 0:
                        if kt % 2 == 0 and kt < 30:
                            v_chunk2(kt + 2)
                        if kt % 4 == 1 and kt // 4 + 1 <= 7:
                            k_chunk(0, kt // 4 + 1)
                    if hp == 0 and qc == 1 and kt % 4 == 1 and kt // 4 < 8:
                        k_chunk(1, kt // 4)
                pending = make_drain(hp, qc, hA, hB, ats.pop(30),
                                     ats.pop(31), last=(hp, qc) == PHASES[-1])
            pending[0]()
            pending[1]()

        if debug_taps:
            nc.sync.dma_start(out=dbg["K0"], in_=K_sb[0].bitcast(f32))
            nc.sync.dma_start(out=dbg["Q0"], in_=Q_sb[0].bitcast(f32))
            nc.sync.dma_start(out=dbg["vt"], in_=vt.bitcast(f32))

    with tile.TileContext(nc) as tc:
        for _ in range(reps):
            with ExitStack() as ctx:
                body(ctx, tc)
    nc.compile()
    return nc


def _prep_in_maps(inputs: dict) -> list:
    x = np.ascontiguousarray(np.asarray(inputs["x"], dtype=np.float32))
    norm_w = np.asarray(inputs["norm_w"], dtype=np.float32)
    norm_b = np.asarray(inputs["norm_b"], dtype=np.float32)
    qkv_w = np.asarray(inputs["qkv_w"], dtype=np.float32)
    qkv_b = np.asarray(inputs["qkv_b"], dtype=np.float32)
    proj_w = np.asarray(inputs["proj_w"], dtype=np.float32)
    proj_b = np.asarray(inputs["proj_b"], dtype=np.float32)

    xr = x.reshape(B, C, N)
    wq_t = np.ascontiguousarray(qkv_w[0:C].T).astype(np.float16)
    wk_t = np.ascontiguousarray(qkv_w[C:2 * C].T).astype(np.float16)
    wv_t = np.ascontiguousarray(qkv_w[2 * C:3 * C].T).astype(np.float16)
    wp_t = np.ascontiguousarray(proj_w.T)

    sm = np.zeros((128, 28), np.float32)
    sm[:, 0:2] = qkv_b[0:C].reshape(2, 128).T
    sm[:, 2:4] = qkv_b[C:2 * C].reshape(2, 128).T
    sm[:, 4:6] = qkv_b[2 * C:3 * C].reshape(2, 128).T
    sm[:, 6:8] = norm_w.reshape(2, 128).T
    sm[:, 8:10] = norm_b.reshape(2, 128).T
    sm[:, 10:12] = proj_b.reshape(2, 128).T
    cgrp = np.arange(C) // (C // G)
    gm3 = (cgrp.reshape(2, 128)[:, :, None] == np.arange(8)[None, None, :])
    sm[:, 12:28] = gm3.transpose(1, 0, 2).reshape(128, 16).astype(np.float32)
    gmask_t = np.ascontiguousarray(
        (np.arange(8)[:, None] == cgrp[None, :]).astype(np.float32))

    shared = dict(wq_t=wq_t, wk_t=wk_t, wv_t=wv_t, wp_t=wp_t,
                  smalls=sm, gmask_t=gmask_t)
    in_maps = []
    for core in range(N_CORES):
        b = core // 4
        qo = (core % 4) * NQ
        m = dict(shared)
        # rotate tokens so this core's queries sit at columns 0:NQ --
        # attention is permutation-equivariant over keys, so this is exact
        xrot = np.ascontiguousarray(np.roll(xr[b], -qo, axis=1))
        m["x_full"] = xrot.astype(np.float16)
        m["x_q"] = np.ascontiguousarray(xrot[:, 0:NQ])
        in_maps.append(m)
    return in_maps


def kernel(**inputs) -> np.ndarray:
    from concourse.bass_utils import run_bass_kernel_spmd

    if "nc" not in _CACHE:
        _CACHE["nc"] = _build()
    nc = _CACHE["nc"]

    in_maps = _prep_in_maps(inputs)
    res = run_bass_kernel_spmd(nc, in_maps, core_ids=list(range(N_CORES)))

    out = np.empty((B, C, N), dtype=np.float32)
    for core in range(N_CORES):
        b = core // 4
        qo = (core % 4) * NQ
        out[b][:, qo:qo + NQ] = res.results[core]["out"]
    return out.reshape(B, C, 16, 16, 16)
